# revision 3
# baseline (speedup 1.0000x reference)
"""Trainium2 Bass kernel for nn_CompatibilityModel (embedding_lookup + MLP + training-mode BN).

Strategy
--------
Data parallel over 8 NeuronCores: each core owns B/8 = 131072 rows.

Math restructuring (host-side, exact):
  * The 50-dim encoded feature x is a fixed linear map of u = [one-hot(breed1,15),
    one-hot(size1,3), one-hot(energy1,3), one-hot(temp1,9), (same for pet2),
    age1/15, social1, weight1/100, age2/15, social2, weight2]  (66 dims).
    So z1 := x @ W1 = u @ W1eff with W1eff = A2 @ W1 folded on host.  The
    embedding gathers become part of one K=66 matmul against one-hot rows.
  * Training-mode BN of y = z + b over the batch: (y - mu_y) = (z - mu_z), so the
    linear bias cancels and BN+ReLU is h = relu(a * z + c) with a = gamma/sqrt(var_z+eps),
    c = beta - a*mu_z. a/c are per-feature scalars -> one ScalarE activation op.
  * Layer-1 batch stats (mu_z1, var_z1) are *exact* linear/bilinear functionals of
    the input one-hot second moments: E[z1] = W1eff^T E[u], E[z1^2]_j = w_j^T E[uu^T] w_j.
    E[u], E[uu^T] are computed on host in float64 from joint histograms (bincount).
  * Layer-2/3 stats need the full nonlinear forward, so the device computes
    pre-BN activations z2 (resp z3), spills them to DRAM, and emits per-shard
    (mean, var) partials via bn_stats/bn_aggr.  Host combines shard stats
    exactly (equal shard sizes) and launches the next stage.

Device pipeline (per core, 128 chunks x 1024 rows):
  Launch A: DMA idx streams (as f32) -> PE broadcast-matmul (K=8 selector) ->
            DVE is_equal vs iota -> one-hot U[60] (+ numerics DMA'd into U[60:66])
            -> PE z1 = W1eff^T u -> ScalarE relu(a1*z1+c1) -> PE z2 = W2^T h1
            -> DVE bn_stats partials; ScalarE copy z2 -> DRAM spill.
  Launch B: load z2 -> ScalarE relu(a2*z2+c2) -> PE z3 = W3^T h2 -> stats + spill.
  Launch C: load z3 -> ScalarE relu(a3*z3+c3) -> PE z4 = W4^T h3 -> ScalarE
            sigmoid(z4+b4) -> DMA out.
"""

import json

import numpy as np

import concourse.bass as bass
import concourse.mybir as mybir
import concourse.bass_utils as _bass_utils
import concourse.bass2jax as _bass2jax
from concourse.bass_utils import run_bass_kernel_spmd
from concourse.tile import TileContext


# --------------------------------------------------------------------------- wait splitting
# This walrus build rejects instructions carrying more than one semaphore
# wait ("Too many sync wait commands").  Tile routinely emits 2-3 waits per
# instruction, so split the extras onto standalone EventSemaphore
# instructions placed immediately before, on the same engine.
def _split_multi_waits(bir_json: bytes) -> bytes:
    m = json.loads(bir_json)
    n_split = 0
    for f in m.get("functions", []):
        for bb in f.get("blocks", []):
            out = []
            for ins in bb.get("instructions", []):
                si = ins.get("sync_info") or {}
                ow = si.get("on_wait") or []
                if len(ow) > 1:
                    for k, w in enumerate(ow[:-1]):
                        out.append({
                            "name": f"{ins['name']}-wsplit{k}",
                            "opcode": "EventSemaphore",
                            "engine": ins["engine"],
                            "ins": [],
                            "outs": [],
                            "sync_info": {"on_update": [], "on_wait": [w]},
                        })
                        n_split += 1
                    si["on_wait"] = [ow[-1]]
                out.append(ins)
            bb["instructions"] = out
    return json.dumps(m).encode()


_orig_compile_bir_kernel = _bass_utils.compile_bir_kernel


def _patched_compile_bir_kernel(bir_json, tmpdir, neff_name="file.neff"):
    return _orig_compile_bir_kernel(_split_multi_waits(bir_json), tmpdir, neff_name)


_bass_utils.compile_bir_kernel = _patched_compile_bir_kernel
_bass2jax.compile_bir_kernel = _patched_compile_bir_kernel

F32 = mybir.dt.float32
AF = mybir.ActivationFunctionType
OP = mybir.AluOpType

B = 1 << 20
N_CORES = 8
SHARD = B // N_CORES           # 131072 rows per core
FD = 1024                      # rows per chunk (free dim)
NCH = SHARD // FD              # 128 chunks
MM = 512                       # fp32 matmul max moving free dim

EMB = 8
N_BREEDS, N_TEMPS = 15, 9
CAT_SIZES = [N_BREEDS, 3, 3, N_TEMPS] * 2          # 8 categorical streams
CAT_OFFS = np.concatenate([[0], np.cumsum(CAT_SIZES)]).astype(int)  # [0,15,18,21,30,45,48,51,60]
NCAT = int(CAT_OFFS[-1])       # 60
NU = 66                        # one-hot dims + 6 numerics
H1, H2, H3 = 128, 64, 32
EPS = 1e-5

_cache = {}


# ----------------------------------------------------------------------------- host math
def _build_w1eff(breed_emb, temp_emb, W1):
    """A2 @ W1 in float64; A2 maps u (66) -> x (50)."""
    A2 = np.zeros((NU, 50), np.float64)
    be = np.asarray(breed_emb, np.float64)
    te = np.asarray(temp_emb, np.float64)
    A2[0:15, 0:8] = be
    A2[15:18, 8:11] = np.eye(3)
    A2[18:21, 11:14] = np.eye(3)
    A2[21:30, 14:22] = te
    A2[30:45, 25:33] = be
    A2[45:48, 33:36] = np.eye(3)
    A2[48:51, 36:39] = np.eye(3)
    A2[51:60, 39:47] = te
    A2[60, 22] = 1.0   # age1/15
    A2[61, 23] = 1.0   # social1
    A2[62, 24] = 1.0   # weight1/100
    A2[63, 47] = 1.0
    A2[64, 48] = 1.0
    A2[65, 49] = 1.0
    return A2 @ np.asarray(W1, np.float64)


def _host_stats1(cats, nums, W1eff):
    """Exact E[z1], Var[z1] via E[u], E[uu^T] in float64 (z1 = u @ W1eff)."""
    n = cats[0].shape[0]
    cats = [c.astype(np.int64) for c in cats]
    M = np.zeros((NU, NU), np.float64)
    Eu = np.zeros(NU, np.float64)
    for i, ci in enumerate(cats):
        Ki, oi = CAT_SIZES[i], CAT_OFFS[i]
        pi = np.bincount(ci, minlength=Ki) / n
        Eu[oi:oi + Ki] = pi
        M[oi:oi + Ki, oi:oi + Ki] = np.diag(pi)
        for j in range(i):
            Kj, oj = CAT_SIZES[j], CAT_OFFS[j]
            joint = np.bincount(ci * Kj + cats[j],
                                minlength=Ki * Kj).reshape(Ki, Kj) / n
            M[oi:oi + Ki, oj:oj + Kj] = joint
            M[oj:oj + Kj, oi:oi + Ki] = joint.T
        for j, xj in enumerate(nums):
            s = np.bincount(ci, weights=xj, minlength=Ki) / n
            M[oi:oi + Ki, NCAT + j] = s
            M[NCAT + j, oi:oi + Ki] = s
    for i, xi in enumerate(nums):
        Eu[NCAT + i] = xi.mean(dtype=np.float64)
        for j, xj in enumerate(nums):
            if j <= i:
                v = np.dot(xi, xj) / n
                M[NCAT + i, NCAT + j] = v
                M[NCAT + j, NCAT + i] = v
    Ez = W1eff.T @ Eu
    Ez2 = np.sum(W1eff * (M @ W1eff), axis=0)
    return Ez, Ez2 - Ez * Ez


def _combine_shard_stats(mv_list):
    """mv_list: per-core [F, 2] (mean, biased var) with equal counts -> global."""
    m = np.stack([mv[:, 0] for mv in mv_list]).astype(np.float64)
    v = np.stack([mv[:, 1] for mv in mv_list]).astype(np.float64)
    mu = m.mean(0)
    var = (v + m * m).mean(0) - mu * mu
    return mu, var


def _bn_affine(gamma, beta, mu, var):
    a = np.asarray(gamma, np.float64) / np.sqrt(var + EPS)
    c = np.asarray(beta, np.float64) - a * mu
    return a.astype(np.float32), c.astype(np.float32)


# ----------------------------------------------------------------------------- bass programs
def _mm_tiles(nc, out, lhsT, rhs, fd=FD):
    """fp32 matmul in 512-column slices."""
    for j in range(0, fd, MM):
        nc.tensor.matmul(out[:, j:j + MM], lhsT, rhs[:, j:j + MM],
                         start=True, stop=True)


def _bn_stats_acc(nc, acc, src, ci, fd=FD):
    """bn_stats of [P, fd] PSUM into acc[:, ci*k:(ci+1)*k, :] (k = fd/512 sub-ops)."""
    k = fd // MM
    for j in range(k):
        nc.vector.bn_stats(out=acc[:, ci * k + j, :],
                           in_=src[:, j * MM:(j + 1) * MM])


# const blob column layout for launch A: [128, CBW]
CB_SSEL = 0            # [8, 60]   cols 0:60
CB_W1C = 60            # [60, 128] cols 60:188
CB_W1N = 188           # [6, 128]  cols 188:316
CB_W2 = 316            # [128, 64] cols 316:380
CB_IOTA = 380          # [60, 1]   col 380
CB_A1 = 381            # [128, 1]  col 381
CB_C1 = 382            # [128, 1]  col 382
CBW = 383


def build_launch_A(nch=NCH, fd=FD):
    nc = bass.Bass()
    inpk = nc.dram_tensor("inpk", [nch, 14, fd], F32, kind="ExternalInput")
    cblob = nc.dram_tensor("cblob", [H1, CBW], F32, kind="ExternalInput")
    z2sp = nc.dram_tensor("z2sp", [nch, H2, fd], F32, kind="ExternalOutput")
    st2 = nc.dram_tensor("st2", [H2, 2], F32, kind="ExternalOutput")

    with TileContext(nc) as tc:
        with (
            tc.tile_pool(name="consts", bufs=1) as consts,
            tc.tile_pool(name="io", bufs=3) as io,
            tc.tile_pool(name="work", bufs=2) as work,
            tc.tile_pool(name="spill", bufs=3) as spillp,
            tc.tile_pool(name="acc", bufs=1) as accp,
            tc.tile_pool(name="psW", bufs=1, space="PSUM") as psW,
            tc.tile_pool(name="psB", bufs=1, space="PSUM") as psB,
            tc.tile_pool(name="psZ1", bufs=1, space="PSUM") as psZ1,
            tc.tile_pool(name="psZ2", bufs=1, space="PSUM") as psZ2,
        ):
            cb = consts.tile([H1, CBW], F32)
            nc.sync.dma_start(out=cb, in_=cblob[:, :])
            s_t = cb[0:8, CB_SSEL:CB_SSEL + NCAT]
            w1c_t = cb[0:NCAT, CB_W1C:CB_W1C + H1]
            w1n_t = cb[0:NU - NCAT, CB_W1N:CB_W1N + H1]
            w2_t = cb[0:H1, CB_W2:CB_W2 + H2]
            iota_t = cb[0:NCAT, CB_IOTA:CB_IOTA + 1]
            a1_t = cb[0:H1, CB_A1:CB_A1 + 1]
            c1_t = cb[0:H1, CB_C1:CB_C1 + 1]

            # warm-ups: absorb the const-blob DMA wait into each engine's clock
            ps_w = psW.tile([1, 1], F32)
            nc.tensor.matmul(ps_w, cb[0:1, 0:1], cb[0:1, 0:1], start=True, stop=True)
            scr_v = consts.tile([1, 1], F32)
            nc.vector.tensor_copy(scr_v, cb[0:1, 0:1])
            scr_s = consts.tile([1, 1], F32)
            nc.scalar.copy(scr_s, cb[0:1, 0:1])

            stats2 = accp.tile([H2, nch * (fd // MM), 6], F32)

            for i in range(nch):
                idx_t = io.tile([8, fd], F32, tag="idx")
                nc.sync.dma_start(out=idx_t, in_=inpk[i, 0:8, :])
                num_t = io.tile([6, fd], F32, tag="num")
                nc.sync.dma_start(out=num_t, in_=inpk[i, 8:14, :])
                # broadcast each idx stream across its one-hot partition range
                ps_b = psB.tile([NCAT, fd], F32)
                _mm_tiles(nc, ps_b, s_t, idx_t[:, :], fd)
                u_t = work.tile([NCAT, fd], F32, tag="u")
                nc.vector.tensor_scalar(u_t[:, :], ps_b[:, :],
                                        iota_t, None, OP.is_equal)
                ps_z1 = psZ1.tile([H1, fd], F32)
                for j in range(0, fd, MM):
                    nc.tensor.matmul(ps_z1[:, j:j + MM], w1c_t, u_t[:, j:j + MM],
                                     start=True, stop=False)
                    nc.tensor.matmul(ps_z1[:, j:j + MM], w1n_t, num_t[:, j:j + MM],
                                     start=False, stop=True)
                h1_t = work.tile([H1, fd], F32, tag="h1")
                nc.scalar.activation(out=h1_t, in_=ps_z1[:, :], func=AF.Relu,
                                     bias=c1_t, scale=a1_t)
                ps_z2 = psZ2.tile([H2, fd], F32)
                _mm_tiles(nc, ps_z2, w2_t, h1_t, fd)
                _bn_stats_acc(nc, stats2, ps_z2, i, fd)
                z2_t = spillp.tile([H2, fd], F32, tag="z2")
                nc.scalar.copy(out=z2_t, in_=ps_z2[:, :])
                nc.sync.dma_start(out=z2sp[i], in_=z2_t)

            mv2 = accp.tile([H2, 2], F32)
            nc.vector.bn_aggr(out=mv2, in_=stats2[:, :, :])
            nc.sync.dma_start(out=st2[:, :], in_=mv2)
    return nc


def build_launch_B(nch=NCH, fd=FD):
    # const blob [64, 34]: w3 cols 0:32, a2 col 32, c2 col 33
    nc = bass.Bass()
    z2sp = nc.dram_tensor("z2sp", [nch, H2, fd], F32, kind="ExternalInput")
    cblob = nc.dram_tensor("cblob", [H2, H3 + 2], F32, kind="ExternalInput")
    z3sp = nc.dram_tensor("z3sp", [nch, H3, fd], F32, kind="ExternalOutput")
    st3 = nc.dram_tensor("st3", [H3, 2], F32, kind="ExternalOutput")

    with TileContext(nc) as tc:
        with (
            tc.tile_pool(name="consts", bufs=1) as consts,
            tc.tile_pool(name="io", bufs=3) as io,
            tc.tile_pool(name="work", bufs=2) as work,
            tc.tile_pool(name="spill", bufs=3) as spillp,
            tc.tile_pool(name="acc", bufs=1) as accp,
            tc.tile_pool(name="psW", bufs=1, space="PSUM") as psW,
            tc.tile_pool(name="psZ3", bufs=2, space="PSUM") as psZ3,
        ):
            cb = consts.tile([H2, H3 + 2], F32)
            nc.sync.dma_start(out=cb, in_=cblob[:, :])
            w3_t = cb[0:H2, 0:H3]
            a2_t = cb[0:H2, H3:H3 + 1]
            c2_t = cb[0:H2, H3 + 1:H3 + 2]

            ps_w = psW.tile([1, 1], F32)
            nc.tensor.matmul(ps_w, cb[0:1, 0:1], cb[0:1, 0:1], start=True, stop=True)
            scr_v = consts.tile([1, 1], F32)
            nc.vector.tensor_copy(scr_v, cb[0:1, 0:1])
            scr_s = consts.tile([1, 1], F32)
            nc.scalar.copy(scr_s, cb[0:1, 0:1])

            stats3 = accp.tile([H3, nch * (fd // MM), 6], F32)

            for i in range(nch):
                z2_t = io.tile([H2, fd], F32)
                nc.sync.dma_start(out=z2_t, in_=z2sp[i])
                h2_t = work.tile([H2, fd], F32, tag="h2")
                nc.scalar.activation(out=h2_t, in_=z2_t[:, :], func=AF.Relu,
                                     bias=c2_t, scale=a2_t)
                ps_z3 = psZ3.tile([H3, fd], F32)
                _mm_tiles(nc, ps_z3, w3_t, h2_t, fd)
                _bn_stats_acc(nc, stats3, ps_z3, i, fd)
                z3_t = spillp.tile([H3, fd], F32, tag="z3")
                nc.scalar.copy(out=z3_t, in_=ps_z3[:, :])
                nc.sync.dma_start(out=z3sp[i], in_=z3_t)

            mv3 = accp.tile([H3, 2], F32)
            nc.vector.bn_aggr(out=mv3, in_=stats3[:, :, :])
            nc.sync.dma_start(out=st3[:, :], in_=mv3)
    return nc


def build_launch_C(nch=NCH, fd=FD):
    # const blob [32, 4]: w4 col 0, a3 col 1, c3 col 2, b4 at [0, 3]
    nc = bass.Bass()
    z3sp = nc.dram_tensor("z3sp", [nch, H3, fd], F32, kind="ExternalInput")
    cblob = nc.dram_tensor("cblob", [H3, 4], F32, kind="ExternalInput")
    outy = nc.dram_tensor("outy", [nch, fd], F32, kind="ExternalOutput")

    with TileContext(nc) as tc:
        with (
            tc.tile_pool(name="consts", bufs=1) as consts,
            tc.tile_pool(name="io", bufs=3) as io,
            tc.tile_pool(name="work", bufs=2) as work,
            tc.tile_pool(name="spill", bufs=3) as spillp,
            tc.tile_pool(name="psW", bufs=1, space="PSUM") as psW,
            tc.tile_pool(name="psZ4", bufs=2, space="PSUM") as psZ4,
        ):
            cb = consts.tile([H3, 4], F32)
            nc.sync.dma_start(out=cb, in_=cblob[:, :])
            w4_t = cb[0:H3, 0:1]
            a3_t = cb[0:H3, 1:2]
            c3_t = cb[0:H3, 2:3]
            b4_t = cb[0:1, 3:4]

            ps_w = psW.tile([1, 1], F32)
            nc.tensor.matmul(ps_w, cb[0:1, 0:1], cb[0:1, 0:1], start=True, stop=True)
            scr_v = consts.tile([1, 1], F32)
            nc.vector.tensor_copy(scr_v, cb[0:1, 0:1])
            scr_s = consts.tile([1, 1], F32)
            nc.scalar.copy(scr_s, cb[0:1, 0:1])

            for i in range(nch):
                z3_t = io.tile([H3, fd], F32)
                nc.sync.dma_start(out=z3_t, in_=z3sp[i])
                h3_t = work.tile([H3, fd], F32, tag="h3")
                nc.scalar.activation(out=h3_t, in_=z3_t[:, :], func=AF.Relu,
                                     bias=c3_t, scale=a3_t)
                ps_z4 = psZ4.tile([1, fd], F32)
                _mm_tiles(nc, ps_z4, w4_t, h3_t, fd)
                y_t = spillp.tile([1, fd], F32, tag="y")
                nc.scalar.activation(out=y_t, in_=ps_z4[:, :], func=AF.Sigmoid,
                                     bias=b4_t, scale=1.0)
                nc.sync.dma_start(out=outy[i], in_=y_t)
    return nc


# ----------------------------------------------------------------------------- driver
def _get_programs():
    if "A" not in _cache:
        _cache["A"] = build_launch_A()
        _cache["B"] = build_launch_B()
        _cache["C"] = build_launch_C()
    return _cache["A"], _cache["B"], _cache["C"]


import os


def _run(prog, in_maps, cores, label):
    tr = bool(os.environ.get("BASS_KERNEL_TRACE"))
    r = run_bass_kernel_spmd(prog, in_maps, cores, trace=tr)
    if tr and r.exec_time_ns:
        _cache["hw_exec_ns"] = _cache.get("hw_exec_ns", 0) + r.exec_time_ns
        _cache[f"ns_{label}"] = r.exec_time_ns
        if r.instructions_and_trace:
            _cache[f"trace_{label}"] = r.instructions_and_trace[1]
    return r.results


def kernel(**inputs):
    inp = {k: np.asarray(v) for k, v in inputs.items()}
    cores = list(range(N_CORES))
    _cache.pop("hw_exec_ns", None)

    W1eff64 = _build_w1eff(inp["breed_emb"], inp["temp_emb"], inp["W1"])

    cats_all = [inp["pet1_breed"], inp["pet1_size"], inp["pet1_energy"], inp["pet1_temp"],
                inp["pet2_breed"], inp["pet2_size"], inp["pet2_energy"], inp["pet2_temp"]]
    nums_all = [inp["pet1_age"] / 15.0, inp["pet1_social"], inp["pet1_weight"] / 100.0,
                inp["pet2_age"] / 15.0, inp["pet2_social"], inp["pet2_weight"] / 100.0]
    nums_all = [np.asarray(x, np.float32) for x in nums_all]

    # exact layer-1 batch stats on host (float64)
    mu1, var1 = _host_stats1(cats_all, [x.astype(np.float64) for x in nums_all], W1eff64)
    a1, c1 = _bn_affine(inp["gamma1"], inp["beta1"], mu1, var1)

    # selector matrix & iota for the on-device one-hot build
    s_sel = np.zeros((8, NCAT), np.float32)
    iota = np.zeros((NCAT, 1), np.float32)
    for i in range(8):
        o, k = CAT_OFFS[i], CAT_SIZES[i]
        s_sel[i, o:o + k] = 1.0
        iota[o:o + k, 0] = np.arange(k, dtype=np.float32)

    progA, progB, progC = _get_programs()

    # ---- launch A
    cbA = np.zeros((H1, CBW), np.float32)
    cbA[0:8, CB_SSEL:CB_SSEL + NCAT] = s_sel
    cbA[0:NCAT, CB_W1C:CB_W1C + H1] = W1eff64[0:NCAT].astype(np.float32)
    cbA[0:NU - NCAT, CB_W1N:CB_W1N + H1] = W1eff64[NCAT:NU].astype(np.float32)
    cbA[0:H1, CB_W2:CB_W2 + H2] = np.asarray(inp["W2"], np.float32)
    cbA[0:NCAT, CB_IOTA] = iota[:, 0]
    cbA[0:H1, CB_A1] = a1
    cbA[0:H1, CB_C1] = c1

    in_maps = []
    for c in cores:
        sl = slice(c * SHARD, (c + 1) * SHARD)
        streams = [cat[sl].astype(np.float32).reshape(NCH, FD) for cat in cats_all] + \
                  [x[sl].reshape(NCH, FD) for x in nums_all]
        inpk = np.stack(streams, axis=1)                    # [NCH, 14, FD]
        in_maps.append({"inpk": np.ascontiguousarray(inpk), "cblob": cbA})
    resA = _run(progA, in_maps, cores, "A")

    mu2, var2 = _combine_shard_stats([r["st2"] for r in resA])
    a2, c2 = _bn_affine(inp["gamma2"], inp["beta2"], mu2, var2)

    # ---- launch B
    cbB = np.zeros((H2, H3 + 2), np.float32)
    cbB[:, 0:H3] = np.asarray(inp["W3"], np.float32)
    cbB[:, H3] = a2
    cbB[:, H3 + 1] = c2
    in_maps = [{"z2sp": resA[c]["z2sp"], "cblob": cbB} for c in cores]
    resB = _run(progB, in_maps, cores, "B")

    mu3, var3 = _combine_shard_stats([r["st3"] for r in resB])
    a3, c3 = _bn_affine(inp["gamma3"], inp["beta3"], mu3, var3)

    # ---- launch C
    cbC = np.zeros((H3, 4), np.float32)
    cbC[:, 0] = np.asarray(inp["W4"], np.float32)[:, 0]
    cbC[:, 1] = a3
    cbC[:, 2] = c3
    cbC[0, 3] = float(np.asarray(inp["b4"]).reshape(-1)[0])
    in_maps = [{"z3sp": resB[c]["z3sp"], "cblob": cbC} for c in cores]
    resC = _run(progC, in_maps, cores, "C")

    return np.concatenate([resC[c]["outy"].reshape(SHARD) for c in cores])



# revision 4
# speedup vs baseline: 1.1719x; 1.1719x over previous
"""Trainium2 fused single-launch kernel for nn_CompatibilityModel.

Data parallel over 8 cores, one NEFF launch, per-shard BN stats for
layers 2/3 (layer-1 stats exact on host via joint histograms).

Per core (131072 rows = 128 chunks x 1024):
  pass1: cat DMA [8,2048]/pair -> PE broadcast matmul -> DVE is_equal
         one-hot u[0:60] (numerics DMA'd to u[60:66]) -> PE z1 (K=66)
         -> ScalarE relu(a1*z1+c1) -> bf16 h1 -> PE z2 (pair-packed
         PSUM [128,1024]) -> ScalarE copy -> Z2 resident bf16
         [128,65536] -> DVE bn_stats.
  stats2 on device: bn_aggr + rowgroup combine -> a2=gamma/sigma,
         coa2=c2/a2; a2 folded into W3 (relu(a*z+c)=a*relu(z+c/a)).
  pass2: gpsimd in-place h2' = max(z2+coa2, 0) on Z2 -> PE z3
         (block-diag W3p, 4 slices/PSUM tile) -> DVE bn_stats3.
  stats3 combine -> a3, coa3; a3 folded into W4.
  pass3: PE z3 recomputed from resident h2' (slice-pairs packed
         [128,512] PSUM) -> ScalarE relu(z3+coa3) -> PE z4 (block-diag
         W4p, [4,2048] PSUM) -> ScalarE sigmoid(+b4) -> DMA out.

All matmuls bf16 (fp32 is 4 cycles/row on PE, bf16 is 1).
"""

import json
import os

import numpy as np
import ml_dtypes

import concourse.bass as bass
import concourse.mybir as mybir
import concourse.bass_utils as _bass_utils
import concourse.bass2jax as _bass2jax
from concourse.bass_utils import run_bass_kernel_spmd
from concourse.tile import TileContext

BF = ml_dtypes.bfloat16


# --------------------------------------------------------------------------- wait splitting
# This walrus build rejects instructions carrying more than one semaphore
# wait; split extras onto standalone EventSemaphore instructions.
def _split_multi_waits(bir_json: bytes) -> bytes:
    m = json.loads(bir_json)
    for f in m.get("functions", []):
        for bb in f.get("blocks", []):
            out = []
            for ins in bb.get("instructions", []):
                si = ins.get("sync_info") or {}
                ow = si.get("on_wait") or []
                if len(ow) > 1:
                    for k, w in enumerate(ow[:-1]):
                        out.append({
                            "name": f"{ins['name']}-wsplit{k}",
                            "opcode": "EventSemaphore",
                            "engine": ins["engine"],
                            "ins": [],
                            "outs": [],
                            "sync_info": {"on_update": [], "on_wait": [w]},
                        })
                    si["on_wait"] = [ow[-1]]
                out.append(ins)
            bb["instructions"] = out
    return json.dumps(m).encode()


_orig_compile_bir_kernel = _bass_utils.compile_bir_kernel


def _patched_compile_bir_kernel(bir_json, tmpdir, neff_name="file.neff"):
    return _orig_compile_bir_kernel(_split_multi_waits(bir_json), tmpdir, neff_name)


_bass_utils.compile_bir_kernel = _patched_compile_bir_kernel
_bass2jax.compile_bir_kernel = _patched_compile_bir_kernel

F32 = mybir.dt.float32
BF16 = mybir.dt.bfloat16
AF = mybir.ActivationFunctionType
OP = mybir.AluOpType

B = 1 << 20
N_CORES = 8
SHARD = B // N_CORES           # 131072
FD = 1024                      # rows per chunk
NCH = SHARD // FD              # 128
NPAIR = NCH // 2               # 64
Z2C = SHARD // 2               # 65536 Z2 columns
NSLICE = Z2C // 512            # 128
YC = SHARD // 4                # 32768 output columns

EMB = 8
N_BREEDS, N_TEMPS = 15, 9
CAT_SIZES = [N_BREEDS, 3, 3, N_TEMPS] * 2
CAT_OFFS = np.concatenate([[0], np.cumsum(CAT_SIZES)]).astype(int)
NCAT = int(CAT_OFFS[-1])       # 60
NU = 66
H1, H2, H3 = 128, 64, 32
EPS = 1e-5

# f32 const blob columns: [128, CF]
CF_IOTA = 0     # [60,1]
CF_A1 = 1       # [128,1]
CF_C1 = 2       # [128,1]
CF_G2 = 3       # [64,1]
CF_BT2 = 4      # [64,1]
CF_G3 = 5       # [32,1]
CF_BT3 = 6      # [32,1]
CF_EPS = 7      # [64,1]
CF_B4 = 8       # [4,1]
CF_W = 9

# bf16 const blob columns: [128, CBW]
CB_W1 = 0       # [66,128]
CB_W2 = 128     # [128,64]
CB_W3 = 192     # [128,64]  block-diag W3
CB_W4 = 256     # [128,4]   block-diag w4
CB_SSEL = 260   # [8,60]
CBW = 320

_cache = {}


# ----------------------------------------------------------------------------- host math
def _build_w1eff(breed_emb, temp_emb, W1):
    A2 = np.zeros((NU, 50), np.float64)
    be = np.asarray(breed_emb, np.float64)
    te = np.asarray(temp_emb, np.float64)
    A2[0:15, 0:8] = be
    A2[15:18, 8:11] = np.eye(3)
    A2[18:21, 11:14] = np.eye(3)
    A2[21:30, 14:22] = te
    A2[30:45, 25:33] = be
    A2[45:48, 33:36] = np.eye(3)
    A2[48:51, 36:39] = np.eye(3)
    A2[51:60, 39:47] = te
    A2[60, 22] = 1.0
    A2[61, 23] = 1.0
    A2[62, 24] = 1.0
    A2[63, 47] = 1.0
    A2[64, 48] = 1.0
    A2[65, 49] = 1.0
    return A2 @ np.asarray(W1, np.float64)


def _host_stats1(cats, nums, W1eff):
    """Exact E[z1], Var[z1] via E[u], E[uu^T] in float64."""
    n = cats[0].shape[0]
    cats = [c.astype(np.int64) for c in cats]
    M = np.zeros((NU, NU), np.float64)
    Eu = np.zeros(NU, np.float64)
    for i, ci in enumerate(cats):
        Ki, oi = CAT_SIZES[i], CAT_OFFS[i]
        pi = np.bincount(ci, minlength=Ki) / n
        Eu[oi:oi + Ki] = pi
        M[oi:oi + Ki, oi:oi + Ki] = np.diag(pi)
        for j in range(i):
            Kj, oj = CAT_SIZES[j], CAT_OFFS[j]
            joint = np.bincount(ci * Kj + cats[j],
                                minlength=Ki * Kj).reshape(Ki, Kj) / n
            M[oi:oi + Ki, oj:oj + Kj] = joint
            M[oj:oj + Kj, oi:oi + Ki] = joint.T
        for j, xj in enumerate(nums):
            s = np.bincount(ci, weights=xj, minlength=Ki) / n
            M[oi:oi + Ki, NCAT + j] = s
            M[NCAT + j, oi:oi + Ki] = s
    for i, xi in enumerate(nums):
        Eu[NCAT + i] = xi.mean(dtype=np.float64)
        for j, xj in enumerate(nums):
            if j <= i:
                v = np.dot(xi, xj) / n
                M[NCAT + i, NCAT + j] = v
                M[NCAT + j, NCAT + i] = v
    Ez = W1eff.T @ Eu
    Ez2 = np.sum(W1eff * (M @ W1eff), axis=0)
    return Ez, Ez2 - Ez * Ez


def _out_perm():
    """row r of a shard -> flat index into y_dev [4, YC]."""
    r = np.arange(SHARD)
    c = r // FD
    q = r % FD
    x = (c // 2) * FD + q          # Z2 column
    j = x // 512                   # slice
    sx = x % 512
    t = j // 2                     # slice pair
    sp = j % 2
    g = 2 * sp + (c % 2)
    return g * YC + t * 512 + sx


# ----------------------------------------------------------------------------- program
_SMALL_N = [0]


def _small_f32(pool, n):
    _SMALL_N[0] += 1
    return pool.tile([n, 1], F32, name=f"sm{_SMALL_N[0]}")


def build_fused():
    nc = bass.Bass()
    catk = nc.dram_tensor("catk", [8, SHARD], BF16, kind="ExternalInput")
    numk = nc.dram_tensor("numk", [6, SHARD], BF16, kind="ExternalInput")
    cbf = nc.dram_tensor("cbf", [H1, CF_W], F32, kind="ExternalInput")
    cbb = nc.dram_tensor("cbb", [H1, CBW], BF16, kind="ExternalInput")
    yout = nc.dram_tensor("yout", [4, YC], F32, kind="ExternalOutput")

    with TileContext(nc) as tc:
        with (
            tc.tile_pool(name="consts", bufs=1) as consts,
            tc.tile_pool(name="z2res", bufs=1) as z2res,
            tc.tile_pool(name="accp", bufs=1) as accp,
        ):
            cf = consts.tile([H1, CF_W], F32)
            nc.sync.dma_start(out=cf, in_=cbf[:, :])
            cb = consts.tile([H1, CBW], BF16)
            nc.scalar.dma_start(out=cb, in_=cbb[:, :])

            iota_t = cf[0:NCAT, CF_IOTA:CF_IOTA + 1]
            a1_t = cf[0:H1, CF_A1:CF_A1 + 1]
            c1_t = cf[0:H1, CF_C1:CF_C1 + 1]
            g2_t = cf[0:H2, CF_G2:CF_G2 + 1]
            bt2_t = cf[0:H2, CF_BT2:CF_BT2 + 1]
            g3_t = cf[0:H3, CF_G3:CF_G3 + 1]
            bt3_t = cf[0:H3, CF_BT3:CF_BT3 + 1]
            eps_t = cf[0:H2, CF_EPS:CF_EPS + 1]
            b4_t = cf[0:4, CF_B4:CF_B4 + 1]

            w1_t = cb[0:NU, CB_W1:CB_W1 + H1]
            w2_t = cb[0:H1, CB_W2:CB_W2 + H2]
            w3_t = cb[0:H1, CB_W3:CB_W3 + H2]
            w4_t = cb[0:H1, CB_W4:CB_W4 + 4]
            ssel_t = cb[0:8, CB_SSEL:CB_SSEL + NCAT]

            # warmups: absorb const DMA wait into each engine's clock
            with tc.tile_pool(name="warm", bufs=1, space="PSUM") as warmp:
                ps_w = warmp.tile([1, 1], F32)
                nc.tensor.matmul(ps_w, cb[0:1, 0:1], cb[0:1, 0:1],
                                 start=True, stop=True)
            scr_v = consts.tile([1, 1], F32)
            nc.vector.tensor_copy(scr_v, cf[0:1, 0:1])
            scr_s = consts.tile([1, 1], F32)
            nc.scalar.copy(scr_s, cf[0:1, 0:1])
            scr_g = consts.tile([1, 1], BF16)
            nc.gpsimd.tensor_copy(scr_g, cb[0:1, 0:1])

            Z2 = z2res.tile([H1, Z2C], BF16)
            acc2 = accp.tile([H1, NCH, 6], F32)

            dummy_n = [0]

            def pe_keepalive(psd_pool, n=2):
                for _ in range(n):
                    dummy_n[0] += 1
                    psd = psd_pool.tile([1, 512], F32, name=f"dum{dummy_n[0]}",
                                        tag="dum")
                    nc.tensor.matmul(psd[:, :], cb[0:1, 0:1], Z2[0:1, 0:512],
                                     start=True, stop=True, skip_group_check=True)

            # ---------------- pass 1 (blocks of 2 pairs; matmuls grouped
            # by weight so LDWEIGHTS amortizes: alternating weights costs
            # ~538ns/mm vs 249ns same-weight)
            with (
                tc.tile_pool(name="catp", bufs=4) as catp,
                tc.tile_pool(name="up", bufs=4) as up,
                tc.tile_pool(name="h1p", bufs=4) as h1p,
                tc.tile_pool(name="psb", bufs=2, space="PSUM") as psb,
                tc.tile_pool(name="psz1", bufs=2, space="PSUM") as psz1,
                tc.tile_pool(name="psz2", bufs=2, space="PSUM") as psz2,
            ):
                for blk in range(NPAIR // 2):
                    prs = [2 * blk, 2 * blk + 1]
                    u_ts = []
                    for p in prs:
                        pc = p * 2048
                        cat_t = catp.tile([8, 2048], BF16, tag="cat",
                                          name=f"cat{p}")
                        nc.sync.dma_start(out=cat_t, in_=catk[:, pc:pc + 2048])
                        u_t = up.tile([NU, 2048], BF16, tag="u", name=f"u{p}")
                        nc.sync.dma_start(out=u_t[NCAT:NU, :],
                                          in_=numk[:, pc:pc + 2048])
                        for sl in range(4):
                            pzb = psb.tile([NCAT, 512], F32, tag="b",
                                           name=f"b{p}_{sl}")
                            nc.tensor.matmul(pzb[:, :], ssel_t,
                                             cat_t[:, sl * 512:(sl + 1) * 512],
                                             start=True, stop=True)
                            nc.vector.tensor_scalar(
                                u_t[0:NCAT, sl * 512:(sl + 1) * 512],
                                pzb[:, :], iota_t, None, OP.is_equal)
                        u_ts.append(u_t)
                    h1_ts = []
                    for p, u_t in zip(prs, u_ts):
                        h1_t = h1p.tile([H1, 2048], BF16, tag="h1",
                                        name=f"h1{p}")
                        for h in range(2):
                            pz1 = psz1.tile([H1, FD], F32, tag="z1",
                                            name=f"z1{p}_{h}")
                            for j in range(0, FD, 512):
                                nc.tensor.matmul(
                                    pz1[:, j:j + 512], w1_t,
                                    u_t[:, h * FD + j:h * FD + j + 512],
                                    start=True, stop=True)
                            nc.scalar.activation(
                                out=h1_t[:, h * FD:(h + 1) * FD],
                                in_=pz1[:, :], func=AF.Relu, bias=c1_t,
                                scale=a1_t)
                        h1_ts.append(h1_t)
                    for p, h1_t in zip(prs, h1_ts):
                        for j in range(2):
                            pz2 = psz2.tile([H1, 512], F32, tag="z2",
                                            name=f"z2_{p}_{j}")
                            for h in range(2):
                                nc.tensor.matmul(
                                    pz2[h * H2:h * H2 + H2, :], w2_t,
                                    h1_t[:, h * FD + j * 512:h * FD + (j + 1) * 512],
                                    start=True, stop=True)
                            zc = slice(p * FD + j * 512, p * FD + (j + 1) * 512)
                            nc.scalar.copy(out=Z2[:, zc], in_=pz2[:, :])
                            nc.vector.bn_stats(out=acc2[:, 2 * p + j, :],
                                               in_=Z2[:, zc])

            # ---------------- stats2 -> a2, coa2, W3p
            sm = accp
            mv2 = sm.tile([H1, 2], F32)
            nc.vector.bn_aggr(out=mv2, in_=acc2[:, :, :])

            sq2 = _small_f32(sm, H1)
            nc.vector.tensor_tensor(out=sq2, in0=mv2[:, 0:1], in1=mv2[:, 0:1],
                                    op=OP.mult)
            st2 = sm.tile([H1, 2], F32)
            nc.vector.tensor_copy(st2[:, 0:1], mv2[:, 0:1])
            nc.vector.tensor_tensor(out=st2[:, 1:2], in0=mv2[:, 1:2], in1=sq2,
                                    op=OP.add)
            hi2 = sm.tile([H2, 2], F32)
            nc.sync.dma_start(out=hi2, in_=st2[H2:H1, :])
            cm2 = sm.tile([H2, 2], F32)
            nc.vector.tensor_tensor(out=cm2, in0=st2[0:H2, :], in1=hi2[:, :],
                                    op=OP.add)
            nc.vector.tensor_scalar(cm2[:, :], cm2[:, :], 0.5, None, OP.mult)
            msq2 = _small_f32(sm, H2)
            nc.vector.tensor_tensor(out=msq2, in0=cm2[:, 0:1], in1=cm2[:, 0:1],
                                    op=OP.mult)
            var2 = _small_f32(sm, H2)
            nc.vector.tensor_tensor(out=var2, in0=cm2[:, 1:2], in1=msq2,
                                    op=OP.subtract)
            sd2 = _small_f32(sm, H2)
            nc.scalar.activation(out=sd2, in_=var2[:, :], func=AF.Sqrt,
                                 bias=eps_t, scale=1.0)
            inv2 = _small_f32(sm, H2)
            nc.vector.reciprocal(out=inv2, in_=sd2[:, :])
            a2v = _small_f32(sm, H2)
            nc.vector.tensor_tensor(out=a2v, in0=inv2[:, :], in1=g2_t,
                                    op=OP.mult)
            ra2 = _small_f32(sm, H2)
            nc.vector.reciprocal(out=ra2, in_=a2v[:, :])
            tb2 = _small_f32(sm, H2)
            nc.vector.tensor_tensor(out=tb2, in0=bt2_t, in1=ra2[:, :],
                                    op=OP.mult)
            coa2 = _small_f32(sm, H2)
            nc.vector.tensor_tensor(out=coa2, in0=tb2[:, :], in1=cm2[:, 0:1],
                                    op=OP.subtract)
            a2rep = _small_f32(sm, H1)
            nc.vector.tensor_copy(a2rep[0:H2, :], a2v[:, :])
            nc.sync.dma_start(out=a2rep[H2:H1, :], in_=a2v[:, :])
            coa2rep = _small_f32(sm, H1)
            nc.vector.tensor_copy(coa2rep[0:H2, :], coa2[:, :])
            nc.sync.dma_start(out=coa2rep[H2:H1, :], in_=coa2[:, :])
            w3p = sm.tile([H1, H2], BF16)
            nc.vector.tensor_scalar(w3p[:, :], w3_t, a2rep, None, OP.mult)

            acc3 = accp.tile([H2, NSLICE, 6], F32)

            # ---------------- pass 2: in-place act2 on Z2, z3, stats3
            with (
                tc.tile_pool(name="psz3", bufs=2, space="PSUM") as psz3,
            ):
                for t in range(NSLICE // 4):
                    pz3 = psz3.tile([H2, 2048], F32, tag="z3")
                    for kk in range(2):
                        a2s = slice((4 * t + 2 * kk) * 512,
                                    (4 * t + 2 * kk + 2) * 512)
                        nc.scalar.activation(out=Z2[:, a2s], in_=Z2[:, a2s],
                                             func=AF.Relu, bias=coa2rep,
                                             scale=1.0)
                    for k in range(4):
                        zs = slice((4 * t + k) * 512, (4 * t + k + 1) * 512)
                        nc.tensor.matmul(pz3[:, k * 512:(k + 1) * 512],
                                         w3p[:, :], Z2[:, zs],
                                         start=True, stop=True)
                    for k in range(4):
                        nc.vector.bn_stats(out=acc3[:, 4 * t + k, :],
                                           in_=pz3[:, k * 512:(k + 1) * 512])

            # ---------------- stats3 -> a3, coa3, W4p
            mv3 = sm.tile([H2, 2], F32)
            nc.vector.bn_aggr(out=mv3, in_=acc3[:, :, :])
            sq3 = _small_f32(sm, H2)
            nc.vector.tensor_tensor(out=sq3, in0=mv3[:, 0:1], in1=mv3[:, 0:1],
                                    op=OP.mult)
            st3 = sm.tile([H2, 2], F32)
            nc.vector.tensor_copy(st3[:, 0:1], mv3[:, 0:1])
            nc.vector.tensor_tensor(out=st3[:, 1:2], in0=mv3[:, 1:2], in1=sq3,
                                    op=OP.add)
            hi3 = sm.tile([H3, 2], F32)
            nc.sync.dma_start(out=hi3, in_=st3[H3:H2, :])
            cm3 = sm.tile([H3, 2], F32)
            nc.vector.tensor_tensor(out=cm3, in0=st3[0:H3, :], in1=hi3[:, :],
                                    op=OP.add)
            nc.vector.tensor_scalar(cm3[:, :], cm3[:, :], 0.5, None, OP.mult)
            msq3 = _small_f32(sm, H3)
            nc.vector.tensor_tensor(out=msq3, in0=cm3[:, 0:1], in1=cm3[:, 0:1],
                                    op=OP.mult)
            var3 = _small_f32(sm, H3)
            nc.vector.tensor_tensor(out=var3, in0=cm3[:, 1:2], in1=msq3,
                                    op=OP.subtract)
            sd3 = _small_f32(sm, H3)
            nc.scalar.activation(out=sd3, in_=var3[:, :], func=AF.Sqrt,
                                 bias=eps_t[0:H3, :], scale=1.0)
            inv3 = _small_f32(sm, H3)
            nc.vector.reciprocal(out=inv3, in_=sd3[:, :])
            a3v = _small_f32(sm, H3)
            nc.vector.tensor_tensor(out=a3v, in0=inv3[:, :], in1=g3_t,
                                    op=OP.mult)
            ra3 = _small_f32(sm, H3)
            nc.vector.reciprocal(out=ra3, in_=a3v[:, :])
            tb3 = _small_f32(sm, H3)
            nc.vector.tensor_tensor(out=tb3, in0=bt3_t, in1=ra3[:, :],
                                    op=OP.mult)
            coa3 = _small_f32(sm, H3)
            nc.vector.tensor_tensor(out=coa3, in0=tb3[:, :], in1=cm3[:, 0:1],
                                    op=OP.subtract)
            a3rep = _small_f32(sm, H1)
            nc.vector.tensor_copy(a3rep[0:H3, :], a3v[:, :])
            for g in range(1, 4):
                nc.sync.dma_start(out=a3rep[g * H3:(g + 1) * H3, :],
                                  in_=a3v[:, :])
            coa3rep = _small_f32(sm, H1)
            nc.vector.tensor_copy(coa3rep[0:H3, :], coa3[:, :])
            for g in range(1, 4):
                nc.sync.dma_start(out=coa3rep[g * H3:(g + 1) * H3, :],
                                  in_=coa3[:, :])
            w4p = sm.tile([H1, 4], BF16)
            nc.vector.tensor_scalar(w4p[:, :], w4_t, a3rep, None, OP.mult)

            # ---------------- pass 3: recompute z3, act3, z4, sigmoid, out
            with (
                tc.tile_pool(name="h3p", bufs=2) as h3p,
                tc.tile_pool(name="yp", bufs=2) as yp,
                tc.tile_pool(name="psz34", bufs=2, space="PSUM") as psz34,
                tc.tile_pool(name="psz4", bufs=2, space="PSUM") as psz4,
            ):
                for t4 in range(NSLICE // 4):
                    pz4 = psz4.tile([4, 1024], F32, tag="z4")
                    for t in range(t4 * 2, t4 * 2 + 2):
                        pz34 = psz34.tile([H1, 512], F32, tag="z34")
                        for sp in range(2):
                            s = 2 * t + sp
                            zs = slice(s * 512, (s + 1) * 512)
                            nc.tensor.matmul(pz34[sp * H2:(sp + 1) * H2, :],
                                             w3p[:, :], Z2[:, zs],
                                             start=True, stop=True)
                        h3_t = h3p.tile([H1, 512], BF16, tag="h3")
                        nc.scalar.activation(out=h3_t, in_=pz34[:, :],
                                             func=AF.Relu, bias=coa3rep,
                                             scale=1.0)
                        k = t % 2
                        nc.tensor.matmul(pz4[:, k * 512:(k + 1) * 512],
                                         w4p[:, :], h3_t[:, :],
                                         start=True, stop=True)
                    y_t = yp.tile([4, 1024], F32, tag="y")
                    nc.scalar.activation(out=y_t, in_=pz4[:, :],
                                         func=AF.Sigmoid, bias=b4_t, scale=1.0)
                    nc.scalar.dma_start(out=yout[:, t4 * 1024:(t4 + 1) * 1024],
                                          in_=y_t)
    return nc


def _get_program():
    if "F" not in _cache:
        _cache["F"] = build_fused()
    return _cache["F"]


def _run(prog, in_maps, cores, label):
    tr = bool(os.environ.get("BASS_KERNEL_TRACE"))
    r = run_bass_kernel_spmd(prog, in_maps, cores, trace=tr)
    if tr and r.exec_time_ns:
        _cache["hw_exec_ns"] = _cache.get("hw_exec_ns", 0) + r.exec_time_ns
        _cache[f"ns_{label}"] = r.exec_time_ns
        if r.instructions_and_trace:
            _cache[f"trace_{label}"] = r.instructions_and_trace[1]
    return r.results


def kernel(**inputs):
    inp = {k: np.asarray(v) for k, v in inputs.items()}
    cores = list(range(N_CORES))
    _cache.pop("hw_exec_ns", None)

    W1eff64 = _build_w1eff(inp["breed_emb"], inp["temp_emb"], inp["W1"])

    cats_all = [inp["pet1_breed"], inp["pet1_size"], inp["pet1_energy"],
                inp["pet1_temp"], inp["pet2_breed"], inp["pet2_size"],
                inp["pet2_energy"], inp["pet2_temp"]]
    nums_all = [inp["pet1_age"] / 15.0, inp["pet1_social"],
                inp["pet1_weight"] / 100.0, inp["pet2_age"] / 15.0,
                inp["pet2_social"], inp["pet2_weight"] / 100.0]
    nums_all = [np.asarray(x, np.float32) for x in nums_all]

    mu1, var1 = _host_stats1(cats_all, [x.astype(np.float64) for x in nums_all],
                             W1eff64)
    a1 = (np.asarray(inp["gamma1"], np.float64) / np.sqrt(var1 + EPS))
    c1 = (np.asarray(inp["beta1"], np.float64) - a1 * mu1)

    s_sel = np.zeros((8, NCAT), np.float32)
    iota = np.zeros((NCAT, 1), np.float32)
    for i in range(8):
        o, k = CAT_OFFS[i], CAT_SIZES[i]
        s_sel[i, o:o + k] = 1.0
        iota[o:o + k, 0] = np.arange(k, dtype=np.float32)

    cbf = np.zeros((H1, CF_W), np.float32)
    cbf[0:NCAT, CF_IOTA] = iota[:, 0]
    cbf[0:H1, CF_A1] = a1.astype(np.float32)
    cbf[0:H1, CF_C1] = c1.astype(np.float32)
    cbf[0:H2, CF_G2] = np.asarray(inp["gamma2"], np.float32)
    cbf[0:H2, CF_BT2] = np.asarray(inp["beta2"], np.float32)
    cbf[0:H3, CF_G3] = np.asarray(inp["gamma3"], np.float32)
    cbf[0:H3, CF_BT3] = np.asarray(inp["beta3"], np.float32)
    cbf[0:H2, CF_EPS] = EPS
    cbf[0:4, CF_B4] = float(np.asarray(inp["b4"]).reshape(-1)[0])

    W3blk = np.zeros((H1, H2), np.float32)
    W3 = np.asarray(inp["W3"], np.float32)
    W3blk[0:H2, 0:H3] = W3
    W3blk[H2:H1, H3:H2] = W3
    W4blk = np.zeros((H1, 4), np.float32)
    w4 = np.asarray(inp["W4"], np.float32)[:, 0]
    for g in range(4):
        W4blk[g * H3:(g + 1) * H3, g] = w4

    cbb = np.zeros((H1, CBW), np.float32)
    cbb[0:NU, CB_W1:CB_W1 + H1] = W1eff64.astype(np.float32)
    cbb[0:H1, CB_W2:CB_W2 + H2] = np.asarray(inp["W2"], np.float32)
    cbb[0:H1, CB_W3:CB_W3 + H2] = W3blk
    cbb[0:H1, CB_W4:CB_W4 + 4] = W4blk
    cbb[0:8, CB_SSEL:CB_SSEL + NCAT] = s_sel
    cbb = cbb.astype(BF)

    prog = _get_program()

    cat_mat = np.stack(cats_all).astype(BF)       # [8, B]
    num_mat = np.stack(nums_all).astype(BF)       # [6, B]

    in_maps = []
    for c in cores:
        sl = slice(c * SHARD, (c + 1) * SHARD)
        in_maps.append({
            "catk": np.ascontiguousarray(cat_mat[:, sl]),
            "numk": np.ascontiguousarray(num_mat[:, sl]),
            "cbf": cbf, "cbb": cbb,
        })
    res = _run(prog, in_maps, cores, "F")

    perm = _cache.get("perm")
    if perm is None:
        perm = _out_perm()
        _cache["perm"] = perm
    out = np.empty(B, np.float32)
    for c in cores:
        out[c * SHARD:(c + 1) * SHARD] = res[c]["yout"].reshape(-1)[perm]
    return out


# revision 5
# speedup vs baseline: 1.2717x; 1.0852x over previous
"""Trainium2 fused single-launch kernel for nn_CompatibilityModel.

Data parallel over 8 cores, one NEFF launch, per-shard BN stats for
layers 2/3 (layer-1 stats exact on host via joint histograms).

Per core (131072 rows = 128 chunks x 1024):
  pass1: cat DMA [8,2048]/pair -> PE broadcast matmul -> DVE is_equal
         one-hot u[0:60] (numerics DMA'd to u[60:66]) -> PE z1 (K=66)
         -> ScalarE relu(a1*z1+c1) -> bf16 h1 -> PE z2 (pair-packed
         PSUM [128,1024]) -> ScalarE copy -> Z2 resident bf16
         [128,65536] -> DVE bn_stats.
  stats2 on device: bn_aggr + rowgroup combine -> a2=gamma/sigma,
         coa2=c2/a2; a2 folded into W3 (relu(a*z+c)=a*relu(z+c/a)).
  pass2: gpsimd in-place h2' = max(z2+coa2, 0) on Z2 -> PE z3
         (block-diag W3p, 4 slices/PSUM tile) -> DVE bn_stats3.
  stats3 combine -> a3, coa3; a3 folded into W4.
  pass3: PE z3 recomputed from resident h2' (slice-pairs packed
         [128,512] PSUM) -> ScalarE relu(z3+coa3) -> PE z4 (block-diag
         W4p, [4,2048] PSUM) -> ScalarE sigmoid(+b4) -> DMA out.

All matmuls bf16 (fp32 is 4 cycles/row on PE, bf16 is 1).
"""

import json
import os

import numpy as np
import ml_dtypes

import concourse.bass as bass
import concourse.mybir as mybir
import concourse.bass_utils as _bass_utils
import concourse.bass2jax as _bass2jax
from concourse.bass_utils import run_bass_kernel_spmd
from concourse.tile import TileContext

BF = ml_dtypes.bfloat16


# --------------------------------------------------------------------------- wait splitting
# This walrus build rejects instructions carrying more than one semaphore
# wait; split extras onto standalone EventSemaphore instructions.
def _split_multi_waits(bir_json: bytes) -> bytes:
    m = json.loads(bir_json)
    for f in m.get("functions", []):
        for bb in f.get("blocks", []):
            out = []
            for ins in bb.get("instructions", []):
                si = ins.get("sync_info") or {}
                ow = si.get("on_wait") or []
                if len(ow) > 1:
                    for k, w in enumerate(ow[:-1]):
                        out.append({
                            "name": f"{ins['name']}-wsplit{k}",
                            "opcode": "EventSemaphore",
                            "engine": ins["engine"],
                            "ins": [],
                            "outs": [],
                            "sync_info": {"on_update": [], "on_wait": [w]},
                        })
                    si["on_wait"] = [ow[-1]]
                out.append(ins)
            bb["instructions"] = out
    return json.dumps(m).encode()


_orig_compile_bir_kernel = _bass_utils.compile_bir_kernel


def _patched_compile_bir_kernel(bir_json, tmpdir, neff_name="file.neff"):
    return _orig_compile_bir_kernel(_split_multi_waits(bir_json), tmpdir, neff_name)


_bass_utils.compile_bir_kernel = _patched_compile_bir_kernel
_bass2jax.compile_bir_kernel = _patched_compile_bir_kernel

F32 = mybir.dt.float32
BF16 = mybir.dt.bfloat16
AF = mybir.ActivationFunctionType
OP = mybir.AluOpType

B = 1 << 20
N_CORES = 8
SHARD = B // N_CORES           # 131072
FD = 1024                      # rows per chunk
NCH = SHARD // FD              # 128
NPAIR = NCH // 2               # 64
Z2C = SHARD // 2               # 65536 Z2 columns
NSLICE = Z2C // 512            # 128
YC = SHARD // 4                # 32768 output columns

EMB = 8
N_BREEDS, N_TEMPS = 15, 9
CAT_SIZES = [N_BREEDS, 3, 3, N_TEMPS] * 2
CAT_OFFS = np.concatenate([[0], np.cumsum(CAT_SIZES)]).astype(int)
NCAT = int(CAT_OFFS[-1])       # 60
NU = 66
H1, H2, H3 = 128, 64, 32
EPS = 1e-5

# f32 const blob columns: [128, CF]
CF_IOTA = 0     # [60,1]
CF_A1 = 1       # [128,1]
CF_C1 = 2       # [128,1]
CF_G2 = 3       # [64,1]
CF_BT2 = 4      # [64,1]
CF_G3 = 5       # [32,1]
CF_BT3 = 6      # [32,1]
CF_EPS = 7      # [64,1]
CF_B4 = 8       # [4,1]
CF_W = 9

# bf16 const blob columns: [128, CBW]
CB_W1 = 0       # [58,128]  recentered cat dims + numerics
CB_W2 = 128     # [128,64]
CB_W3 = 192     # [128,64]  block-diag W3
CB_W4 = 256     # [128,4]   block-diag w4
CB_SSEL = 260   # [16,116]  packed-pair selector
CBW = 380

# packed one-hot: stream s keeps K_s-1 dims (last dim recentered into bias)
CAT2_SIZES = [k - 1 for k in CAT_SIZES]            # [14,2,2,8]*2
CAT2_OFFS = np.concatenate([[0], np.cumsum(CAT2_SIZES)]).astype(int)
NCAT2 = int(CAT2_OFFS[-1])     # 52
NU2 = NCAT2 + 6                # 58 per chunk
ODD = 64                       # odd-chunk partition base in packed tiles

_cache = {}


# ----------------------------------------------------------------------------- host math
def _build_w1eff(breed_emb, temp_emb, W1):
    A2 = np.zeros((NU, 50), np.float64)
    be = np.asarray(breed_emb, np.float64)
    te = np.asarray(temp_emb, np.float64)
    A2[0:15, 0:8] = be
    A2[15:18, 8:11] = np.eye(3)
    A2[18:21, 11:14] = np.eye(3)
    A2[21:30, 14:22] = te
    A2[30:45, 25:33] = be
    A2[45:48, 33:36] = np.eye(3)
    A2[48:51, 36:39] = np.eye(3)
    A2[51:60, 39:47] = te
    A2[60, 22] = 1.0
    A2[61, 23] = 1.0
    A2[62, 24] = 1.0
    A2[63, 47] = 1.0
    A2[64, 48] = 1.0
    A2[65, 49] = 1.0
    return A2 @ np.asarray(W1, np.float64)


def _host_stats1(cats, nums, W1eff):
    """Exact E[z1], Var[z1] via E[u], E[uu^T] in float64."""
    n = cats[0].shape[0]
    cats = [c.astype(np.int64) for c in cats]
    M = np.zeros((NU, NU), np.float64)
    Eu = np.zeros(NU, np.float64)
    for i, ci in enumerate(cats):
        Ki, oi = CAT_SIZES[i], CAT_OFFS[i]
        pi = np.bincount(ci, minlength=Ki) / n
        Eu[oi:oi + Ki] = pi
        M[oi:oi + Ki, oi:oi + Ki] = np.diag(pi)
        for j in range(i):
            Kj, oj = CAT_SIZES[j], CAT_OFFS[j]
            joint = np.bincount(ci * Kj + cats[j],
                                minlength=Ki * Kj).reshape(Ki, Kj) / n
            M[oi:oi + Ki, oj:oj + Kj] = joint
            M[oj:oj + Kj, oi:oi + Ki] = joint.T
        for j, xj in enumerate(nums):
            s = np.bincount(ci, weights=xj, minlength=Ki) / n
            M[oi:oi + Ki, NCAT + j] = s
            M[NCAT + j, oi:oi + Ki] = s
    for i, xi in enumerate(nums):
        Eu[NCAT + i] = xi.mean(dtype=np.float64)
        for j, xj in enumerate(nums):
            if j <= i:
                v = np.dot(xi, xj) / n
                M[NCAT + i, NCAT + j] = v
                M[NCAT + j, NCAT + i] = v
    Ez = W1eff.T @ Eu
    Ez2 = np.sum(W1eff * (M @ W1eff), axis=0)
    return Ez, Ez2 - Ez * Ez


def _out_perm():
    """row r of a shard -> flat index into y_dev [4, YC]."""
    r = np.arange(SHARD)
    c = r // FD
    q = r % FD
    x = (c // 2) * FD + q          # Z2 column
    j = x // 512                   # slice
    sx = x % 512
    t = j // 2                     # slice pair
    sp = j % 2
    g = 2 * sp + (c % 2)
    return g * YC + t * 512 + sx


# ----------------------------------------------------------------------------- program
_SMALL_N = [0]


def _small_f32(pool, n):
    _SMALL_N[0] += 1
    return pool.tile([n, 1], F32, name=f"sm{_SMALL_N[0]}")


def build_fused():
    nc = bass.Bass()
    catk = nc.dram_tensor("catk", [16, SHARD // 2], BF16, kind="ExternalInput")
    numk = nc.dram_tensor("numk", [12, SHARD // 2], BF16, kind="ExternalInput")
    cbf = nc.dram_tensor("cbf", [H1, CF_W], F32, kind="ExternalInput")
    cbb = nc.dram_tensor("cbb", [H1, CBW], BF16, kind="ExternalInput")
    yout = nc.dram_tensor("yout", [4, YC], F32, kind="ExternalOutput")

    with TileContext(nc) as tc:
        with (
            tc.tile_pool(name="consts", bufs=1) as consts,
            tc.tile_pool(name="z2res", bufs=1) as z2res,
            tc.tile_pool(name="accp", bufs=1) as accp,
        ):
            cf = consts.tile([H1, CF_W], F32)
            nc.sync.dma_start(out=cf, in_=cbf[:, :])
            cb = consts.tile([H1, CBW], BF16)
            nc.scalar.dma_start(out=cb, in_=cbb[:, :])

            iota_t = cf[0:116, CF_IOTA:CF_IOTA + 1]
            a1_t = cf[0:H1, CF_A1:CF_A1 + 1]
            c1_t = cf[0:H1, CF_C1:CF_C1 + 1]
            g2_t = cf[0:H2, CF_G2:CF_G2 + 1]
            bt2_t = cf[0:H2, CF_BT2:CF_BT2 + 1]
            g3_t = cf[0:H3, CF_G3:CF_G3 + 1]
            bt3_t = cf[0:H3, CF_BT3:CF_BT3 + 1]
            eps_t = cf[0:H2, CF_EPS:CF_EPS + 1]
            b4_t = cf[0:4, CF_B4:CF_B4 + 1]

            w1_t = cb[0:NU2, CB_W1:CB_W1 + H1]
            w1o_t = cb[ODD:ODD + NU2, CB_W1:CB_W1 + H1]
            w2_t = cb[0:H1, CB_W2:CB_W2 + H2]
            w3_t = cb[0:H1, CB_W3:CB_W3 + H2]
            w4_t = cb[0:H1, CB_W4:CB_W4 + 4]
            ssel_t = cb[0:16, CB_SSEL:CB_SSEL + 116]

            # warmups: absorb const DMA wait into each engine's clock
            with tc.tile_pool(name="warm", bufs=1, space="PSUM") as warmp:
                ps_w = warmp.tile([1, 1], F32)
                nc.tensor.matmul(ps_w, cb[0:1, 0:1], cb[0:1, 0:1],
                                 start=True, stop=True)
            scr_v = consts.tile([1, 1], F32)
            nc.vector.tensor_copy(scr_v, cf[0:1, 0:1])
            scr_s = consts.tile([1, 1], F32)
            nc.scalar.copy(scr_s, cf[0:1, 0:1])
            scr_g = consts.tile([1, 1], BF16)
            nc.gpsimd.tensor_copy(scr_g, cb[0:1, 0:1])

            Z2 = z2res.tile([H1, Z2C], BF16)
            acc2 = accp.tile([H1, NCH, 6], F32)

            dummy_n = [0]

            def pe_keepalive(psd_pool, n=2):
                for _ in range(n):
                    dummy_n[0] += 1
                    psd = psd_pool.tile([1, 512], F32, name=f"dum{dummy_n[0]}",
                                        tag="dum")
                    nc.tensor.matmul(psd[:, :], cb[0:1, 0:1], Z2[0:1, 0:512],
                                     start=True, stop=True, skip_group_check=True)

            # ---------------- pass 1: packed pairs (even chunk at
            # partitions 0:58, odd at 64:122 of one u tile; one broadcast
            # matmul + one is_equal covers both chunks)
            with (
                tc.tile_pool(name="catp", bufs=4) as catp,
                tc.tile_pool(name="up", bufs=3) as up,
                tc.tile_pool(name="h1p", bufs=3) as h1p,
                tc.tile_pool(name="psb", bufs=2, space="PSUM") as psb,
                tc.tile_pool(name="psz1", bufs=2, space="PSUM") as psz1,
                tc.tile_pool(name="psz2", bufs=2, space="PSUM") as psz2,
            ):
                for p in range(NPAIR):
                    pc = p * FD
                    cat_t = catp.tile([16, FD], BF16, tag="cat")
                    nc.sync.dma_start(out=cat_t, in_=catk[:, pc:pc + FD])
                    u_t = up.tile([122, FD], BF16, tag="u")
                    for sl in range(2):
                        pzb = psb.tile([116, 512], F32, tag="b")
                        nc.tensor.matmul(pzb[:, :], ssel_t,
                                         cat_t[:, sl * 512:(sl + 1) * 512],
                                         start=True, stop=True)
                        nc.vector.tensor_scalar(
                            u_t[0:116, sl * 512:(sl + 1) * 512],
                            pzb[:, :], iota_t, None, OP.is_equal)
                    # numerics overwrite the is_equal zeros at [52:58]/[116:122]
                    nc.sync.dma_start(out=u_t[NCAT2:NU2, :],
                                      in_=numk[0:6, pc:pc + FD])
                    nc.sync.dma_start(out=u_t[ODD + NCAT2:ODD + NU2, :],
                                      in_=numk[6:12, pc:pc + FD])
                    pz2s = [psz2.tile([H1, 512], F32, tag="z2",
                                      name=f"z2_{p}_{j}") for j in range(2)]
                    for h in range(2):
                        pz1 = psz1.tile([H1, FD], F32, tag="z1")
                        for j in range(0, FD, 512):
                            nc.tensor.matmul(
                                pz1[:, j:j + 512], w1o_t if h else w1_t,
                                u_t[ODD * h:ODD * h + NU2, j:j + 512],
                                start=True, stop=True)
                        h1_t = h1p.tile([H1, FD], BF16, tag="h1")
                        nc.scalar.activation(out=h1_t, in_=pz1[:, :],
                                             func=AF.Relu, bias=c1_t,
                                             scale=a1_t)
                        for j in range(2):
                            nc.tensor.matmul(
                                pz2s[j][h * H2:h * H2 + H2, :], w2_t,
                                h1_t[:, j * 512:(j + 1) * 512],
                                start=True, stop=True)
                    for j in range(2):
                        zc = slice(p * FD + j * 512, p * FD + (j + 1) * 512)
                        nc.scalar.copy(out=Z2[:, zc], in_=pz2s[j][:, :])
                        nc.vector.bn_stats(out=acc2[:, 2 * p + j, :],
                                           in_=Z2[:, zc])
            # ---------------- stats2 -> a2, coa2, W3p
            sm = accp
            mv2 = sm.tile([H1, 2], F32)
            nc.vector.bn_aggr(out=mv2, in_=acc2[:, :, :])

            sq2 = _small_f32(sm, H1)
            nc.vector.tensor_tensor(out=sq2, in0=mv2[:, 0:1], in1=mv2[:, 0:1],
                                    op=OP.mult)
            st2 = sm.tile([H1, 2], F32)
            nc.vector.tensor_copy(st2[:, 0:1], mv2[:, 0:1])
            nc.vector.tensor_tensor(out=st2[:, 1:2], in0=mv2[:, 1:2], in1=sq2,
                                    op=OP.add)
            hi2 = sm.tile([H2, 2], F32)
            nc.sync.dma_start(out=hi2, in_=st2[H2:H1, :])
            cm2 = sm.tile([H2, 2], F32)
            nc.vector.tensor_tensor(out=cm2, in0=st2[0:H2, :], in1=hi2[:, :],
                                    op=OP.add)
            nc.vector.tensor_scalar(cm2[:, :], cm2[:, :], 0.5, None, OP.mult)
            msq2 = _small_f32(sm, H2)
            nc.vector.tensor_tensor(out=msq2, in0=cm2[:, 0:1], in1=cm2[:, 0:1],
                                    op=OP.mult)
            var2 = _small_f32(sm, H2)
            nc.vector.tensor_tensor(out=var2, in0=cm2[:, 1:2], in1=msq2,
                                    op=OP.subtract)
            sd2 = _small_f32(sm, H2)
            nc.scalar.activation(out=sd2, in_=var2[:, :], func=AF.Sqrt,
                                 bias=eps_t, scale=1.0)
            inv2 = _small_f32(sm, H2)
            nc.vector.reciprocal(out=inv2, in_=sd2[:, :])
            a2v = _small_f32(sm, H2)
            nc.vector.tensor_tensor(out=a2v, in0=inv2[:, :], in1=g2_t,
                                    op=OP.mult)
            ra2 = _small_f32(sm, H2)
            nc.vector.reciprocal(out=ra2, in_=a2v[:, :])
            tb2 = _small_f32(sm, H2)
            nc.vector.tensor_tensor(out=tb2, in0=bt2_t, in1=ra2[:, :],
                                    op=OP.mult)
            coa2 = _small_f32(sm, H2)
            nc.vector.tensor_tensor(out=coa2, in0=tb2[:, :], in1=cm2[:, 0:1],
                                    op=OP.subtract)
            a2rep = _small_f32(sm, H1)
            nc.vector.tensor_copy(a2rep[0:H2, :], a2v[:, :])
            nc.sync.dma_start(out=a2rep[H2:H1, :], in_=a2v[:, :])
            coa2rep = _small_f32(sm, H1)
            nc.vector.tensor_copy(coa2rep[0:H2, :], coa2[:, :])
            nc.sync.dma_start(out=coa2rep[H2:H1, :], in_=coa2[:, :])
            w3p = sm.tile([H1, H2], BF16)
            nc.vector.tensor_scalar(w3p[:, :], w3_t, a2rep, None, OP.mult)

            acc3 = accp.tile([H2, NSLICE, 6], F32)

            # ---------------- pass 2: in-place act2 on Z2, z3, stats3
            with (
                tc.tile_pool(name="psz3", bufs=2, space="PSUM") as psz3,
            ):
                for t in range(NSLICE // 4):
                    pz3 = psz3.tile([H2, 2048], F32, tag="z3")
                    for kk in range(2):
                        a2s = slice((4 * t + 2 * kk) * 512,
                                    (4 * t + 2 * kk + 2) * 512)
                        nc.scalar.activation(out=Z2[:, a2s], in_=Z2[:, a2s],
                                             func=AF.Relu, bias=coa2rep,
                                             scale=1.0)
                    for k in range(4):
                        zs = slice((4 * t + k) * 512, (4 * t + k + 1) * 512)
                        nc.tensor.matmul(pz3[:, k * 512:(k + 1) * 512],
                                         w3p[:, :], Z2[:, zs],
                                         start=True, stop=True)
                    for k in range(4):
                        nc.vector.bn_stats(out=acc3[:, 4 * t + k, :],
                                           in_=pz3[:, k * 512:(k + 1) * 512])

            # ---------------- stats3 -> a3, coa3, W4p
            mv3 = sm.tile([H2, 2], F32)
            nc.vector.bn_aggr(out=mv3, in_=acc3[:, :, :])
            sq3 = _small_f32(sm, H2)
            nc.vector.tensor_tensor(out=sq3, in0=mv3[:, 0:1], in1=mv3[:, 0:1],
                                    op=OP.mult)
            st3 = sm.tile([H2, 2], F32)
            nc.vector.tensor_copy(st3[:, 0:1], mv3[:, 0:1])
            nc.vector.tensor_tensor(out=st3[:, 1:2], in0=mv3[:, 1:2], in1=sq3,
                                    op=OP.add)
            hi3 = sm.tile([H3, 2], F32)
            nc.sync.dma_start(out=hi3, in_=st3[H3:H2, :])
            cm3 = sm.tile([H3, 2], F32)
            nc.vector.tensor_tensor(out=cm3, in0=st3[0:H3, :], in1=hi3[:, :],
                                    op=OP.add)
            nc.vector.tensor_scalar(cm3[:, :], cm3[:, :], 0.5, None, OP.mult)
            msq3 = _small_f32(sm, H3)
            nc.vector.tensor_tensor(out=msq3, in0=cm3[:, 0:1], in1=cm3[:, 0:1],
                                    op=OP.mult)
            var3 = _small_f32(sm, H3)
            nc.vector.tensor_tensor(out=var3, in0=cm3[:, 1:2], in1=msq3,
                                    op=OP.subtract)
            sd3 = _small_f32(sm, H3)
            nc.scalar.activation(out=sd3, in_=var3[:, :], func=AF.Sqrt,
                                 bias=eps_t[0:H3, :], scale=1.0)
            inv3 = _small_f32(sm, H3)
            nc.vector.reciprocal(out=inv3, in_=sd3[:, :])
            a3v = _small_f32(sm, H3)
            nc.vector.tensor_tensor(out=a3v, in0=inv3[:, :], in1=g3_t,
                                    op=OP.mult)
            ra3 = _small_f32(sm, H3)
            nc.vector.reciprocal(out=ra3, in_=a3v[:, :])
            tb3 = _small_f32(sm, H3)
            nc.vector.tensor_tensor(out=tb3, in0=bt3_t, in1=ra3[:, :],
                                    op=OP.mult)
            coa3 = _small_f32(sm, H3)
            nc.vector.tensor_tensor(out=coa3, in0=tb3[:, :], in1=cm3[:, 0:1],
                                    op=OP.subtract)
            a3rep = _small_f32(sm, H1)
            nc.vector.tensor_copy(a3rep[0:H3, :], a3v[:, :])
            for g in range(1, 4):
                nc.sync.dma_start(out=a3rep[g * H3:(g + 1) * H3, :],
                                  in_=a3v[:, :])
            coa3rep = _small_f32(sm, H1)
            nc.vector.tensor_copy(coa3rep[0:H3, :], coa3[:, :])
            for g in range(1, 4):
                nc.sync.dma_start(out=coa3rep[g * H3:(g + 1) * H3, :],
                                  in_=coa3[:, :])
            w4p = sm.tile([H1, 4], BF16)
            nc.vector.tensor_scalar(w4p[:, :], w4_t, a3rep, None, OP.mult)

            # ---------------- pass 3: recompute z3, act3, z4, sigmoid, out
            with (
                tc.tile_pool(name="h3p", bufs=2) as h3p,
                tc.tile_pool(name="yp", bufs=2) as yp,
                tc.tile_pool(name="psz34", bufs=2, space="PSUM") as psz34,
                tc.tile_pool(name="psz4", bufs=2, space="PSUM") as psz4,
            ):
                for t4 in range(NSLICE // 4):
                    pz4 = psz4.tile([4, 1024], F32, tag="z4")
                    for t in range(t4 * 2, t4 * 2 + 2):
                        pz34 = psz34.tile([H1, 512], F32, tag="z34")
                        for sp in range(2):
                            s = 2 * t + sp
                            zs = slice(s * 512, (s + 1) * 512)
                            nc.tensor.matmul(pz34[sp * H2:(sp + 1) * H2, :],
                                             w3p[:, :], Z2[:, zs],
                                             start=True, stop=True)
                        h3_t = h3p.tile([H1, 512], BF16, tag="h3")
                        nc.scalar.activation(out=h3_t, in_=pz34[:, :],
                                             func=AF.Relu, bias=coa3rep,
                                             scale=1.0)
                        k = t % 2
                        nc.tensor.matmul(pz4[:, k * 512:(k + 1) * 512],
                                         w4p[:, :], h3_t[:, :],
                                         start=True, stop=True)
                    y_t = yp.tile([4, 1024], F32, tag="y")
                    nc.scalar.activation(out=y_t, in_=pz4[:, :],
                                         func=AF.Sigmoid, bias=b4_t, scale=1.0)
                    nc.scalar.dma_start(out=yout[:, t4 * 1024:(t4 + 1) * 1024],
                                          in_=y_t)
    return nc


def _get_program():
    if "F" not in _cache:
        _cache["F"] = build_fused()
    return _cache["F"]


def _run(prog, in_maps, cores, label):
    tr = bool(os.environ.get("BASS_KERNEL_TRACE"))
    r = run_bass_kernel_spmd(prog, in_maps, cores, trace=tr)
    if tr and r.exec_time_ns:
        _cache["hw_exec_ns"] = _cache.get("hw_exec_ns", 0) + r.exec_time_ns
        _cache[f"ns_{label}"] = r.exec_time_ns
        if r.instructions_and_trace:
            _cache[f"trace_{label}"] = r.instructions_and_trace[1]
    return r.results


def kernel(**inputs):
    inp = {k: np.asarray(v) for k, v in inputs.items()}
    cores = list(range(N_CORES))
    _cache.pop("hw_exec_ns", None)

    W1eff64 = _build_w1eff(inp["breed_emb"], inp["temp_emb"], inp["W1"])

    cats_all = [inp["pet1_breed"], inp["pet1_size"], inp["pet1_energy"],
                inp["pet1_temp"], inp["pet2_breed"], inp["pet2_size"],
                inp["pet2_energy"], inp["pet2_temp"]]
    nums_all = [inp["pet1_age"] / 15.0, inp["pet1_social"],
                inp["pet1_weight"] / 100.0, inp["pet2_age"] / 15.0,
                inp["pet2_social"], inp["pet2_weight"] / 100.0]
    nums_all = [np.asarray(x, np.float32) for x in nums_all]

    mu1, var1 = _host_stats1(cats_all, [x.astype(np.float64) for x in nums_all],
                             W1eff64)
    a1 = (np.asarray(inp["gamma1"], np.float64) / np.sqrt(var1 + EPS))
    c1 = (np.asarray(inp["beta1"], np.float64) - a1 * mu1)

    # recenter: z1 = b* + W1p^T u' with u' dropping each stream's last dim
    bstar = np.zeros(H1, np.float64)
    W1p = np.zeros((NU2, H1), np.float64)
    for i in range(8):
        o, k = CAT_OFFS[i], CAT_SIZES[i]
        o2, k2 = CAT2_OFFS[i], CAT2_SIZES[i]
        last = W1eff64[o + k - 1]
        bstar += last
        W1p[o2:o2 + k2] = W1eff64[o:o + k - 1] - last
    W1p[NCAT2:NU2] = W1eff64[NCAT:NU]
    c1p = c1 + a1 * bstar

    # packed selector [16, 116] and iota [122,1]: even chunk one-hot at
    # partitions 0:52 (+num 52:58), odd at 64:116 (+num 116:122)
    s_sel2 = np.zeros((16, 116), np.float32)
    iota2 = np.full((122, 1), 255.0, np.float32)
    for i in range(8):
        o2, k2 = CAT2_OFFS[i], CAT2_SIZES[i]
        s_sel2[i, o2:o2 + k2] = 1.0
        s_sel2[8 + i, ODD + o2:ODD + o2 + k2] = 1.0
        iota2[o2:o2 + k2, 0] = np.arange(k2, dtype=np.float32)
        iota2[ODD + o2:ODD + o2 + k2, 0] = np.arange(k2, dtype=np.float32)

    cbf = np.zeros((H1, CF_W), np.float32)
    cbf[0:122, CF_IOTA] = iota2[:, 0]
    cbf[0:H1, CF_A1] = a1.astype(np.float32)
    cbf[0:H1, CF_C1] = c1p.astype(np.float32)
    cbf[0:H2, CF_G2] = np.asarray(inp["gamma2"], np.float32)
    cbf[0:H2, CF_BT2] = np.asarray(inp["beta2"], np.float32)
    cbf[0:H3, CF_G3] = np.asarray(inp["gamma3"], np.float32)
    cbf[0:H3, CF_BT3] = np.asarray(inp["beta3"], np.float32)
    cbf[0:H2, CF_EPS] = EPS
    cbf[0:4, CF_B4] = float(np.asarray(inp["b4"]).reshape(-1)[0])

    W3blk = np.zeros((H1, H2), np.float32)
    W3 = np.asarray(inp["W3"], np.float32)
    W3blk[0:H2, 0:H3] = W3
    W3blk[H2:H1, H3:H2] = W3
    W4blk = np.zeros((H1, 4), np.float32)
    w4 = np.asarray(inp["W4"], np.float32)[:, 0]
    for g in range(4):
        W4blk[g * H3:(g + 1) * H3, g] = w4

    cbb = np.zeros((H1, CBW), np.float32)
    cbb[0:NU2, CB_W1:CB_W1 + H1] = W1p.astype(np.float32)
    cbb[ODD:ODD + NU2, CB_W1:CB_W1 + H1] = W1p.astype(np.float32)
    cbb[0:H1, CB_W2:CB_W2 + H2] = np.asarray(inp["W2"], np.float32)
    cbb[0:H1, CB_W3:CB_W3 + H2] = W3blk
    cbb[0:H1, CB_W4:CB_W4 + 4] = W4blk
    cbb[0:16, CB_SSEL:CB_SSEL + 116] = s_sel2
    cbb = cbb.astype(BF)

    prog = _get_program()

    cat_mat = np.stack(cats_all).astype(BF)       # [8, B]
    num_mat = np.stack(nums_all).astype(BF)       # [6, B]

    in_maps = []
    for c in cores:
        sl = slice(c * SHARD, (c + 1) * SHARD)
        # pack pairs: row s+8h = stream s of chunk-parity h, col p*FD+q
        cm = cat_mat[:, sl].reshape(8, NPAIR, 2, FD)
        cat2 = np.ascontiguousarray(
            cm.transpose(2, 0, 1, 3).reshape(16, SHARD // 2))
        nm = num_mat[:, sl].reshape(6, NPAIR, 2, FD)
        num2 = np.ascontiguousarray(
            nm.transpose(2, 0, 1, 3).reshape(12, SHARD // 2))
        in_maps.append({
            "catk": cat2, "numk": num2,
            "cbf": cbf, "cbb": cbb,
        })
    res = _run(prog, in_maps, cores, "F")

    perm = _cache.get("perm")
    if perm is None:
        perm = _out_perm()
        _cache["perm"] = perm
    out = np.empty(B, np.float32)
    for c in cores:
        out[c * SHARD:(c + 1) * SHARD] = res[c]["yout"].reshape(-1)[perm]
    return out


# revision 6
# speedup vs baseline: 1.2870x; 1.0121x over previous
"""Trainium2 fused single-launch kernel for nn_CompatibilityModel.

Data parallel over 8 cores, one NEFF launch, per-shard BN stats for
layers 2/3 (layer-1 stats exact on host via joint histograms).

Per core (131072 rows = 128 chunks x 1024):
  pass1: cat DMA [8,2048]/pair -> PE broadcast matmul -> DVE is_equal
         one-hot u[0:60] (numerics DMA'd to u[60:66]) -> PE z1 (K=66)
         -> ScalarE relu(a1*z1+c1) -> bf16 h1 -> PE z2 (pair-packed
         PSUM [128,1024]) -> ScalarE copy -> Z2 resident bf16
         [128,65536] -> DVE bn_stats.
  stats2 on device: bn_aggr + rowgroup combine -> a2=gamma/sigma,
         coa2=c2/a2; a2 folded into W3 (relu(a*z+c)=a*relu(z+c/a)).
  pass2: gpsimd in-place h2' = max(z2+coa2, 0) on Z2 -> PE z3
         (block-diag W3p, 4 slices/PSUM tile) -> DVE bn_stats3.
  stats3 combine -> a3, coa3; a3 folded into W4.
  pass3: PE z3 recomputed from resident h2' (slice-pairs packed
         [128,512] PSUM) -> ScalarE relu(z3+coa3) -> PE z4 (block-diag
         W4p, [4,2048] PSUM) -> ScalarE sigmoid(+b4) -> DMA out.

All matmuls bf16 (fp32 is 4 cycles/row on PE, bf16 is 1).
"""

import json
import os

import numpy as np
import ml_dtypes

import concourse.bass as bass
import concourse.mybir as mybir
import concourse.bass_utils as _bass_utils
import concourse.bass2jax as _bass2jax
from concourse.bass_utils import run_bass_kernel_spmd
from concourse.tile import TileContext

BF = ml_dtypes.bfloat16


# --------------------------------------------------------------------------- wait splitting
# This walrus build rejects instructions carrying more than one semaphore
# wait; split extras onto standalone EventSemaphore instructions.
def _split_multi_waits(bir_json: bytes) -> bytes:
    m = json.loads(bir_json)
    for f in m.get("functions", []):
        for bb in f.get("blocks", []):
            out = []
            for ins in bb.get("instructions", []):
                si = ins.get("sync_info") or {}
                ow = si.get("on_wait") or []
                if len(ow) > 1:
                    for k, w in enumerate(ow[:-1]):
                        out.append({
                            "name": f"{ins['name']}-wsplit{k}",
                            "opcode": "EventSemaphore",
                            "engine": ins["engine"],
                            "ins": [],
                            "outs": [],
                            "sync_info": {"on_update": [], "on_wait": [w]},
                        })
                    si["on_wait"] = [ow[-1]]
                out.append(ins)
            bb["instructions"] = out
    return json.dumps(m).encode()


_orig_compile_bir_kernel = _bass_utils.compile_bir_kernel


def _patched_compile_bir_kernel(bir_json, tmpdir, neff_name="file.neff"):
    return _orig_compile_bir_kernel(_split_multi_waits(bir_json), tmpdir, neff_name)


_bass_utils.compile_bir_kernel = _patched_compile_bir_kernel
_bass2jax.compile_bir_kernel = _patched_compile_bir_kernel

F32 = mybir.dt.float32
BF16 = mybir.dt.bfloat16
AF = mybir.ActivationFunctionType
OP = mybir.AluOpType

B = 1 << 20
N_CORES = 8
SHARD = B // N_CORES           # 131072
FD = 1024                      # rows per chunk
NCH = SHARD // FD              # 128
NPAIR = NCH // 2               # 64
Z2C = SHARD // 2               # 65536 Z2 columns
NSLICE = Z2C // 512            # 128
YC = SHARD // 4                # 32768 output columns

EMB = 8
N_BREEDS, N_TEMPS = 15, 9
CAT_SIZES = [N_BREEDS, 3, 3, N_TEMPS] * 2
CAT_OFFS = np.concatenate([[0], np.cumsum(CAT_SIZES)]).astype(int)
NCAT = int(CAT_OFFS[-1])       # 60
NU = 66
H1, H2, H3 = 128, 64, 32
EPS = 1e-5

# f32 const blob columns: [128, CF]
CF_IOTA = 0     # [60,1]
CF_A1 = 1       # [128,1]
CF_C1 = 2       # [128,1]
CF_G2 = 3       # [64,1]
CF_BT2 = 4      # [64,1]
CF_G3 = 5       # [32,1]
CF_BT3 = 6      # [32,1]
CF_EPS = 7      # [64,1]
CF_B4 = 8       # [4,1]
CF_W = 9

# bf16 const blob columns: [128, CBW]
CB_W1 = 0       # [58,128]  recentered cat dims + numerics
CB_W2 = 128     # [128,64]
CB_W3 = 192     # [128,64]  block-diag W3
CB_W4 = 256     # [128,4]   block-diag w4
CB_SSEL = 260   # [16,116]  packed-pair selector
CBW = 380

# packed one-hot: stream s keeps K_s-1 dims (last dim recentered into bias)
CAT2_SIZES = [k - 1 for k in CAT_SIZES]            # [14,2,2,8]*2
CAT2_OFFS = np.concatenate([[0], np.cumsum(CAT2_SIZES)]).astype(int)
NCAT2 = int(CAT2_OFFS[-1])     # 52
NU2 = NCAT2 + 6                # 58 per chunk
ODD = 64                       # odd-chunk partition base in packed tiles

_cache = {}


# ----------------------------------------------------------------------------- host math
def _build_w1eff(breed_emb, temp_emb, W1):
    A2 = np.zeros((NU, 50), np.float64)
    be = np.asarray(breed_emb, np.float64)
    te = np.asarray(temp_emb, np.float64)
    A2[0:15, 0:8] = be
    A2[15:18, 8:11] = np.eye(3)
    A2[18:21, 11:14] = np.eye(3)
    A2[21:30, 14:22] = te
    A2[30:45, 25:33] = be
    A2[45:48, 33:36] = np.eye(3)
    A2[48:51, 36:39] = np.eye(3)
    A2[51:60, 39:47] = te
    A2[60, 22] = 1.0
    A2[61, 23] = 1.0
    A2[62, 24] = 1.0
    A2[63, 47] = 1.0
    A2[64, 48] = 1.0
    A2[65, 49] = 1.0
    return A2 @ np.asarray(W1, np.float64)


def _host_stats1(cats, nums, W1eff):
    """Exact E[z1], Var[z1] via E[u], E[uu^T] in float64."""
    n = cats[0].shape[0]
    cats = [c.astype(np.int64) for c in cats]
    M = np.zeros((NU, NU), np.float64)
    Eu = np.zeros(NU, np.float64)
    for i, ci in enumerate(cats):
        Ki, oi = CAT_SIZES[i], CAT_OFFS[i]
        pi = np.bincount(ci, minlength=Ki) / n
        Eu[oi:oi + Ki] = pi
        M[oi:oi + Ki, oi:oi + Ki] = np.diag(pi)
        for j in range(i):
            Kj, oj = CAT_SIZES[j], CAT_OFFS[j]
            joint = np.bincount(ci * Kj + cats[j],
                                minlength=Ki * Kj).reshape(Ki, Kj) / n
            M[oi:oi + Ki, oj:oj + Kj] = joint
            M[oj:oj + Kj, oi:oi + Ki] = joint.T
        for j, xj in enumerate(nums):
            s = np.bincount(ci, weights=xj, minlength=Ki) / n
            M[oi:oi + Ki, NCAT + j] = s
            M[NCAT + j, oi:oi + Ki] = s
    for i, xi in enumerate(nums):
        Eu[NCAT + i] = xi.mean(dtype=np.float64)
        for j, xj in enumerate(nums):
            if j <= i:
                v = np.dot(xi, xj) / n
                M[NCAT + i, NCAT + j] = v
                M[NCAT + j, NCAT + i] = v
    Ez = W1eff.T @ Eu
    Ez2 = np.sum(W1eff * (M @ W1eff), axis=0)
    return Ez, Ez2 - Ez * Ez


def _out_perm():
    """row r of a shard -> flat index into y_dev [4, YC]."""
    r = np.arange(SHARD)
    c = r // FD
    q = r % FD
    x = (c // 2) * FD + q          # Z2 column
    j = x // 512                   # slice
    sx = x % 512
    t = j // 2                     # slice pair
    sp = j % 2
    g = 2 * sp + (c % 2)
    return g * YC + t * 512 + sx


# ----------------------------------------------------------------------------- program
_SMALL_N = [0]


def _small_f32(pool, n):
    _SMALL_N[0] += 1
    return pool.tile([n, 1], F32, name=f"sm{_SMALL_N[0]}")


def build_fused():
    nc = bass.Bass()
    catk = nc.dram_tensor("catk", [16, SHARD // 2], BF16, kind="ExternalInput")
    numk = nc.dram_tensor("numk", [12, SHARD // 2], BF16, kind="ExternalInput")
    cbf = nc.dram_tensor("cbf", [H1, CF_W], F32, kind="ExternalInput")
    cbb = nc.dram_tensor("cbb", [H1, CBW], BF16, kind="ExternalInput")
    yout = nc.dram_tensor("yout", [4, YC], F32, kind="ExternalOutput")

    with TileContext(nc) as tc:
        with (
            tc.tile_pool(name="consts", bufs=1) as consts,
            tc.tile_pool(name="z2res", bufs=1) as z2res,
            tc.tile_pool(name="accp", bufs=1) as accp,
        ):
            cf = consts.tile([H1, CF_W], F32)
            nc.sync.dma_start(out=cf, in_=cbf[:, :])
            cb = consts.tile([H1, CBW], BF16)
            nc.scalar.dma_start(out=cb, in_=cbb[:, :])

            iota_t = cf[0:116, CF_IOTA:CF_IOTA + 1]
            a1_t = cf[0:H1, CF_A1:CF_A1 + 1]
            c1_t = cf[0:H1, CF_C1:CF_C1 + 1]
            g2_t = cf[0:H2, CF_G2:CF_G2 + 1]
            bt2_t = cf[0:H2, CF_BT2:CF_BT2 + 1]
            g3_t = cf[0:H3, CF_G3:CF_G3 + 1]
            bt3_t = cf[0:H3, CF_BT3:CF_BT3 + 1]
            eps_t = cf[0:H2, CF_EPS:CF_EPS + 1]
            b4_t = cf[0:4, CF_B4:CF_B4 + 1]

            w1_t = cb[0:NU2, CB_W1:CB_W1 + H1]
            w1o_t = cb[ODD:ODD + NU2, CB_W1:CB_W1 + H1]
            w2_t = cb[0:H1, CB_W2:CB_W2 + H2]
            w3_t = cb[0:H1, CB_W3:CB_W3 + H2]
            w4_t = cb[0:H1, CB_W4:CB_W4 + 4]
            ssel_t = cb[0:16, CB_SSEL:CB_SSEL + 116]

            # warmups: absorb const DMA wait into each engine's clock
            with tc.tile_pool(name="warm", bufs=1, space="PSUM") as warmp:
                ps_w = warmp.tile([1, 1], F32)
                nc.tensor.matmul(ps_w, cb[0:1, 0:1], cb[0:1, 0:1],
                                 start=True, stop=True)
            scr_v = consts.tile([1, 1], F32)
            nc.vector.tensor_copy(scr_v, cf[0:1, 0:1])
            scr_s = consts.tile([1, 1], F32)
            nc.scalar.copy(scr_s, cf[0:1, 0:1])
            scr_g = consts.tile([1, 1], BF16)
            nc.gpsimd.tensor_copy(scr_g, cb[0:1, 0:1])

            Z2 = z2res.tile([H1, Z2C], BF16)
            acc2 = accp.tile([H1, NCH, 6], F32)

            dummy_n = [0]

            def pe_keepalive(psd_pool, n=2):
                for _ in range(n):
                    dummy_n[0] += 1
                    psd = psd_pool.tile([1, 512], F32, name=f"dum{dummy_n[0]}",
                                        tag="dum")
                    nc.tensor.matmul(psd[:, :], cb[0:1, 0:1], Z2[0:1, 0:512],
                                     start=True, stop=True, skip_group_check=True)

            # ---------------- pass 1: packed pairs (even chunk at
            # partitions 0:58, odd at 64:122 of one u tile; one broadcast
            # matmul + one is_equal covers both chunks)
            with (
                tc.tile_pool(name="catp", bufs=4) as catp,
                tc.tile_pool(name="up", bufs=3) as up,
                tc.tile_pool(name="h1p", bufs=3) as h1p,
                tc.tile_pool(name="psb", bufs=2, space="PSUM") as psb,
                tc.tile_pool(name="psz1", bufs=2, space="PSUM") as psz1,
                tc.tile_pool(name="psz2", bufs=2, space="PSUM") as psz2,
            ):
                for p in range(NPAIR):
                    pc = p * FD
                    cat_t = catp.tile([16, FD], BF16, tag="cat")
                    nc.sync.dma_start(out=cat_t, in_=catk[:, pc:pc + FD])
                    u_t = up.tile([122, FD], BF16, tag="u")
                    for sl in range(2):
                        pzb = psb.tile([116, 512], F32, tag="b")
                        nc.tensor.matmul(pzb[:, :], ssel_t,
                                         cat_t[:, sl * 512:(sl + 1) * 512],
                                         start=True, stop=True)
                        nc.vector.tensor_scalar(
                            u_t[0:116, sl * 512:(sl + 1) * 512],
                            pzb[:, :], iota_t, None, OP.is_equal)
                    # numerics overwrite the is_equal zeros at [52:58]/[116:122]
                    nc.sync.dma_start(out=u_t[NCAT2:NU2, :],
                                      in_=numk[0:6, pc:pc + FD])
                    nc.sync.dma_start(out=u_t[ODD + NCAT2:ODD + NU2, :],
                                      in_=numk[6:12, pc:pc + FD])
                    pz2s = [psz2.tile([H1, 512], F32, tag="z2",
                                      name=f"z2_{p}_{j}") for j in range(2)]
                    for h in range(2):
                        pz1 = psz1.tile([H1, FD], F32, tag="z1")
                        for j in range(0, FD, 512):
                            nc.tensor.matmul(
                                pz1[:, j:j + 512], w1o_t if h else w1_t,
                                u_t[ODD * h:ODD * h + NU2, j:j + 512],
                                start=True, stop=True)
                        h1_t = h1p.tile([H1, FD], BF16, tag="h1")
                        nc.scalar.activation(out=h1_t, in_=pz1[:, :],
                                             func=AF.Relu, bias=c1_t,
                                             scale=a1_t)
                        for j in range(2):
                            nc.tensor.matmul(
                                pz2s[j][h * H2:h * H2 + H2, :], w2_t,
                                h1_t[:, j * 512:(j + 1) * 512],
                                start=True, stop=True)
                    for j in range(2):
                        zc = slice(p * FD + j * 512, p * FD + (j + 1) * 512)
                        nc.scalar.copy(out=Z2[:, zc], in_=pz2s[j][:, :])
                        nc.vector.bn_stats(out=acc2[:, 2 * p + j, :],
                                           in_=Z2[:, zc])
            # ---------------- stats2 -> a2, coa2, W3p
            sm = accp
            mv2 = sm.tile([H1, 2], F32)
            nc.vector.bn_aggr(out=mv2, in_=acc2[:, :, :])

            sq2 = _small_f32(sm, H1)
            nc.vector.tensor_tensor(out=sq2, in0=mv2[:, 0:1], in1=mv2[:, 0:1],
                                    op=OP.mult)
            st2 = sm.tile([H1, 2], F32)
            nc.vector.tensor_copy(st2[:, 0:1], mv2[:, 0:1])
            nc.vector.tensor_tensor(out=st2[:, 1:2], in0=mv2[:, 1:2], in1=sq2,
                                    op=OP.add)
            hi2 = sm.tile([H2, 2], F32)
            nc.sync.dma_start(out=hi2, in_=st2[H2:H1, :])
            cm2 = sm.tile([H2, 2], F32)
            nc.vector.tensor_tensor(out=cm2, in0=st2[0:H2, :], in1=hi2[:, :],
                                    op=OP.add)
            nc.vector.tensor_scalar(cm2[:, :], cm2[:, :], 0.5, None, OP.mult)
            msq2 = _small_f32(sm, H2)
            nc.vector.tensor_tensor(out=msq2, in0=cm2[:, 0:1], in1=cm2[:, 0:1],
                                    op=OP.mult)
            var2 = _small_f32(sm, H2)
            nc.vector.tensor_tensor(out=var2, in0=cm2[:, 1:2], in1=msq2,
                                    op=OP.subtract)
            sd2 = _small_f32(sm, H2)
            nc.scalar.activation(out=sd2, in_=var2[:, :], func=AF.Sqrt,
                                 bias=eps_t, scale=1.0)
            inv2 = _small_f32(sm, H2)
            nc.vector.reciprocal(out=inv2, in_=sd2[:, :])
            a2v = _small_f32(sm, H2)
            nc.vector.tensor_tensor(out=a2v, in0=inv2[:, :], in1=g2_t,
                                    op=OP.mult)
            ra2 = _small_f32(sm, H2)
            nc.vector.reciprocal(out=ra2, in_=a2v[:, :])
            tb2 = _small_f32(sm, H2)
            nc.vector.tensor_tensor(out=tb2, in0=bt2_t, in1=ra2[:, :],
                                    op=OP.mult)
            coa2 = _small_f32(sm, H2)
            nc.vector.tensor_tensor(out=coa2, in0=tb2[:, :], in1=cm2[:, 0:1],
                                    op=OP.subtract)
            a2rep = _small_f32(sm, H1)
            nc.vector.tensor_copy(a2rep[0:H2, :], a2v[:, :])
            nc.sync.dma_start(out=a2rep[H2:H1, :], in_=a2v[:, :])
            coa2rep = _small_f32(sm, H1)
            nc.vector.tensor_copy(coa2rep[0:H2, :], coa2[:, :])
            nc.sync.dma_start(out=coa2rep[H2:H1, :], in_=coa2[:, :])
            w3p = sm.tile([H1, H2], BF16)
            nc.vector.tensor_scalar(w3p[:, :], w3_t, a2rep, None, OP.mult)

            acc3 = accp.tile([H1, NSLICE // 2, 6], F32)

            # ---------------- pass 2: in-place act2 on Z2, z3 (slice-parity
            # packed into [128,1024] PSUM via partition offsets), stats3
            with (
                tc.tile_pool(name="psz3", bufs=3, space="PSUM") as psz3,
            ):
                for t in range(NSLICE // 4):
                    pz3 = psz3.tile([H1, 1024], F32, tag="z3")
                    for kk in range(2):
                        a2s = slice((4 * t + 2 * kk) * 512,
                                    (4 * t + 2 * kk + 2) * 512)
                        nc.scalar.activation(out=Z2[:, a2s], in_=Z2[:, a2s],
                                             func=AF.Relu, bias=coa2rep,
                                             scale=1.0)
                    for k in range(4):
                        sl = 4 * t + k
                        zs = slice(sl * 512, (sl + 1) * 512)
                        po = H2 * (k % 2)
                        co = 512 * (k // 2)
                        nc.tensor.matmul(pz3[po:po + H2, co:co + 512],
                                         w3p[:, :], Z2[:, zs],
                                         start=True, stop=True)
                    for k in range(2):
                        nc.vector.bn_stats(out=acc3[:, 2 * t + k, :],
                                           in_=pz3[:, k * 512:(k + 1) * 512])

            # ---------------- stats3 -> a3, coa3, W4p
            mv3 = sm.tile([H1, 2], F32)
            nc.vector.bn_aggr(out=mv3, in_=acc3[:, :, :])
            sq3 = _small_f32(sm, H1)
            nc.vector.tensor_tensor(out=sq3, in0=mv3[:, 0:1], in1=mv3[:, 0:1],
                                    op=OP.mult)
            st3 = sm.tile([H1, 2], F32)
            nc.vector.tensor_copy(st3[:, 0:1], mv3[:, 0:1])
            nc.vector.tensor_tensor(out=st3[:, 1:2], in0=mv3[:, 1:2], in1=sq3,
                                    op=OP.add)
            hi3b = sm.tile([H2, 2], F32)
            nc.sync.dma_start(out=hi3b, in_=st3[H2:H1, :])
            cmA = sm.tile([H2, 2], F32)
            nc.vector.tensor_tensor(out=cmA, in0=st3[0:H2, :], in1=hi3b[:, :],
                                    op=OP.add)
            hi3 = sm.tile([H3, 2], F32)
            nc.sync.dma_start(out=hi3, in_=cmA[H3:H2, :])
            cm3 = sm.tile([H3, 2], F32)
            nc.vector.tensor_tensor(out=cm3, in0=cmA[0:H3, :], in1=hi3[:, :],
                                    op=OP.add)
            nc.vector.tensor_scalar(cm3[:, :], cm3[:, :], 0.25, None, OP.mult)
            msq3 = _small_f32(sm, H3)
            nc.vector.tensor_tensor(out=msq3, in0=cm3[:, 0:1], in1=cm3[:, 0:1],
                                    op=OP.mult)
            var3 = _small_f32(sm, H3)
            nc.vector.tensor_tensor(out=var3, in0=cm3[:, 1:2], in1=msq3,
                                    op=OP.subtract)
            sd3 = _small_f32(sm, H3)
            nc.scalar.activation(out=sd3, in_=var3[:, :], func=AF.Sqrt,
                                 bias=eps_t[0:H3, :], scale=1.0)
            inv3 = _small_f32(sm, H3)
            nc.vector.reciprocal(out=inv3, in_=sd3[:, :])
            a3v = _small_f32(sm, H3)
            nc.vector.tensor_tensor(out=a3v, in0=inv3[:, :], in1=g3_t,
                                    op=OP.mult)
            ra3 = _small_f32(sm, H3)
            nc.vector.reciprocal(out=ra3, in_=a3v[:, :])
            tb3 = _small_f32(sm, H3)
            nc.vector.tensor_tensor(out=tb3, in0=bt3_t, in1=ra3[:, :],
                                    op=OP.mult)
            coa3 = _small_f32(sm, H3)
            nc.vector.tensor_tensor(out=coa3, in0=tb3[:, :], in1=cm3[:, 0:1],
                                    op=OP.subtract)
            a3rep = _small_f32(sm, H1)
            nc.vector.tensor_copy(a3rep[0:H3, :], a3v[:, :])
            for g in range(1, 4):
                nc.sync.dma_start(out=a3rep[g * H3:(g + 1) * H3, :],
                                  in_=a3v[:, :])
            coa3rep = _small_f32(sm, H1)
            nc.vector.tensor_copy(coa3rep[0:H3, :], coa3[:, :])
            for g in range(1, 4):
                nc.sync.dma_start(out=coa3rep[g * H3:(g + 1) * H3, :],
                                  in_=coa3[:, :])
            w4p = sm.tile([H1, 4], BF16)
            nc.vector.tensor_scalar(w4p[:, :], w4_t, a3rep, None, OP.mult)

            # ---------------- pass 3: recompute z3, act3, z4, sigmoid, out
            with (
                tc.tile_pool(name="h3p", bufs=2) as h3p,
                tc.tile_pool(name="yp", bufs=2) as yp,
                tc.tile_pool(name="psz34", bufs=2, space="PSUM") as psz34,
                tc.tile_pool(name="psz4", bufs=2, space="PSUM") as psz4,
            ):
                for t4 in range(NSLICE // 4):
                    pz4 = psz4.tile([4, 1024], F32, tag="z4")
                    h3_ts = []
                    for t in range(t4 * 2, t4 * 2 + 2):
                        pz34 = psz34.tile([H1, 512], F32, tag="z34")
                        for sp in range(2):
                            s = 2 * t + sp
                            zs = slice(s * 512, (s + 1) * 512)
                            nc.tensor.matmul(pz34[sp * H2:(sp + 1) * H2, :],
                                             w3p[:, :], Z2[:, zs],
                                             start=True, stop=True)
                        h3_t = h3p.tile([H1, 512], BF16, tag="h3")
                        nc.scalar.activation(out=h3_t, in_=pz34[:, :],
                                             func=AF.Relu, bias=coa3rep,
                                             scale=1.0)
                        h3_ts.append(h3_t)
                    for k, h3_t in enumerate(h3_ts):
                        nc.tensor.matmul(pz4[:, k * 512:(k + 1) * 512],
                                         w4p[:, :], h3_t[:, :],
                                         start=True, stop=True)
                    y_t = yp.tile([4, 1024], F32, tag="y")
                    nc.scalar.activation(out=y_t, in_=pz4[:, :],
                                         func=AF.Sigmoid, bias=b4_t, scale=1.0)
                    nc.sync.dma_start(out=yout[:, t4 * 1024:(t4 + 1) * 1024],
                                      in_=y_t)
    return nc


def _get_program():
    if "F" not in _cache:
        _cache["F"] = build_fused()
    return _cache["F"]


def _run(prog, in_maps, cores, label):
    tr = bool(os.environ.get("BASS_KERNEL_TRACE"))
    r = run_bass_kernel_spmd(prog, in_maps, cores, trace=tr)
    if tr and r.exec_time_ns:
        _cache["hw_exec_ns"] = _cache.get("hw_exec_ns", 0) + r.exec_time_ns
        _cache[f"ns_{label}"] = r.exec_time_ns
        if r.instructions_and_trace:
            _cache[f"trace_{label}"] = r.instructions_and_trace[1]
    return r.results


def kernel(**inputs):
    inp = {k: np.asarray(v) for k, v in inputs.items()}
    cores = list(range(N_CORES))
    _cache.pop("hw_exec_ns", None)

    W1eff64 = _build_w1eff(inp["breed_emb"], inp["temp_emb"], inp["W1"])

    cats_all = [inp["pet1_breed"], inp["pet1_size"], inp["pet1_energy"],
                inp["pet1_temp"], inp["pet2_breed"], inp["pet2_size"],
                inp["pet2_energy"], inp["pet2_temp"]]
    nums_all = [inp["pet1_age"] / 15.0, inp["pet1_social"],
                inp["pet1_weight"] / 100.0, inp["pet2_age"] / 15.0,
                inp["pet2_social"], inp["pet2_weight"] / 100.0]
    nums_all = [np.asarray(x, np.float32) for x in nums_all]

    mu1, var1 = _host_stats1(cats_all, [x.astype(np.float64) for x in nums_all],
                             W1eff64)
    a1 = (np.asarray(inp["gamma1"], np.float64) / np.sqrt(var1 + EPS))
    c1 = (np.asarray(inp["beta1"], np.float64) - a1 * mu1)

    # recenter: z1 = b* + W1p^T u' with u' dropping each stream's last dim
    bstar = np.zeros(H1, np.float64)
    W1p = np.zeros((NU2, H1), np.float64)
    for i in range(8):
        o, k = CAT_OFFS[i], CAT_SIZES[i]
        o2, k2 = CAT2_OFFS[i], CAT2_SIZES[i]
        last = W1eff64[o + k - 1]
        bstar += last
        W1p[o2:o2 + k2] = W1eff64[o:o + k - 1] - last
    W1p[NCAT2:NU2] = W1eff64[NCAT:NU]
    c1p = c1 + a1 * bstar

    # packed selector [16, 116] and iota [122,1]: even chunk one-hot at
    # partitions 0:52 (+num 52:58), odd at 64:116 (+num 116:122)
    s_sel2 = np.zeros((16, 116), np.float32)
    iota2 = np.full((122, 1), 255.0, np.float32)
    for i in range(8):
        o2, k2 = CAT2_OFFS[i], CAT2_SIZES[i]
        s_sel2[i, o2:o2 + k2] = 1.0
        s_sel2[8 + i, ODD + o2:ODD + o2 + k2] = 1.0
        iota2[o2:o2 + k2, 0] = np.arange(k2, dtype=np.float32)
        iota2[ODD + o2:ODD + o2 + k2, 0] = np.arange(k2, dtype=np.float32)

    cbf = np.zeros((H1, CF_W), np.float32)
    cbf[0:122, CF_IOTA] = iota2[:, 0]
    cbf[0:H1, CF_A1] = a1.astype(np.float32)
    cbf[0:H1, CF_C1] = c1p.astype(np.float32)
    cbf[0:H2, CF_G2] = np.asarray(inp["gamma2"], np.float32)
    cbf[0:H2, CF_BT2] = np.asarray(inp["beta2"], np.float32)
    cbf[0:H3, CF_G3] = np.asarray(inp["gamma3"], np.float32)
    cbf[0:H3, CF_BT3] = np.asarray(inp["beta3"], np.float32)
    cbf[0:H2, CF_EPS] = EPS
    cbf[0:4, CF_B4] = float(np.asarray(inp["b4"]).reshape(-1)[0])

    W3blk = np.zeros((H1, H2), np.float32)
    W3 = np.asarray(inp["W3"], np.float32)
    W3blk[0:H2, 0:H3] = W3
    W3blk[H2:H1, H3:H2] = W3
    W4blk = np.zeros((H1, 4), np.float32)
    w4 = np.asarray(inp["W4"], np.float32)[:, 0]
    for g in range(4):
        W4blk[g * H3:(g + 1) * H3, g] = w4

    cbb = np.zeros((H1, CBW), np.float32)
    cbb[0:NU2, CB_W1:CB_W1 + H1] = W1p.astype(np.float32)
    cbb[ODD:ODD + NU2, CB_W1:CB_W1 + H1] = W1p.astype(np.float32)
    cbb[0:H1, CB_W2:CB_W2 + H2] = np.asarray(inp["W2"], np.float32)
    cbb[0:H1, CB_W3:CB_W3 + H2] = W3blk
    cbb[0:H1, CB_W4:CB_W4 + 4] = W4blk
    cbb[0:16, CB_SSEL:CB_SSEL + 116] = s_sel2
    cbb = cbb.astype(BF)

    prog = _get_program()

    cat_mat = np.stack(cats_all).astype(BF)       # [8, B]
    num_mat = np.stack(nums_all).astype(BF)       # [6, B]

    in_maps = []
    for c in cores:
        sl = slice(c * SHARD, (c + 1) * SHARD)
        # pack pairs: row s+8h = stream s of chunk-parity h, col p*FD+q
        cm = cat_mat[:, sl].reshape(8, NPAIR, 2, FD)
        cat2 = np.ascontiguousarray(
            cm.transpose(2, 0, 1, 3).reshape(16, SHARD // 2))
        nm = num_mat[:, sl].reshape(6, NPAIR, 2, FD)
        num2 = np.ascontiguousarray(
            nm.transpose(2, 0, 1, 3).reshape(12, SHARD // 2))
        in_maps.append({
            "catk": cat2, "numk": num2,
            "cbf": cbf, "cbb": cbb,
        })
    res = _run(prog, in_maps, cores, "F")

    perm = _cache.get("perm")
    if perm is None:
        perm = _out_perm()
        _cache["perm"] = perm
    out = np.empty(B, np.float32)
    for c in cores:
        out[c * SHARD:(c + 1) * SHARD] = res[c]["yout"].reshape(-1)[perm]
    return out


# revision 7
# speedup vs baseline: 1.3119x; 1.0193x over previous
"""Trainium2 fused single-launch kernel for nn_CompatibilityModel.

Data parallel over 8 cores, one NEFF launch, per-shard BN stats for
layers 2/3 (layer-1 stats exact on host via joint histograms).

Per core (131072 rows = 128 chunks x 1024):
  pass1: cat DMA [8,2048]/pair -> PE broadcast matmul -> DVE is_equal
         one-hot u[0:60] (numerics DMA'd to u[60:66]) -> PE z1 (K=66)
         -> ScalarE relu(a1*z1+c1) -> bf16 h1 -> PE z2 (pair-packed
         PSUM [128,1024]) -> ScalarE copy -> Z2 resident bf16
         [128,65536] -> DVE bn_stats.
  stats2 on device: bn_aggr + rowgroup combine -> a2=gamma/sigma,
         coa2=c2/a2; a2 folded into W3 (relu(a*z+c)=a*relu(z+c/a)).
  pass2: gpsimd in-place h2' = max(z2+coa2, 0) on Z2 -> PE z3
         (block-diag W3p, 4 slices/PSUM tile) -> DVE bn_stats3.
  stats3 combine -> a3, coa3; a3 folded into W4.
  pass3: PE z3 recomputed from resident h2' (slice-pairs packed
         [128,512] PSUM) -> ScalarE relu(z3+coa3) -> PE z4 (block-diag
         W4p, [4,2048] PSUM) -> ScalarE sigmoid(+b4) -> DMA out.

All matmuls bf16 (fp32 is 4 cycles/row on PE, bf16 is 1).
"""

import json
import os

import numpy as np
import ml_dtypes

import concourse.bass as bass
import concourse.mybir as mybir
import concourse.bass_utils as _bass_utils
import concourse.bass2jax as _bass2jax
from concourse.bass_utils import run_bass_kernel_spmd
from concourse.tile import TileContext

BF = ml_dtypes.bfloat16


# --------------------------------------------------------------------------- wait splitting
# This walrus build rejects instructions carrying more than one semaphore
# wait; split extras onto standalone EventSemaphore instructions.
def _split_multi_waits(bir_json: bytes) -> bytes:
    m = json.loads(bir_json)
    for f in m.get("functions", []):
        for bb in f.get("blocks", []):
            out = []
            for ins in bb.get("instructions", []):
                si = ins.get("sync_info") or {}
                ow = si.get("on_wait") or []
                if len(ow) > 1:
                    for k, w in enumerate(ow[:-1]):
                        out.append({
                            "name": f"{ins['name']}-wsplit{k}",
                            "opcode": "EventSemaphore",
                            "engine": ins["engine"],
                            "ins": [],
                            "outs": [],
                            "sync_info": {"on_update": [], "on_wait": [w]},
                        })
                    si["on_wait"] = [ow[-1]]
                out.append(ins)
            bb["instructions"] = out
    return json.dumps(m).encode()


_orig_compile_bir_kernel = _bass_utils.compile_bir_kernel


def _patched_compile_bir_kernel(bir_json, tmpdir, neff_name="file.neff"):
    return _orig_compile_bir_kernel(_split_multi_waits(bir_json), tmpdir, neff_name)


_bass_utils.compile_bir_kernel = _patched_compile_bir_kernel
_bass2jax.compile_bir_kernel = _patched_compile_bir_kernel

F32 = mybir.dt.float32
BF16 = mybir.dt.bfloat16
AF = mybir.ActivationFunctionType
OP = mybir.AluOpType

B = 1 << 20
N_CORES = 8
SHARD = B // N_CORES           # 131072
FD = 1024                      # rows per chunk
NCH = SHARD // FD              # 128
NPAIR = NCH // 2               # 64
Z2C = SHARD // 2               # 65536 Z2 columns
NSLICE = Z2C // 512            # 128
YC = SHARD // 4                # 32768 output columns

EMB = 8
N_BREEDS, N_TEMPS = 15, 9
CAT_SIZES = [N_BREEDS, 3, 3, N_TEMPS] * 2
CAT_OFFS = np.concatenate([[0], np.cumsum(CAT_SIZES)]).astype(int)
NCAT = int(CAT_OFFS[-1])       # 60
NU = 66
H1, H2, H3 = 128, 64, 32
EPS = 1e-5

# f32 const blob columns: [128, CF]
CF_IOTA = 0     # [60,1]
CF_A1 = 1       # [128,1]
CF_C1 = 2       # [128,1]
CF_G2 = 3       # [64,1]
CF_BT2 = 4      # [64,1]
CF_G3 = 5       # [32,1]
CF_BT3 = 6      # [32,1]
CF_EPS = 7      # [64,1]
CF_B4 = 8       # [4,1]
CF_W = 9

# bf16 const blob columns: [128, CBW]
CB_W1 = 0       # [58,128]  recentered cat dims + numerics
CB_W2 = 128     # [128,64]
CB_W3 = 192     # [128,64]  block-diag W3
CB_W4 = 256     # [128,4]   block-diag w4
CB_SSEL = 260   # [16,116]  packed-pair selector
CBW = 380

# packed one-hot: stream s keeps K_s-1 dims (last dim recentered into bias)
CAT2_SIZES = [k - 1 for k in CAT_SIZES]            # [14,2,2,8]*2
CAT2_OFFS = np.concatenate([[0], np.cumsum(CAT2_SIZES)]).astype(int)
NCAT2 = int(CAT2_OFFS[-1])     # 52
NU2 = NCAT2 + 6                # 58 per chunk
ODD = 64                       # odd-chunk partition base in packed tiles

_cache = {}


# ----------------------------------------------------------------------------- host math
def _build_w1eff(breed_emb, temp_emb, W1):
    A2 = np.zeros((NU, 50), np.float64)
    be = np.asarray(breed_emb, np.float64)
    te = np.asarray(temp_emb, np.float64)
    A2[0:15, 0:8] = be
    A2[15:18, 8:11] = np.eye(3)
    A2[18:21, 11:14] = np.eye(3)
    A2[21:30, 14:22] = te
    A2[30:45, 25:33] = be
    A2[45:48, 33:36] = np.eye(3)
    A2[48:51, 36:39] = np.eye(3)
    A2[51:60, 39:47] = te
    A2[60, 22] = 1.0
    A2[61, 23] = 1.0
    A2[62, 24] = 1.0
    A2[63, 47] = 1.0
    A2[64, 48] = 1.0
    A2[65, 49] = 1.0
    return A2 @ np.asarray(W1, np.float64)


def _host_stats1(cats, nums, W1eff):
    """Exact E[z1], Var[z1] via E[u], E[uu^T] in float64."""
    n = cats[0].shape[0]
    cats = [c.astype(np.int64) for c in cats]
    M = np.zeros((NU, NU), np.float64)
    Eu = np.zeros(NU, np.float64)
    for i, ci in enumerate(cats):
        Ki, oi = CAT_SIZES[i], CAT_OFFS[i]
        pi = np.bincount(ci, minlength=Ki) / n
        Eu[oi:oi + Ki] = pi
        M[oi:oi + Ki, oi:oi + Ki] = np.diag(pi)
        for j in range(i):
            Kj, oj = CAT_SIZES[j], CAT_OFFS[j]
            joint = np.bincount(ci * Kj + cats[j],
                                minlength=Ki * Kj).reshape(Ki, Kj) / n
            M[oi:oi + Ki, oj:oj + Kj] = joint
            M[oj:oj + Kj, oi:oi + Ki] = joint.T
        for j, xj in enumerate(nums):
            s = np.bincount(ci, weights=xj, minlength=Ki) / n
            M[oi:oi + Ki, NCAT + j] = s
            M[NCAT + j, oi:oi + Ki] = s
    for i, xi in enumerate(nums):
        Eu[NCAT + i] = xi.mean(dtype=np.float64)
        for j, xj in enumerate(nums):
            if j <= i:
                v = np.dot(xi, xj) / n
                M[NCAT + i, NCAT + j] = v
                M[NCAT + j, NCAT + i] = v
    Ez = W1eff.T @ Eu
    Ez2 = np.sum(W1eff * (M @ W1eff), axis=0)
    return Ez, Ez2 - Ez * Ez


def _out_perm():
    """row r of a shard -> flat index into y_dev [4, YC]."""
    r = np.arange(SHARD)
    c = r // FD
    q = r % FD
    x = (c // 2) * FD + q          # Z2 column
    j = x // 512                   # slice
    sx = x % 512
    t = j // 2                     # slice pair
    sp = j % 2
    g = 2 * sp + (c % 2)
    return g * YC + t * 512 + sx


# ----------------------------------------------------------------------------- program
_SMALL_N = [0]


def _small_f32(pool, n):
    _SMALL_N[0] += 1
    return pool.tile([n, 1], F32, name=f"sm{_SMALL_N[0]}")


def build_fused():
    nc = bass.Bass()
    catk = nc.dram_tensor("catk", [16, SHARD // 2], BF16, kind="ExternalInput")
    numk = nc.dram_tensor("numk", [12, SHARD // 2], BF16, kind="ExternalInput")
    cbf = nc.dram_tensor("cbf", [H1, CF_W], F32, kind="ExternalInput")
    cbb = nc.dram_tensor("cbb", [H1, CBW], BF16, kind="ExternalInput")
    yout = nc.dram_tensor("yout", [4, YC], F32, kind="ExternalOutput")

    with TileContext(nc) as tc:
        with (
            tc.tile_pool(name="consts", bufs=1) as consts,
            tc.tile_pool(name="z2res", bufs=1) as z2res,
            tc.tile_pool(name="accp", bufs=1) as accp,
        ):
            cf = consts.tile([H1, CF_W], F32)
            nc.sync.dma_start(out=cf, in_=cbf[:, :])
            cb = consts.tile([H1, CBW], BF16)
            nc.scalar.dma_start(out=cb, in_=cbb[:, :])

            iota_t = cf[0:116, CF_IOTA:CF_IOTA + 1]
            a1_t = cf[0:H1, CF_A1:CF_A1 + 1]
            c1_t = cf[0:H1, CF_C1:CF_C1 + 1]
            g2_t = cf[0:H2, CF_G2:CF_G2 + 1]
            bt2_t = cf[0:H2, CF_BT2:CF_BT2 + 1]
            g3_t = cf[0:H3, CF_G3:CF_G3 + 1]
            bt3_t = cf[0:H3, CF_BT3:CF_BT3 + 1]
            eps_t = cf[0:H2, CF_EPS:CF_EPS + 1]
            b4_t = cf[0:4, CF_B4:CF_B4 + 1]

            w1_t = cb[0:NU2, CB_W1:CB_W1 + H1]
            w1o_t = cb[ODD:ODD + NU2, CB_W1:CB_W1 + H1]
            w2_t = cb[0:H1, CB_W2:CB_W2 + H2]
            w3_t = cb[0:H1, CB_W3:CB_W3 + H2]
            w4_t = cb[0:H1, CB_W4:CB_W4 + 4]
            ssel_t = cb[0:16, CB_SSEL:CB_SSEL + 116]

            # warmups: absorb const DMA wait into each engine's clock
            with tc.tile_pool(name="warm", bufs=1, space="PSUM") as warmp:
                ps_w = warmp.tile([1, 1], F32)
                nc.tensor.matmul(ps_w, cb[0:1, 0:1], cb[0:1, 0:1],
                                 start=True, stop=True)
            scr_v = consts.tile([1, 1], F32)
            nc.vector.tensor_copy(scr_v, cf[0:1, 0:1])
            scr_s = consts.tile([1, 1], F32)
            nc.scalar.copy(scr_s, cf[0:1, 0:1])
            scr_g = consts.tile([1, 1], BF16)
            nc.gpsimd.tensor_copy(scr_g, cb[0:1, 0:1])

            Z2 = z2res.tile([H1, Z2C], BF16)
            acc2 = accp.tile([H1, NCH, 6], F32)

            dummy_n = [0]

            def pe_keepalive(psd_pool, n=2):
                for _ in range(n):
                    dummy_n[0] += 1
                    psd = psd_pool.tile([1, 512], F32, name=f"dum{dummy_n[0]}",
                                        tag="dum")
                    nc.tensor.matmul(psd[:, :], cb[0:1, 0:1], Z2[0:1, 0:512],
                                     start=True, stop=True, skip_group_check=True)

            # ---------------- pass 1: packed pairs (even chunk at
            # partitions 0:58, odd at 64:122 of one u tile; one broadcast
            # matmul + one is_equal covers both chunks)
            with (
                tc.tile_pool(name="catp", bufs=6) as catp,
                tc.tile_pool(name="up", bufs=4) as up,
                tc.tile_pool(name="h1p", bufs=4) as h1p,
                tc.tile_pool(name="psb", bufs=2, space="PSUM") as psb,
                tc.tile_pool(name="psz1", bufs=2, space="PSUM") as psz1,
                tc.tile_pool(name="psz2", bufs=2, space="PSUM") as psz2,
            ):
                for p in range(NPAIR):
                    pc = p * FD
                    cat_t = catp.tile([16, FD], BF16, tag="cat")
                    nc.sync.dma_start(out=cat_t, in_=catk[:, pc:pc + FD])
                    u_t = up.tile([122, FD], BF16, tag="u")
                    for sl in range(2):
                        pzb = psb.tile([116, 512], F32, tag="b")
                        nc.tensor.matmul(pzb[:, :], ssel_t,
                                         cat_t[:, sl * 512:(sl + 1) * 512],
                                         start=True, stop=True)
                        nc.vector.tensor_scalar(
                            u_t[0:116, sl * 512:(sl + 1) * 512],
                            pzb[:, :], iota_t, None, OP.is_equal)
                    # numerics overwrite the is_equal zeros at [52:58]/[116:122]
                    nc.sync.dma_start(out=u_t[NCAT2:NU2, :],
                                      in_=numk[0:6, pc:pc + FD])
                    nc.sync.dma_start(out=u_t[ODD + NCAT2:ODD + NU2, :],
                                      in_=numk[6:12, pc:pc + FD])
                    pz2s = [psz2.tile([H1, 512], F32, tag="z2",
                                      name=f"z2_{p}_{j}") for j in range(2)]
                    for h in range(2):
                        pz1 = psz1.tile([H1, FD], F32, tag="z1")
                        for j in range(0, FD, 512):
                            nc.tensor.matmul(
                                pz1[:, j:j + 512], w1o_t if h else w1_t,
                                u_t[ODD * h:ODD * h + NU2, j:j + 512],
                                start=True, stop=True)
                        h1_t = h1p.tile([H1, FD], BF16, tag="h1")
                        nc.scalar.activation(out=h1_t, in_=pz1[:, :],
                                             func=AF.Relu, bias=c1_t,
                                             scale=a1_t)
                        for j in range(2):
                            nc.tensor.matmul(
                                pz2s[j][h * H2:h * H2 + H2, :], w2_t,
                                h1_t[:, j * 512:(j + 1) * 512],
                                start=True, stop=True)
                    for j in range(2):
                        zc = slice(p * FD + j * 512, p * FD + (j + 1) * 512)
                        nc.scalar.copy(out=Z2[:, zc], in_=pz2s[j][:, :])
                        nc.vector.bn_stats(out=acc2[:, 2 * p + j, :],
                                           in_=Z2[:, zc])
            # ---------------- stats2 -> a2, coa2, W3p
            sm = accp
            mv2 = sm.tile([H1, 2], F32)
            nc.vector.bn_aggr(out=mv2, in_=acc2[:, :, :])

            sq2 = _small_f32(sm, H1)
            nc.vector.tensor_tensor(out=sq2, in0=mv2[:, 0:1], in1=mv2[:, 0:1],
                                    op=OP.mult)
            st2 = sm.tile([H1, 2], F32)
            nc.vector.tensor_copy(st2[:, 0:1], mv2[:, 0:1])
            nc.vector.tensor_tensor(out=st2[:, 1:2], in0=mv2[:, 1:2], in1=sq2,
                                    op=OP.add)
            hi2 = sm.tile([H2, 2], F32)
            nc.sync.dma_start(out=hi2, in_=st2[H2:H1, :])
            cm2 = sm.tile([H2, 2], F32)
            nc.vector.tensor_tensor(out=cm2, in0=st2[0:H2, :], in1=hi2[:, :],
                                    op=OP.add)
            nc.vector.tensor_scalar(cm2[:, :], cm2[:, :], 0.5, None, OP.mult)
            msq2 = _small_f32(sm, H2)
            nc.vector.tensor_tensor(out=msq2, in0=cm2[:, 0:1], in1=cm2[:, 0:1],
                                    op=OP.mult)
            var2 = _small_f32(sm, H2)
            nc.vector.tensor_tensor(out=var2, in0=cm2[:, 1:2], in1=msq2,
                                    op=OP.subtract)
            sd2 = _small_f32(sm, H2)
            nc.scalar.activation(out=sd2, in_=var2[:, :], func=AF.Sqrt,
                                 bias=eps_t, scale=1.0)
            inv2 = _small_f32(sm, H2)
            nc.vector.reciprocal(out=inv2, in_=sd2[:, :])
            a2v = _small_f32(sm, H2)
            nc.vector.tensor_tensor(out=a2v, in0=inv2[:, :], in1=g2_t,
                                    op=OP.mult)
            ra2 = _small_f32(sm, H2)
            nc.vector.reciprocal(out=ra2, in_=a2v[:, :])
            tb2 = _small_f32(sm, H2)
            nc.vector.tensor_tensor(out=tb2, in0=bt2_t, in1=ra2[:, :],
                                    op=OP.mult)
            coa2 = _small_f32(sm, H2)
            nc.vector.tensor_tensor(out=coa2, in0=tb2[:, :], in1=cm2[:, 0:1],
                                    op=OP.subtract)
            a2rep = _small_f32(sm, H1)
            nc.vector.tensor_copy(a2rep[0:H2, :], a2v[:, :])
            nc.sync.dma_start(out=a2rep[H2:H1, :], in_=a2v[:, :])
            coa2rep = _small_f32(sm, H1)
            nc.vector.tensor_copy(coa2rep[0:H2, :], coa2[:, :])
            nc.sync.dma_start(out=coa2rep[H2:H1, :], in_=coa2[:, :])
            w3p = sm.tile([H1, H2], BF16)
            nc.vector.tensor_scalar(w3p[:, :], w3_t, a2rep, None, OP.mult)

            acc3 = accp.tile([H1, NSLICE // 2, 6], F32)

            # ---------------- pass 2: in-place act2 on Z2, z3 (slice-parity
            # packed into [128,1024] PSUM via partition offsets), stats3
            with (
                tc.tile_pool(name="psz3", bufs=3, space="PSUM") as psz3,
            ):
                for t in range(NSLICE // 4):
                    pz3 = psz3.tile([H1, 1024], F32, tag="z3")
                    for kk in range(2):
                        a2s = slice((4 * t + 2 * kk) * 512,
                                    (4 * t + 2 * kk + 2) * 512)
                        nc.scalar.activation(out=Z2[:, a2s], in_=Z2[:, a2s],
                                             func=AF.Relu, bias=coa2rep,
                                             scale=1.0)
                    for k in range(4):
                        sl = 4 * t + k
                        zs = slice(sl * 512, (sl + 1) * 512)
                        po = H2 * (k % 2)
                        co = 512 * (k // 2)
                        nc.tensor.matmul(pz3[po:po + H2, co:co + 512],
                                         w3p[:, :], Z2[:, zs],
                                         start=True, stop=True)
                    for k in range(2):
                        nc.vector.bn_stats(out=acc3[:, 2 * t + k, :],
                                           in_=pz3[:, k * 512:(k + 1) * 512])

            # ---------------- stats3 -> a3, coa3, W4p
            mv3 = sm.tile([H1, 2], F32)
            nc.vector.bn_aggr(out=mv3, in_=acc3[:, :, :])
            sq3 = _small_f32(sm, H1)
            nc.vector.tensor_tensor(out=sq3, in0=mv3[:, 0:1], in1=mv3[:, 0:1],
                                    op=OP.mult)
            st3 = sm.tile([H1, 2], F32)
            nc.vector.tensor_copy(st3[:, 0:1], mv3[:, 0:1])
            nc.vector.tensor_tensor(out=st3[:, 1:2], in0=mv3[:, 1:2], in1=sq3,
                                    op=OP.add)
            hi3b = sm.tile([H2, 2], F32)
            nc.sync.dma_start(out=hi3b, in_=st3[H2:H1, :])
            cmA = sm.tile([H2, 2], F32)
            nc.vector.tensor_tensor(out=cmA, in0=st3[0:H2, :], in1=hi3b[:, :],
                                    op=OP.add)
            hi3 = sm.tile([H3, 2], F32)
            nc.sync.dma_start(out=hi3, in_=cmA[H3:H2, :])
            cm3 = sm.tile([H3, 2], F32)
            nc.vector.tensor_tensor(out=cm3, in0=cmA[0:H3, :], in1=hi3[:, :],
                                    op=OP.add)
            nc.vector.tensor_scalar(cm3[:, :], cm3[:, :], 0.25, None, OP.mult)
            msq3 = _small_f32(sm, H3)
            nc.vector.tensor_tensor(out=msq3, in0=cm3[:, 0:1], in1=cm3[:, 0:1],
                                    op=OP.mult)
            var3 = _small_f32(sm, H3)
            nc.vector.tensor_tensor(out=var3, in0=cm3[:, 1:2], in1=msq3,
                                    op=OP.subtract)
            sd3 = _small_f32(sm, H3)
            nc.scalar.activation(out=sd3, in_=var3[:, :], func=AF.Sqrt,
                                 bias=eps_t[0:H3, :], scale=1.0)
            inv3 = _small_f32(sm, H3)
            nc.vector.reciprocal(out=inv3, in_=sd3[:, :])
            a3v = _small_f32(sm, H3)
            nc.vector.tensor_tensor(out=a3v, in0=inv3[:, :], in1=g3_t,
                                    op=OP.mult)
            ra3 = _small_f32(sm, H3)
            nc.vector.reciprocal(out=ra3, in_=a3v[:, :])
            tb3 = _small_f32(sm, H3)
            nc.vector.tensor_tensor(out=tb3, in0=bt3_t, in1=ra3[:, :],
                                    op=OP.mult)
            coa3 = _small_f32(sm, H3)
            nc.vector.tensor_tensor(out=coa3, in0=tb3[:, :], in1=cm3[:, 0:1],
                                    op=OP.subtract)
            a3rep = _small_f32(sm, H1)
            nc.vector.tensor_copy(a3rep[0:H3, :], a3v[:, :])
            for g in range(1, 4):
                nc.sync.dma_start(out=a3rep[g * H3:(g + 1) * H3, :],
                                  in_=a3v[:, :])
            coa3rep = _small_f32(sm, H1)
            nc.vector.tensor_copy(coa3rep[0:H3, :], coa3[:, :])
            for g in range(1, 4):
                nc.sync.dma_start(out=coa3rep[g * H3:(g + 1) * H3, :],
                                  in_=coa3[:, :])
            w4p = sm.tile([H1, 4], BF16)
            nc.vector.tensor_scalar(w4p[:, :], w4_t, a3rep, None, OP.mult)

            # ---------------- pass 3: recompute z3, act3, z4, sigmoid, out
            with (
                tc.tile_pool(name="h3p", bufs=2) as h3p,
                tc.tile_pool(name="yp", bufs=2) as yp,
                tc.tile_pool(name="psz34", bufs=2, space="PSUM") as psz34,
                tc.tile_pool(name="psz4", bufs=2, space="PSUM") as psz4,
            ):
                for t4 in range(NSLICE // 4):
                    pz4 = psz4.tile([4, 1024], F32, tag="z4")
                    h3_ts = []
                    for t in range(t4 * 2, t4 * 2 + 2):
                        pz34 = psz34.tile([H1, 512], F32, tag="z34")
                        for sp in range(2):
                            s = 2 * t + sp
                            zs = slice(s * 512, (s + 1) * 512)
                            nc.tensor.matmul(pz34[sp * H2:(sp + 1) * H2, :],
                                             w3p[:, :], Z2[:, zs],
                                             start=True, stop=True)
                        h3_t = h3p.tile([H1, 512], BF16, tag="h3")
                        nc.scalar.activation(out=h3_t, in_=pz34[:, :],
                                             func=AF.Relu, bias=coa3rep,
                                             scale=1.0)
                        h3_ts.append(h3_t)
                    for k, h3_t in enumerate(h3_ts):
                        nc.tensor.matmul(pz4[:, k * 512:(k + 1) * 512],
                                         w4p[:, :], h3_t[:, :],
                                         start=True, stop=True)
                    y_t = yp.tile([4, 1024], F32, tag="y")
                    nc.scalar.activation(out=y_t, in_=pz4[:, :],
                                         func=AF.Sigmoid, bias=b4_t, scale=1.0)
                    nc.sync.dma_start(out=yout[:, t4 * 1024:(t4 + 1) * 1024],
                                      in_=y_t)
    return nc


def _get_program():
    if "F" not in _cache:
        _cache["F"] = build_fused()
    return _cache["F"]


def _run(prog, in_maps, cores, label):
    tr = bool(os.environ.get("BASS_KERNEL_TRACE"))
    r = run_bass_kernel_spmd(prog, in_maps, cores, trace=tr)
    if tr and r.exec_time_ns:
        _cache["hw_exec_ns"] = _cache.get("hw_exec_ns", 0) + r.exec_time_ns
        _cache[f"ns_{label}"] = r.exec_time_ns
        if r.instructions_and_trace:
            _cache[f"trace_{label}"] = r.instructions_and_trace[1]
    return r.results


def kernel(**inputs):
    inp = {k: np.asarray(v) for k, v in inputs.items()}
    cores = list(range(N_CORES))
    _cache.pop("hw_exec_ns", None)

    W1eff64 = _build_w1eff(inp["breed_emb"], inp["temp_emb"], inp["W1"])

    cats_all = [inp["pet1_breed"], inp["pet1_size"], inp["pet1_energy"],
                inp["pet1_temp"], inp["pet2_breed"], inp["pet2_size"],
                inp["pet2_energy"], inp["pet2_temp"]]
    nums_all = [inp["pet1_age"] / 15.0, inp["pet1_social"],
                inp["pet1_weight"] / 100.0, inp["pet2_age"] / 15.0,
                inp["pet2_social"], inp["pet2_weight"] / 100.0]
    nums_all = [np.asarray(x, np.float32) for x in nums_all]

    mu1, var1 = _host_stats1(cats_all, [x.astype(np.float64) for x in nums_all],
                             W1eff64)
    a1 = (np.asarray(inp["gamma1"], np.float64) / np.sqrt(var1 + EPS))
    c1 = (np.asarray(inp["beta1"], np.float64) - a1 * mu1)

    # recenter: z1 = b* + W1p^T u' with u' dropping each stream's last dim
    bstar = np.zeros(H1, np.float64)
    W1p = np.zeros((NU2, H1), np.float64)
    for i in range(8):
        o, k = CAT_OFFS[i], CAT_SIZES[i]
        o2, k2 = CAT2_OFFS[i], CAT2_SIZES[i]
        last = W1eff64[o + k - 1]
        bstar += last
        W1p[o2:o2 + k2] = W1eff64[o:o + k - 1] - last
    W1p[NCAT2:NU2] = W1eff64[NCAT:NU]
    c1p = c1 + a1 * bstar

    # packed selector [16, 116] and iota [122,1]: even chunk one-hot at
    # partitions 0:52 (+num 52:58), odd at 64:116 (+num 116:122)
    s_sel2 = np.zeros((16, 116), np.float32)
    iota2 = np.full((122, 1), 255.0, np.float32)
    for i in range(8):
        o2, k2 = CAT2_OFFS[i], CAT2_SIZES[i]
        s_sel2[i, o2:o2 + k2] = 1.0
        s_sel2[8 + i, ODD + o2:ODD + o2 + k2] = 1.0
        iota2[o2:o2 + k2, 0] = np.arange(k2, dtype=np.float32)
        iota2[ODD + o2:ODD + o2 + k2, 0] = np.arange(k2, dtype=np.float32)

    cbf = np.zeros((H1, CF_W), np.float32)
    cbf[0:122, CF_IOTA] = iota2[:, 0]
    cbf[0:H1, CF_A1] = a1.astype(np.float32)
    cbf[0:H1, CF_C1] = c1p.astype(np.float32)
    cbf[0:H2, CF_G2] = np.asarray(inp["gamma2"], np.float32)
    cbf[0:H2, CF_BT2] = np.asarray(inp["beta2"], np.float32)
    cbf[0:H3, CF_G3] = np.asarray(inp["gamma3"], np.float32)
    cbf[0:H3, CF_BT3] = np.asarray(inp["beta3"], np.float32)
    cbf[0:H2, CF_EPS] = EPS
    cbf[0:4, CF_B4] = float(np.asarray(inp["b4"]).reshape(-1)[0])

    W3blk = np.zeros((H1, H2), np.float32)
    W3 = np.asarray(inp["W3"], np.float32)
    W3blk[0:H2, 0:H3] = W3
    W3blk[H2:H1, H3:H2] = W3
    W4blk = np.zeros((H1, 4), np.float32)
    w4 = np.asarray(inp["W4"], np.float32)[:, 0]
    for g in range(4):
        W4blk[g * H3:(g + 1) * H3, g] = w4

    cbb = np.zeros((H1, CBW), np.float32)
    cbb[0:NU2, CB_W1:CB_W1 + H1] = W1p.astype(np.float32)
    cbb[ODD:ODD + NU2, CB_W1:CB_W1 + H1] = W1p.astype(np.float32)
    cbb[0:H1, CB_W2:CB_W2 + H2] = np.asarray(inp["W2"], np.float32)
    cbb[0:H1, CB_W3:CB_W3 + H2] = W3blk
    cbb[0:H1, CB_W4:CB_W4 + 4] = W4blk
    cbb[0:16, CB_SSEL:CB_SSEL + 116] = s_sel2
    cbb = cbb.astype(BF)

    prog = _get_program()

    cat_mat = np.stack(cats_all).astype(BF)       # [8, B]
    num_mat = np.stack(nums_all).astype(BF)       # [6, B]

    in_maps = []
    for c in cores:
        sl = slice(c * SHARD, (c + 1) * SHARD)
        # pack pairs: row s+8h = stream s of chunk-parity h, col p*FD+q
        cm = cat_mat[:, sl].reshape(8, NPAIR, 2, FD)
        cat2 = np.ascontiguousarray(
            cm.transpose(2, 0, 1, 3).reshape(16, SHARD // 2))
        nm = num_mat[:, sl].reshape(6, NPAIR, 2, FD)
        num2 = np.ascontiguousarray(
            nm.transpose(2, 0, 1, 3).reshape(12, SHARD // 2))
        in_maps.append({
            "catk": cat2, "numk": num2,
            "cbf": cbf, "cbb": cbb,
        })
    res = _run(prog, in_maps, cores, "F")

    perm = _cache.get("perm")
    if perm is None:
        perm = _out_perm()
        _cache["perm"] = perm
    out = np.empty(B, np.float32)
    for c in cores:
        out[c * SHARD:(c + 1) * SHARD] = res[c]["yout"].reshape(-1)[perm]
    return out


# revision 8
# speedup vs baseline: 1.3247x; 1.0098x over previous
"""Trainium2 fused single-launch kernel for nn_CompatibilityModel.

Data parallel over 8 cores, one NEFF launch, per-shard BN stats for
layers 2/3 (layer-1 stats exact on host via joint histograms).

Per core (131072 rows = 128 chunks x 1024):
  pass1: cat DMA [8,2048]/pair -> PE broadcast matmul -> DVE is_equal
         one-hot u[0:60] (numerics DMA'd to u[60:66]) -> PE z1 (K=66)
         -> ScalarE relu(a1*z1+c1) -> bf16 h1 -> PE z2 (pair-packed
         PSUM [128,1024]) -> ScalarE copy -> Z2 resident bf16
         [128,65536] -> DVE bn_stats.
  stats2 on device: bn_aggr + rowgroup combine -> a2=gamma/sigma,
         coa2=c2/a2; a2 folded into W3 (relu(a*z+c)=a*relu(z+c/a)).
  pass2: gpsimd in-place h2' = max(z2+coa2, 0) on Z2 -> PE z3
         (block-diag W3p, 4 slices/PSUM tile) -> DVE bn_stats3.
  stats3 combine -> a3, coa3; a3 folded into W4.
  pass3: PE z3 recomputed from resident h2' (slice-pairs packed
         [128,512] PSUM) -> ScalarE relu(z3+coa3) -> PE z4 (block-diag
         W4p, [4,2048] PSUM) -> ScalarE sigmoid(+b4) -> DMA out.

All matmuls bf16 (fp32 is 4 cycles/row on PE, bf16 is 1).
"""

import json
import os

import numpy as np
import ml_dtypes

import concourse.bass as bass
import concourse.mybir as mybir
import concourse.bass_utils as _bass_utils
import concourse.bass2jax as _bass2jax
from concourse.bass_utils import run_bass_kernel_spmd
from concourse.tile import TileContext

BF = ml_dtypes.bfloat16


# --------------------------------------------------------------------------- wait splitting
# This walrus build rejects instructions carrying more than one semaphore
# wait; split extras onto standalone EventSemaphore instructions.
def _split_multi_waits(bir_json: bytes) -> bytes:
    m = json.loads(bir_json)
    for f in m.get("functions", []):
        for bb in f.get("blocks", []):
            out = []
            for ins in bb.get("instructions", []):
                si = ins.get("sync_info") or {}
                ow = si.get("on_wait") or []
                if len(ow) > 1:
                    for k, w in enumerate(ow[:-1]):
                        out.append({
                            "name": f"{ins['name']}-wsplit{k}",
                            "opcode": "EventSemaphore",
                            "engine": ins["engine"],
                            "ins": [],
                            "outs": [],
                            "sync_info": {"on_update": [], "on_wait": [w]},
                        })
                    si["on_wait"] = [ow[-1]]
                out.append(ins)
            bb["instructions"] = out
    return json.dumps(m).encode()


_orig_compile_bir_kernel = _bass_utils.compile_bir_kernel


def _patched_compile_bir_kernel(bir_json, tmpdir, neff_name="file.neff"):
    return _orig_compile_bir_kernel(_split_multi_waits(bir_json), tmpdir, neff_name)


_bass_utils.compile_bir_kernel = _patched_compile_bir_kernel
_bass2jax.compile_bir_kernel = _patched_compile_bir_kernel

F32 = mybir.dt.float32
BF16 = mybir.dt.bfloat16
AF = mybir.ActivationFunctionType
OP = mybir.AluOpType

B = 1 << 20
N_CORES = 8
SHARD = B // N_CORES           # 131072
FD = 1024                      # rows per chunk
NCH = SHARD // FD              # 128
NPAIR = NCH // 2               # 64
Z2C = SHARD // 2               # 65536 Z2 columns
NSLICE = Z2C // 512            # 128
YC = SHARD // 4                # 32768 output columns

EMB = 8
N_BREEDS, N_TEMPS = 15, 9
CAT_SIZES = [N_BREEDS, 3, 3, N_TEMPS] * 2
CAT_OFFS = np.concatenate([[0], np.cumsum(CAT_SIZES)]).astype(int)
NCAT = int(CAT_OFFS[-1])       # 60
NU = 66
H1, H2, H3 = 128, 64, 32
EPS = 1e-5

# f32 const blob columns: [128, CF]
CF_IOTA = 0     # [60,1]
CF_A1 = 1       # [128,1]
CF_C1 = 2       # [128,1]
CF_G2 = 3       # [64,1]
CF_BT2 = 4      # [64,1]
CF_G3 = 5       # [32,1]
CF_BT3 = 6      # [32,1]
CF_EPS = 7      # [64,1]
CF_B4 = 8       # [4,1]
CF_W = 9

# bf16 const blob columns: [128, CBW]
CB_W1 = 0       # [58,128]  recentered cat dims + numerics
CB_W2 = 128     # [128,64]
CB_W3 = 192     # [128,64]  block-diag W3
CB_W4 = 256     # [128,4]   block-diag w4
CB_SSEL = 260   # [16,116]  packed-pair selector
CBW = 380

# packed one-hot: stream s keeps K_s-1 dims (last dim recentered into bias)
CAT2_SIZES = [k - 1 for k in CAT_SIZES]            # [14,2,2,8]*2
CAT2_OFFS = np.concatenate([[0], np.cumsum(CAT2_SIZES)]).astype(int)
NCAT2 = int(CAT2_OFFS[-1])     # 52
NU2 = NCAT2 + 6                # 58 per chunk
ODD = 64                       # odd-chunk partition base in packed tiles

_cache = {}


# ----------------------------------------------------------------------------- host math
def _build_w1eff(breed_emb, temp_emb, W1):
    A2 = np.zeros((NU, 50), np.float64)
    be = np.asarray(breed_emb, np.float64)
    te = np.asarray(temp_emb, np.float64)
    A2[0:15, 0:8] = be
    A2[15:18, 8:11] = np.eye(3)
    A2[18:21, 11:14] = np.eye(3)
    A2[21:30, 14:22] = te
    A2[30:45, 25:33] = be
    A2[45:48, 33:36] = np.eye(3)
    A2[48:51, 36:39] = np.eye(3)
    A2[51:60, 39:47] = te
    A2[60, 22] = 1.0
    A2[61, 23] = 1.0
    A2[62, 24] = 1.0
    A2[63, 47] = 1.0
    A2[64, 48] = 1.0
    A2[65, 49] = 1.0
    return A2 @ np.asarray(W1, np.float64)


def _host_stats1(cats, nums, W1eff):
    """Exact E[z1], Var[z1] via E[u], E[uu^T] in float64."""
    n = cats[0].shape[0]
    cats = [c.astype(np.int64) for c in cats]
    M = np.zeros((NU, NU), np.float64)
    Eu = np.zeros(NU, np.float64)
    for i, ci in enumerate(cats):
        Ki, oi = CAT_SIZES[i], CAT_OFFS[i]
        pi = np.bincount(ci, minlength=Ki) / n
        Eu[oi:oi + Ki] = pi
        M[oi:oi + Ki, oi:oi + Ki] = np.diag(pi)
        for j in range(i):
            Kj, oj = CAT_SIZES[j], CAT_OFFS[j]
            joint = np.bincount(ci * Kj + cats[j],
                                minlength=Ki * Kj).reshape(Ki, Kj) / n
            M[oi:oi + Ki, oj:oj + Kj] = joint
            M[oj:oj + Kj, oi:oi + Ki] = joint.T
        for j, xj in enumerate(nums):
            s = np.bincount(ci, weights=xj, minlength=Ki) / n
            M[oi:oi + Ki, NCAT + j] = s
            M[NCAT + j, oi:oi + Ki] = s
    for i, xi in enumerate(nums):
        Eu[NCAT + i] = xi.mean(dtype=np.float64)
        for j, xj in enumerate(nums):
            if j <= i:
                v = np.dot(xi, xj) / n
                M[NCAT + i, NCAT + j] = v
                M[NCAT + j, NCAT + i] = v
    Ez = W1eff.T @ Eu
    Ez2 = np.sum(W1eff * (M @ W1eff), axis=0)
    return Ez, Ez2 - Ez * Ez


def _out_perm():
    """row r of a shard -> flat index into y_dev [4, YC]."""
    r = np.arange(SHARD)
    c = r // FD
    q = r % FD
    x = (c // 2) * FD + q          # Z2 column
    j = x // 512                   # slice
    sx = x % 512
    t = j // 2                     # slice pair
    sp = j % 2
    g = 2 * sp + (c % 2)
    return g * YC + t * 512 + sx


# ----------------------------------------------------------------------------- program
_SMALL_N = [0]


def _small_f32(pool, n):
    _SMALL_N[0] += 1
    return pool.tile([n, 1], F32, name=f"sm{_SMALL_N[0]}")


def build_fused():
    nc = bass.Bass()
    catk = nc.dram_tensor("catk", [16, SHARD // 2], BF16, kind="ExternalInput")
    numk = nc.dram_tensor("numk", [12, SHARD // 2], BF16, kind="ExternalInput")
    cbf = nc.dram_tensor("cbf", [H1, CF_W], F32, kind="ExternalInput")
    cbb = nc.dram_tensor("cbb", [H1, CBW], BF16, kind="ExternalInput")
    yout = nc.dram_tensor("yout", [4, YC], F32, kind="ExternalOutput")

    with TileContext(nc) as tc:
        with (
            tc.tile_pool(name="consts", bufs=1) as consts,
            tc.tile_pool(name="z2res", bufs=1) as z2res,
            tc.tile_pool(name="accp", bufs=1) as accp,
        ):
            cf = consts.tile([H1, CF_W], F32)
            nc.sync.dma_start(out=cf, in_=cbf[:, :])
            cb = consts.tile([H1, CBW], BF16)
            nc.scalar.dma_start(out=cb, in_=cbb[:, :])

            iota_t = cf[0:116, CF_IOTA:CF_IOTA + 1]
            a1_t = cf[0:H1, CF_A1:CF_A1 + 1]
            c1_t = cf[0:H1, CF_C1:CF_C1 + 1]
            g2_t = cf[0:H2, CF_G2:CF_G2 + 1]
            bt2_t = cf[0:H2, CF_BT2:CF_BT2 + 1]
            g3_t = cf[0:H3, CF_G3:CF_G3 + 1]
            bt3_t = cf[0:H3, CF_BT3:CF_BT3 + 1]
            eps_t = cf[0:H2, CF_EPS:CF_EPS + 1]
            b4_t = cf[0:4, CF_B4:CF_B4 + 1]

            w1_t = cb[0:NU2, CB_W1:CB_W1 + H1]
            w1o_t = cb[ODD:ODD + NU2, CB_W1:CB_W1 + H1]
            w2_t = cb[0:H1, CB_W2:CB_W2 + H2]
            w3_t = cb[0:H1, CB_W3:CB_W3 + H2]
            w4_t = cb[0:H1, CB_W4:CB_W4 + 4]
            ssel_t = cb[0:16, CB_SSEL:CB_SSEL + 116]

            # warmups: absorb const DMA wait into each engine's clock
            with tc.tile_pool(name="warm", bufs=1, space="PSUM") as warmp:
                ps_w = warmp.tile([1, 1], F32)
                nc.tensor.matmul(ps_w, cb[0:1, 0:1], cb[0:1, 0:1],
                                 start=True, stop=True)
            scr_v = consts.tile([1, 1], F32)
            nc.vector.tensor_copy(scr_v, cf[0:1, 0:1])
            scr_s = consts.tile([1, 1], F32)
            nc.scalar.copy(scr_s, cf[0:1, 0:1])
            scr_g = consts.tile([1, 1], BF16)
            nc.gpsimd.tensor_copy(scr_g, cb[0:1, 0:1])

            Z2 = z2res.tile([H1, Z2C], BF16)
            acc2 = accp.tile([H1, NCH, 6], F32)

            dummy_n = [0]

            def pe_keepalive(psd_pool, n=2):
                for _ in range(n):
                    dummy_n[0] += 1
                    psd = psd_pool.tile([1, 512], F32, name=f"dum{dummy_n[0]}",
                                        tag="dum")
                    nc.tensor.matmul(psd[:, :], cb[0:1, 0:1], Z2[0:1, 0:512],
                                     start=True, stop=True, skip_group_check=True)

            # ---------------- pass 1: packed pairs (even chunk at
            # partitions 0:58, odd at 64:122 of one u tile; one broadcast
            # matmul + one is_equal covers both chunks)
            with (
                tc.tile_pool(name="catp", bufs=6) as catp,
                tc.tile_pool(name="up", bufs=4) as up,
                tc.tile_pool(name="h1p", bufs=4) as h1p,
                tc.tile_pool(name="psb", bufs=2, space="PSUM") as psb,
                tc.tile_pool(name="psz1", bufs=2, space="PSUM") as psz1,
                tc.tile_pool(name="psz2", bufs=2, space="PSUM") as psz2,
            ):
                for p in range(NPAIR):
                    pc = p * FD
                    cat_t = catp.tile([16, FD], BF16, tag="cat")
                    nc.sync.dma_start(out=cat_t, in_=catk[:, pc:pc + FD])
                    u_t = up.tile([122, FD], BF16, tag="u")
                    for sl in range(2):
                        pzb = psb.tile([116, 512], F32, tag="b")
                        nc.tensor.matmul(pzb[:, :], ssel_t,
                                         cat_t[:, sl * 512:(sl + 1) * 512],
                                         start=True, stop=True)
                        nc.vector.tensor_scalar(
                            u_t[0:116, sl * 512:(sl + 1) * 512],
                            pzb[:, :], iota_t, None, OP.is_equal)
                    # numerics overwrite the is_equal zeros at [52:58]/[116:122]
                    nc.sync.dma_start(out=u_t[NCAT2:NU2, :],
                                      in_=numk[0:6, pc:pc + FD])
                    nc.sync.dma_start(out=u_t[ODD + NCAT2:ODD + NU2, :],
                                      in_=numk[6:12, pc:pc + FD])
                    pz2s = [psz2.tile([H1, 512], F32, tag="z2",
                                      name=f"z2_{p}_{j}") for j in range(2)]
                    for h in range(2):
                        pz1 = psz1.tile([H1, FD], F32, tag="z1")
                        for j in range(0, FD, 512):
                            nc.tensor.matmul(
                                pz1[:, j:j + 512], w1o_t if h else w1_t,
                                u_t[ODD * h:ODD * h + NU2, j:j + 512],
                                start=True, stop=True)
                        h1_t = h1p.tile([H1, FD], BF16, tag="h1")
                        nc.scalar.activation(out=h1_t, in_=pz1[:, :],
                                             func=AF.Relu, bias=c1_t,
                                             scale=a1_t)
                        for j in range(2):
                            nc.tensor.matmul(
                                pz2s[j][h * H2:h * H2 + H2, :], w2_t,
                                h1_t[:, j * 512:(j + 1) * 512],
                                start=True, stop=True)
                    for j in range(2):
                        zc = slice(p * FD + j * 512, p * FD + (j + 1) * 512)
                        nc.scalar.copy(out=Z2[:, zc], in_=pz2s[j][:, :])
                        nc.vector.bn_stats(out=acc2[:, 2 * p + j, :],
                                           in_=Z2[:, zc])
            # ---------------- stats2 -> a2, coa2, W3p
            sm = accp
            mv2 = sm.tile([H1, 2], F32)
            nc.vector.bn_aggr(out=mv2, in_=acc2[:, :, :])

            sq2 = _small_f32(sm, H1)
            nc.vector.tensor_tensor(out=sq2, in0=mv2[:, 0:1], in1=mv2[:, 0:1],
                                    op=OP.mult)
            st2 = sm.tile([H1, 2], F32)
            nc.vector.tensor_copy(st2[:, 0:1], mv2[:, 0:1])
            nc.vector.tensor_tensor(out=st2[:, 1:2], in0=mv2[:, 1:2], in1=sq2,
                                    op=OP.add)
            hi2 = sm.tile([H2, 2], F32)
            nc.sync.dma_start(out=hi2, in_=st2[H2:H1, :])
            cm2 = sm.tile([H2, 2], F32)
            nc.vector.tensor_tensor(out=cm2, in0=st2[0:H2, :], in1=hi2[:, :],
                                    op=OP.add)
            nc.vector.tensor_scalar(cm2[:, :], cm2[:, :], 0.5, None, OP.mult)
            msq2 = _small_f32(sm, H2)
            nc.vector.tensor_tensor(out=msq2, in0=cm2[:, 0:1], in1=cm2[:, 0:1],
                                    op=OP.mult)
            var2 = _small_f32(sm, H2)
            nc.vector.tensor_tensor(out=var2, in0=cm2[:, 1:2], in1=msq2,
                                    op=OP.subtract)
            sd2 = _small_f32(sm, H2)
            nc.scalar.activation(out=sd2, in_=var2[:, :], func=AF.Sqrt,
                                 bias=eps_t, scale=1.0)
            inv2 = _small_f32(sm, H2)
            nc.vector.reciprocal(out=inv2, in_=sd2[:, :])
            a2v = _small_f32(sm, H2)
            nc.vector.tensor_tensor(out=a2v, in0=inv2[:, :], in1=g2_t,
                                    op=OP.mult)
            ra2 = _small_f32(sm, H2)
            nc.vector.reciprocal(out=ra2, in_=a2v[:, :])
            tb2 = _small_f32(sm, H2)
            nc.vector.tensor_tensor(out=tb2, in0=bt2_t, in1=ra2[:, :],
                                    op=OP.mult)
            coa2 = _small_f32(sm, H2)
            nc.vector.tensor_tensor(out=coa2, in0=tb2[:, :], in1=cm2[:, 0:1],
                                    op=OP.subtract)
            a2rep = _small_f32(sm, H1)
            nc.vector.tensor_copy(a2rep[0:H2, :], a2v[:, :])
            nc.sync.dma_start(out=a2rep[H2:H1, :], in_=a2v[:, :])
            coa2rep = _small_f32(sm, H1)
            nc.vector.tensor_copy(coa2rep[0:H2, :], coa2[:, :])
            nc.sync.dma_start(out=coa2rep[H2:H1, :], in_=coa2[:, :])
            w3p = sm.tile([H1, H2], BF16)
            nc.vector.tensor_scalar(w3p[:, :], w3_t, a2rep, None, OP.mult)

            acc3 = accp.tile([H1, NSLICE // 2, 6], F32)

            # ---------------- pass 2: in-place act2 on Z2, z3 (slice-parity
            # packed into [128,1024] PSUM via partition offsets), stats3
            with (
                tc.tile_pool(name="psz3", bufs=4, space="PSUM") as psz3,
            ):
                for t in range(NSLICE // 4):
                    pz3 = psz3.tile([H1, 1024], F32, tag="z3")
                    for kk in range(2):
                        a2s = slice((4 * t + 2 * kk) * 512,
                                    (4 * t + 2 * kk + 2) * 512)
                        nc.scalar.activation(out=Z2[:, a2s], in_=Z2[:, a2s],
                                             func=AF.Relu, bias=coa2rep,
                                             scale=1.0)
                    for k in range(4):
                        sl = 4 * t + k
                        zs = slice(sl * 512, (sl + 1) * 512)
                        po = H2 * (k % 2)
                        co = 512 * (k // 2)
                        nc.tensor.matmul(pz3[po:po + H2, co:co + 512],
                                         w3p[:, :], Z2[:, zs],
                                         start=True, stop=True)
                    for k in range(2):
                        nc.vector.bn_stats(out=acc3[:, 2 * t + k, :],
                                           in_=pz3[:, k * 512:(k + 1) * 512])

            # ---------------- stats3 -> a3, coa3, W4p
            mv3 = sm.tile([H1, 2], F32)
            nc.vector.bn_aggr(out=mv3, in_=acc3[:, :, :])
            sq3 = _small_f32(sm, H1)
            nc.vector.tensor_tensor(out=sq3, in0=mv3[:, 0:1], in1=mv3[:, 0:1],
                                    op=OP.mult)
            st3 = sm.tile([H1, 2], F32)
            nc.vector.tensor_copy(st3[:, 0:1], mv3[:, 0:1])
            nc.vector.tensor_tensor(out=st3[:, 1:2], in0=mv3[:, 1:2], in1=sq3,
                                    op=OP.add)
            hi3b = sm.tile([H2, 2], F32)
            nc.sync.dma_start(out=hi3b, in_=st3[H2:H1, :])
            cmA = sm.tile([H2, 2], F32)
            nc.vector.tensor_tensor(out=cmA, in0=st3[0:H2, :], in1=hi3b[:, :],
                                    op=OP.add)
            hi3 = sm.tile([H3, 2], F32)
            nc.sync.dma_start(out=hi3, in_=cmA[H3:H2, :])
            cm3 = sm.tile([H3, 2], F32)
            nc.vector.tensor_tensor(out=cm3, in0=cmA[0:H3, :], in1=hi3[:, :],
                                    op=OP.add)
            nc.vector.tensor_scalar(cm3[:, :], cm3[:, :], 0.25, None, OP.mult)
            msq3 = _small_f32(sm, H3)
            nc.vector.tensor_tensor(out=msq3, in0=cm3[:, 0:1], in1=cm3[:, 0:1],
                                    op=OP.mult)
            var3 = _small_f32(sm, H3)
            nc.vector.tensor_tensor(out=var3, in0=cm3[:, 1:2], in1=msq3,
                                    op=OP.subtract)
            sd3 = _small_f32(sm, H3)
            nc.scalar.activation(out=sd3, in_=var3[:, :], func=AF.Sqrt,
                                 bias=eps_t[0:H3, :], scale=1.0)
            inv3 = _small_f32(sm, H3)
            nc.vector.reciprocal(out=inv3, in_=sd3[:, :])
            a3v = _small_f32(sm, H3)
            nc.vector.tensor_tensor(out=a3v, in0=inv3[:, :], in1=g3_t,
                                    op=OP.mult)
            ra3 = _small_f32(sm, H3)
            nc.vector.reciprocal(out=ra3, in_=a3v[:, :])
            tb3 = _small_f32(sm, H3)
            nc.vector.tensor_tensor(out=tb3, in0=bt3_t, in1=ra3[:, :],
                                    op=OP.mult)
            coa3 = _small_f32(sm, H3)
            nc.vector.tensor_tensor(out=coa3, in0=tb3[:, :], in1=cm3[:, 0:1],
                                    op=OP.subtract)
            a3rep = _small_f32(sm, H1)
            nc.vector.tensor_copy(a3rep[0:H3, :], a3v[:, :])
            for g in range(1, 4):
                nc.sync.dma_start(out=a3rep[g * H3:(g + 1) * H3, :],
                                  in_=a3v[:, :])
            coa3rep = _small_f32(sm, H1)
            nc.vector.tensor_copy(coa3rep[0:H3, :], coa3[:, :])
            for g in range(1, 4):
                nc.sync.dma_start(out=coa3rep[g * H3:(g + 1) * H3, :],
                                  in_=coa3[:, :])
            w4p = sm.tile([H1, 4], BF16)
            nc.vector.tensor_scalar(w4p[:, :], w4_t, a3rep, None, OP.mult)

            # ---------------- pass 3: recompute z3, act3, z4, sigmoid, out
            with (
                tc.tile_pool(name="h3p", bufs=2) as h3p,
                tc.tile_pool(name="yp", bufs=2) as yp,
                tc.tile_pool(name="psz34", bufs=2, space="PSUM") as psz34,
                tc.tile_pool(name="psz4", bufs=2, space="PSUM") as psz4,
            ):
                for t4 in range(NSLICE // 4):
                    pz4 = psz4.tile([4, 1024], F32, tag="z4")
                    pz34 = psz34.tile([H1, 1024], F32, tag="z34")
                    for t in range(t4 * 2, t4 * 2 + 2):
                        co = 512 * (t % 2)
                        for sp in range(2):
                            s = 2 * t + sp
                            zs = slice(s * 512, (s + 1) * 512)
                            nc.tensor.matmul(
                                pz34[sp * H2:(sp + 1) * H2, co:co + 512],
                                w3p[:, :], Z2[:, zs],
                                start=True, stop=True)
                    h3_t = h3p.tile([H1, 1024], BF16, tag="h3")
                    nc.scalar.activation(out=h3_t, in_=pz34[:, :],
                                         func=AF.Relu, bias=coa3rep,
                                         scale=1.0)
                    for k in range(2):
                        nc.tensor.matmul(pz4[:, k * 512:(k + 1) * 512],
                                         w4p[:, :],
                                         h3_t[:, k * 512:(k + 1) * 512],
                                         start=True, stop=True)
                    y_t = yp.tile([4, 1024], F32, tag="y")
                    nc.scalar.activation(out=y_t, in_=pz4[:, :],
                                         func=AF.Sigmoid, bias=b4_t, scale=1.0)
                    nc.sync.dma_start(out=yout[:, t4 * 1024:(t4 + 1) * 1024],
                                      in_=y_t)
    return nc


def _get_program():
    if "F" not in _cache:
        _cache["F"] = build_fused()
    return _cache["F"]


def _run(prog, in_maps, cores, label):
    tr = bool(os.environ.get("BASS_KERNEL_TRACE"))
    r = run_bass_kernel_spmd(prog, in_maps, cores, trace=tr)
    if tr and r.exec_time_ns:
        _cache["hw_exec_ns"] = _cache.get("hw_exec_ns", 0) + r.exec_time_ns
        _cache[f"ns_{label}"] = r.exec_time_ns
        if r.instructions_and_trace:
            _cache[f"trace_{label}"] = r.instructions_and_trace[1]
    return r.results


def kernel(**inputs):
    inp = {k: np.asarray(v) for k, v in inputs.items()}
    cores = list(range(N_CORES))
    _cache.pop("hw_exec_ns", None)

    W1eff64 = _build_w1eff(inp["breed_emb"], inp["temp_emb"], inp["W1"])

    cats_all = [inp["pet1_breed"], inp["pet1_size"], inp["pet1_energy"],
                inp["pet1_temp"], inp["pet2_breed"], inp["pet2_size"],
                inp["pet2_energy"], inp["pet2_temp"]]
    nums_all = [inp["pet1_age"] / 15.0, inp["pet1_social"],
                inp["pet1_weight"] / 100.0, inp["pet2_age"] / 15.0,
                inp["pet2_social"], inp["pet2_weight"] / 100.0]
    nums_all = [np.asarray(x, np.float32) for x in nums_all]

    mu1, var1 = _host_stats1(cats_all, [x.astype(np.float64) for x in nums_all],
                             W1eff64)
    a1 = (np.asarray(inp["gamma1"], np.float64) / np.sqrt(var1 + EPS))
    c1 = (np.asarray(inp["beta1"], np.float64) - a1 * mu1)

    # recenter: z1 = b* + W1p^T u' with u' dropping each stream's last dim
    bstar = np.zeros(H1, np.float64)
    W1p = np.zeros((NU2, H1), np.float64)
    for i in range(8):
        o, k = CAT_OFFS[i], CAT_SIZES[i]
        o2, k2 = CAT2_OFFS[i], CAT2_SIZES[i]
        last = W1eff64[o + k - 1]
        bstar += last
        W1p[o2:o2 + k2] = W1eff64[o:o + k - 1] - last
    W1p[NCAT2:NU2] = W1eff64[NCAT:NU]
    c1p = c1 + a1 * bstar

    # packed selector [16, 116] and iota [122,1]: even chunk one-hot at
    # partitions 0:52 (+num 52:58), odd at 64:116 (+num 116:122)
    s_sel2 = np.zeros((16, 116), np.float32)
    iota2 = np.full((122, 1), 255.0, np.float32)
    for i in range(8):
        o2, k2 = CAT2_OFFS[i], CAT2_SIZES[i]
        s_sel2[i, o2:o2 + k2] = 1.0
        s_sel2[8 + i, ODD + o2:ODD + o2 + k2] = 1.0
        iota2[o2:o2 + k2, 0] = np.arange(k2, dtype=np.float32)
        iota2[ODD + o2:ODD + o2 + k2, 0] = np.arange(k2, dtype=np.float32)

    cbf = np.zeros((H1, CF_W), np.float32)
    cbf[0:122, CF_IOTA] = iota2[:, 0]
    cbf[0:H1, CF_A1] = a1.astype(np.float32)
    cbf[0:H1, CF_C1] = c1p.astype(np.float32)
    cbf[0:H2, CF_G2] = np.asarray(inp["gamma2"], np.float32)
    cbf[0:H2, CF_BT2] = np.asarray(inp["beta2"], np.float32)
    cbf[0:H3, CF_G3] = np.asarray(inp["gamma3"], np.float32)
    cbf[0:H3, CF_BT3] = np.asarray(inp["beta3"], np.float32)
    cbf[0:H2, CF_EPS] = EPS
    cbf[0:4, CF_B4] = float(np.asarray(inp["b4"]).reshape(-1)[0])

    W3blk = np.zeros((H1, H2), np.float32)
    W3 = np.asarray(inp["W3"], np.float32)
    W3blk[0:H2, 0:H3] = W3
    W3blk[H2:H1, H3:H2] = W3
    W4blk = np.zeros((H1, 4), np.float32)
    w4 = np.asarray(inp["W4"], np.float32)[:, 0]
    for g in range(4):
        W4blk[g * H3:(g + 1) * H3, g] = w4

    cbb = np.zeros((H1, CBW), np.float32)
    cbb[0:NU2, CB_W1:CB_W1 + H1] = W1p.astype(np.float32)
    cbb[ODD:ODD + NU2, CB_W1:CB_W1 + H1] = W1p.astype(np.float32)
    cbb[0:H1, CB_W2:CB_W2 + H2] = np.asarray(inp["W2"], np.float32)
    cbb[0:H1, CB_W3:CB_W3 + H2] = W3blk
    cbb[0:H1, CB_W4:CB_W4 + 4] = W4blk
    cbb[0:16, CB_SSEL:CB_SSEL + 116] = s_sel2
    cbb = cbb.astype(BF)

    prog = _get_program()

    cat_mat = np.stack(cats_all).astype(BF)       # [8, B]
    num_mat = np.stack(nums_all).astype(BF)       # [6, B]

    in_maps = []
    for c in cores:
        sl = slice(c * SHARD, (c + 1) * SHARD)
        # pack pairs: row s+8h = stream s of chunk-parity h, col p*FD+q
        cm = cat_mat[:, sl].reshape(8, NPAIR, 2, FD)
        cat2 = np.ascontiguousarray(
            cm.transpose(2, 0, 1, 3).reshape(16, SHARD // 2))
        nm = num_mat[:, sl].reshape(6, NPAIR, 2, FD)
        num2 = np.ascontiguousarray(
            nm.transpose(2, 0, 1, 3).reshape(12, SHARD // 2))
        in_maps.append({
            "catk": cat2, "numk": num2,
            "cbf": cbf, "cbb": cbb,
        })
    res = _run(prog, in_maps, cores, "F")

    perm = _cache.get("perm")
    if perm is None:
        perm = _out_perm()
        _cache["perm"] = perm
    out = np.empty(B, np.float32)
    for c in cores:
        out[c * SHARD:(c + 1) * SHARD] = res[c]["yout"].reshape(-1)[perm]
    return out


# revision 9
# speedup vs baseline: 1.3257x; 1.0007x over previous
"""Trainium2 fused single-launch kernel for nn_CompatibilityModel.

Data parallel over 8 cores, one NEFF launch, per-shard BN stats for
layers 2/3 (layer-1 stats exact on host via joint histograms).

Per core (131072 rows = 128 chunks x 1024):
  pass1: cat DMA [8,2048]/pair -> PE broadcast matmul -> DVE is_equal
         one-hot u[0:60] (numerics DMA'd to u[60:66]) -> PE z1 (K=66)
         -> ScalarE relu(a1*z1+c1) -> bf16 h1 -> PE z2 (pair-packed
         PSUM [128,1024]) -> ScalarE copy -> Z2 resident bf16
         [128,65536] -> DVE bn_stats.
  stats2 on device: bn_aggr + rowgroup combine -> a2=gamma/sigma,
         coa2=c2/a2; a2 folded into W3 (relu(a*z+c)=a*relu(z+c/a)).
  pass2: gpsimd in-place h2' = max(z2+coa2, 0) on Z2 -> PE z3
         (block-diag W3p, 4 slices/PSUM tile) -> DVE bn_stats3.
  stats3 combine -> a3, coa3; a3 folded into W4.
  pass3: PE z3 recomputed from resident h2' (slice-pairs packed
         [128,512] PSUM) -> ScalarE relu(z3+coa3) -> PE z4 (block-diag
         W4p, [4,2048] PSUM) -> ScalarE sigmoid(+b4) -> DMA out.

All matmuls bf16 (fp32 is 4 cycles/row on PE, bf16 is 1).
"""

import json
import os

import numpy as np
import ml_dtypes

import concourse.bass as bass
import concourse.mybir as mybir
import concourse.bass_utils as _bass_utils
import concourse.bass2jax as _bass2jax
from concourse.bass_utils import run_bass_kernel_spmd
from concourse.tile import TileContext

BF = ml_dtypes.bfloat16


# --------------------------------------------------------------------------- wait splitting
# This walrus build rejects instructions carrying more than one semaphore
# wait; split extras onto standalone EventSemaphore instructions.
def _split_multi_waits(bir_json: bytes) -> bytes:
    m = json.loads(bir_json)
    for f in m.get("functions", []):
        for bb in f.get("blocks", []):
            out = []
            for ins in bb.get("instructions", []):
                si = ins.get("sync_info") or {}
                ow = si.get("on_wait") or []
                if len(ow) > 1:
                    for k, w in enumerate(ow[:-1]):
                        out.append({
                            "name": f"{ins['name']}-wsplit{k}",
                            "opcode": "EventSemaphore",
                            "engine": ins["engine"],
                            "ins": [],
                            "outs": [],
                            "sync_info": {"on_update": [], "on_wait": [w]},
                        })
                    si["on_wait"] = [ow[-1]]
                out.append(ins)
            bb["instructions"] = out
    return json.dumps(m).encode()


_orig_compile_bir_kernel = _bass_utils.compile_bir_kernel


def _patched_compile_bir_kernel(bir_json, tmpdir, neff_name="file.neff"):
    return _orig_compile_bir_kernel(_split_multi_waits(bir_json), tmpdir, neff_name)


_bass_utils.compile_bir_kernel = _patched_compile_bir_kernel
_bass2jax.compile_bir_kernel = _patched_compile_bir_kernel

F32 = mybir.dt.float32
BF16 = mybir.dt.bfloat16
AF = mybir.ActivationFunctionType
OP = mybir.AluOpType

B = 1 << 20
N_CORES = 8
SHARD = B // N_CORES           # 131072
FD = 1024                      # rows per chunk
NCH = SHARD // FD              # 128
NPAIR = NCH // 2               # 64
Z2C = SHARD // 2               # 65536 Z2 columns
NSLICE = Z2C // 512            # 128
YC = SHARD // 4                # 32768 output columns

EMB = 8
N_BREEDS, N_TEMPS = 15, 9
CAT_SIZES = [N_BREEDS, 3, 3, N_TEMPS] * 2
CAT_OFFS = np.concatenate([[0], np.cumsum(CAT_SIZES)]).astype(int)
NCAT = int(CAT_OFFS[-1])       # 60
NU = 66
H1, H2, H3 = 128, 64, 32
EPS = 1e-5

# f32 const blob columns: [128, CF]
CF_IOTA = 0     # [60,1]
CF_A1 = 1       # [128,1]
CF_C1 = 2       # [128,1]
CF_G2 = 3       # [64,1]
CF_BT2 = 4      # [64,1]
CF_G3 = 5       # [32,1]
CF_BT3 = 6      # [32,1]
CF_EPS = 7      # [64,1]
CF_B4 = 8       # [4,1]
CF_W = 9

# bf16 const blob columns: [128, CBW]
CB_W1 = 0       # [58,128]  recentered cat dims + numerics
CB_W2 = 128     # [128,64]
CB_W3 = 192     # [128,64]  block-diag W3
CB_W4 = 256     # [128,4]   block-diag w4
CB_SSEL = 260   # [16,116]  packed-pair selector
CBW = 380

# packed one-hot: stream s keeps K_s-1 dims (last dim recentered into bias)
CAT2_SIZES = [k - 1 for k in CAT_SIZES]            # [14,2,2,8]*2
CAT2_OFFS = np.concatenate([[0], np.cumsum(CAT2_SIZES)]).astype(int)
NCAT2 = int(CAT2_OFFS[-1])     # 52
NU2 = NCAT2 + 6                # 58 per chunk
ODD = 64                       # odd-chunk partition base in packed tiles

_cache = {}


# ----------------------------------------------------------------------------- host math
def _build_w1eff(breed_emb, temp_emb, W1):
    A2 = np.zeros((NU, 50), np.float64)
    be = np.asarray(breed_emb, np.float64)
    te = np.asarray(temp_emb, np.float64)
    A2[0:15, 0:8] = be
    A2[15:18, 8:11] = np.eye(3)
    A2[18:21, 11:14] = np.eye(3)
    A2[21:30, 14:22] = te
    A2[30:45, 25:33] = be
    A2[45:48, 33:36] = np.eye(3)
    A2[48:51, 36:39] = np.eye(3)
    A2[51:60, 39:47] = te
    A2[60, 22] = 1.0
    A2[61, 23] = 1.0
    A2[62, 24] = 1.0
    A2[63, 47] = 1.0
    A2[64, 48] = 1.0
    A2[65, 49] = 1.0
    return A2 @ np.asarray(W1, np.float64)


def _host_stats1(cats, nums, W1eff):
    """Exact E[z1], Var[z1] via E[u], E[uu^T] in float64."""
    n = cats[0].shape[0]
    cats = [c.astype(np.int64) for c in cats]
    M = np.zeros((NU, NU), np.float64)
    Eu = np.zeros(NU, np.float64)
    for i, ci in enumerate(cats):
        Ki, oi = CAT_SIZES[i], CAT_OFFS[i]
        pi = np.bincount(ci, minlength=Ki) / n
        Eu[oi:oi + Ki] = pi
        M[oi:oi + Ki, oi:oi + Ki] = np.diag(pi)
        for j in range(i):
            Kj, oj = CAT_SIZES[j], CAT_OFFS[j]
            joint = np.bincount(ci * Kj + cats[j],
                                minlength=Ki * Kj).reshape(Ki, Kj) / n
            M[oi:oi + Ki, oj:oj + Kj] = joint
            M[oj:oj + Kj, oi:oi + Ki] = joint.T
        for j, xj in enumerate(nums):
            s = np.bincount(ci, weights=xj, minlength=Ki) / n
            M[oi:oi + Ki, NCAT + j] = s
            M[NCAT + j, oi:oi + Ki] = s
    for i, xi in enumerate(nums):
        Eu[NCAT + i] = xi.mean(dtype=np.float64)
        for j, xj in enumerate(nums):
            if j <= i:
                v = np.dot(xi, xj) / n
                M[NCAT + i, NCAT + j] = v
                M[NCAT + j, NCAT + i] = v
    Ez = W1eff.T @ Eu
    Ez2 = np.sum(W1eff * (M @ W1eff), axis=0)
    return Ez, Ez2 - Ez * Ez


def _out_perm():
    """row r of a shard -> flat index into y_dev [4, YC]."""
    r = np.arange(SHARD)
    c = r // FD
    q = r % FD
    x = (c // 2) * FD + q          # Z2 column
    j = x // 512                   # slice
    sx = x % 512
    t = j // 2                     # slice pair
    sp = j % 2
    g = 2 * sp + (c % 2)
    return g * YC + t * 512 + sx


# ----------------------------------------------------------------------------- program
_SMALL_N = [0]


def _small_f32(pool, n):
    _SMALL_N[0] += 1
    return pool.tile([n, 1], F32, name=f"sm{_SMALL_N[0]}")


def build_fused():
    nc = bass.Bass()
    catk = nc.dram_tensor("catk", [16, SHARD // 2], BF16, kind="ExternalInput")
    numk = nc.dram_tensor("numk", [12, SHARD // 2], BF16, kind="ExternalInput")
    cbf = nc.dram_tensor("cbf", [H1, CF_W], F32, kind="ExternalInput")
    cbb = nc.dram_tensor("cbb", [H1, CBW], BF16, kind="ExternalInput")
    yout = nc.dram_tensor("yout", [4, YC], F32, kind="ExternalOutput")

    with TileContext(nc) as tc:
        with (
            tc.tile_pool(name="consts", bufs=1) as consts,
            tc.tile_pool(name="z2res", bufs=1) as z2res,
            tc.tile_pool(name="accp", bufs=1) as accp,
        ):
            cf = consts.tile([H1, CF_W], F32)
            nc.sync.dma_start(out=cf, in_=cbf[:, :])
            cb = consts.tile([H1, CBW], BF16)
            nc.scalar.dma_start(out=cb, in_=cbb[:, :])

            iota_t = cf[0:116, CF_IOTA:CF_IOTA + 1]
            a1_t = cf[0:H1, CF_A1:CF_A1 + 1]
            c1_t = cf[0:H1, CF_C1:CF_C1 + 1]
            g2_t = cf[0:H2, CF_G2:CF_G2 + 1]
            bt2_t = cf[0:H2, CF_BT2:CF_BT2 + 1]
            g3_t = cf[0:H3, CF_G3:CF_G3 + 1]
            bt3_t = cf[0:H3, CF_BT3:CF_BT3 + 1]
            eps_t = cf[0:H2, CF_EPS:CF_EPS + 1]
            b4_t = cf[0:4, CF_B4:CF_B4 + 1]

            w1_t = cb[0:NU2, CB_W1:CB_W1 + H1]
            w1o_t = cb[ODD:ODD + NU2, CB_W1:CB_W1 + H1]
            w2_t = cb[0:H1, CB_W2:CB_W2 + H2]
            w3_t = cb[0:H1, CB_W3:CB_W3 + H2]
            w4_t = cb[0:H1, CB_W4:CB_W4 + 4]
            ssel_t = cb[0:16, CB_SSEL:CB_SSEL + 116]

            # warmups: absorb const DMA wait into each engine's clock
            with tc.tile_pool(name="warm", bufs=1, space="PSUM") as warmp:
                ps_w = warmp.tile([1, 1], F32)
                nc.tensor.matmul(ps_w, cb[0:1, 0:1], cb[0:1, 0:1],
                                 start=True, stop=True)
            scr_v = consts.tile([1, 1], F32)
            nc.vector.tensor_copy(scr_v, cf[0:1, 0:1])
            scr_s = consts.tile([1, 1], F32)
            nc.scalar.copy(scr_s, cf[0:1, 0:1])
            scr_g = consts.tile([1, 1], BF16)
            nc.gpsimd.tensor_copy(scr_g, cb[0:1, 0:1])

            Z2 = z2res.tile([H1, Z2C], BF16)
            acc2 = accp.tile([H1, NCH, 6], F32)

            dummy_n = [0]

            def pe_keepalive(psd_pool, n=2):
                for _ in range(n):
                    dummy_n[0] += 1
                    psd = psd_pool.tile([1, 512], F32, name=f"dum{dummy_n[0]}",
                                        tag="dum")
                    nc.tensor.matmul(psd[:, :], cb[0:1, 0:1], Z2[0:1, 0:512],
                                     start=True, stop=True, skip_group_check=True)

            # ---------------- pass 1: packed pairs (even chunk at
            # partitions 0:58, odd at 64:122 of one u tile; one broadcast
            # matmul + one is_equal covers both chunks)
            with (
                tc.tile_pool(name="catp", bufs=6) as catp,
                tc.tile_pool(name="up", bufs=4) as up,
                tc.tile_pool(name="h1p", bufs=4) as h1p,
                tc.tile_pool(name="psb", bufs=2, space="PSUM") as psb,
                tc.tile_pool(name="psz1", bufs=2, space="PSUM") as psz1,
                tc.tile_pool(name="psz2", bufs=2, space="PSUM") as psz2,
            ):
                for p in range(NPAIR):
                    pc = p * FD
                    cat_t = catp.tile([16, FD], BF16, tag="cat")
                    nc.sync.dma_start(out=cat_t, in_=catk[:, pc:pc + FD])
                    u_t = up.tile([122, FD], BF16, tag="u")
                    for sl in range(2):
                        pzb = psb.tile([116, 512], F32, tag="b")
                        nc.tensor.matmul(pzb[:, :], ssel_t,
                                         cat_t[:, sl * 512:(sl + 1) * 512],
                                         start=True, stop=True)
                        nc.vector.tensor_scalar(
                            u_t[0:116, sl * 512:(sl + 1) * 512],
                            pzb[:, :], iota_t, None, OP.is_equal)
                    # numerics overwrite the is_equal zeros at [52:58]/[116:122]
                    nc.sync.dma_start(out=u_t[NCAT2:NU2, :],
                                      in_=numk[0:6, pc:pc + FD])
                    nc.sync.dma_start(out=u_t[ODD + NCAT2:ODD + NU2, :],
                                      in_=numk[6:12, pc:pc + FD])
                    pz2s = [psz2.tile([H1, 512], F32, tag="z2",
                                      name=f"z2_{p}_{j}") for j in range(2)]
                    for h in range(2):
                        pz1 = psz1.tile([H1, FD], F32, tag="z1")
                        for j in range(0, FD, 512):
                            nc.tensor.matmul(
                                pz1[:, j:j + 512], w1o_t if h else w1_t,
                                u_t[ODD * h:ODD * h + NU2, j:j + 512],
                                start=True, stop=True)
                        h1_t = h1p.tile([H1, FD], BF16, tag="h1")
                        nc.scalar.activation(out=h1_t, in_=pz1[:, :],
                                             func=AF.Relu, bias=c1_t,
                                             scale=a1_t)
                        for j in range(2):
                            nc.tensor.matmul(
                                pz2s[j][h * H2:h * H2 + H2, :], w2_t,
                                h1_t[:, j * 512:(j + 1) * 512],
                                start=True, stop=True)
                    for j in range(2):
                        zc = slice(p * FD + j * 512, p * FD + (j + 1) * 512)
                        nc.scalar.copy(out=Z2[:, zc], in_=pz2s[j][:, :])
                        nc.vector.bn_stats(out=acc2[:, 2 * p + j, :],
                                           in_=Z2[:, zc])
            # ---------------- stats2 -> a2, coa2, W3p
            sm = accp
            mv2 = sm.tile([H1, 2], F32)
            nc.vector.bn_aggr(out=mv2, in_=acc2[:, :, :])

            sq2 = _small_f32(sm, H1)
            nc.vector.tensor_tensor(out=sq2, in0=mv2[:, 0:1], in1=mv2[:, 0:1],
                                    op=OP.mult)
            st2 = sm.tile([H1, 2], F32)
            nc.vector.tensor_copy(st2[:, 0:1], mv2[:, 0:1])
            nc.vector.tensor_tensor(out=st2[:, 1:2], in0=mv2[:, 1:2], in1=sq2,
                                    op=OP.add)
            hi2 = sm.tile([H2, 2], F32)
            nc.sync.dma_start(out=hi2, in_=st2[H2:H1, :])
            cm2 = sm.tile([H2, 2], F32)
            nc.vector.tensor_tensor(out=cm2, in0=st2[0:H2, :], in1=hi2[:, :],
                                    op=OP.add)
            nc.vector.tensor_scalar(cm2[:, :], cm2[:, :], 0.5, None, OP.mult)
            msq2 = _small_f32(sm, H2)
            nc.vector.tensor_tensor(out=msq2, in0=cm2[:, 0:1], in1=cm2[:, 0:1],
                                    op=OP.mult)
            var2 = _small_f32(sm, H2)
            nc.vector.tensor_tensor(out=var2, in0=cm2[:, 1:2], in1=msq2,
                                    op=OP.subtract)
            sd2 = _small_f32(sm, H2)
            nc.scalar.activation(out=sd2, in_=var2[:, :], func=AF.Sqrt,
                                 bias=eps_t, scale=1.0)
            inv2 = _small_f32(sm, H2)
            nc.vector.reciprocal(out=inv2, in_=sd2[:, :])
            a2v = _small_f32(sm, H2)
            nc.vector.tensor_tensor(out=a2v, in0=inv2[:, :], in1=g2_t,
                                    op=OP.mult)
            ra2 = _small_f32(sm, H2)
            nc.vector.reciprocal(out=ra2, in_=a2v[:, :])
            tb2 = _small_f32(sm, H2)
            nc.vector.tensor_tensor(out=tb2, in0=bt2_t, in1=ra2[:, :],
                                    op=OP.mult)
            coa2 = _small_f32(sm, H2)
            nc.vector.tensor_tensor(out=coa2, in0=tb2[:, :], in1=cm2[:, 0:1],
                                    op=OP.subtract)
            a2rep = _small_f32(sm, H1)
            nc.vector.tensor_copy(a2rep[0:H2, :], a2v[:, :])
            nc.sync.dma_start(out=a2rep[H2:H1, :], in_=a2v[:, :])
            coa2rep = _small_f32(sm, H1)
            nc.vector.tensor_copy(coa2rep[0:H2, :], coa2[:, :])
            nc.sync.dma_start(out=coa2rep[H2:H1, :], in_=coa2[:, :])
            w3p = sm.tile([H1, H2], BF16)
            nc.vector.tensor_scalar(w3p[:, :], w3_t, a2rep, None, OP.mult)

            acc3 = accp.tile([H1, NSLICE // 2, 6], F32)

            # ---------------- pass 2: in-place act2 on Z2, z3 (slice-parity
            # packed into [128,1024] PSUM via partition offsets), stats3
            with (
                tc.tile_pool(name="psz3", bufs=4, space="PSUM") as psz3,
            ):
                for t in range(NSLICE // 4):
                    pz3 = psz3.tile([H1, 1024], F32, tag="z3")
                    a2s = slice((4 * t) * 512, (4 * t + 4) * 512)
                    nc.scalar.activation(out=Z2[:, a2s], in_=Z2[:, a2s],
                                         func=AF.Relu, bias=coa2rep,
                                         scale=1.0)
                    for k in range(4):
                        sl = 4 * t + k
                        zs = slice(sl * 512, (sl + 1) * 512)
                        po = H2 * (k % 2)
                        co = 512 * (k // 2)
                        nc.tensor.matmul(pz3[po:po + H2, co:co + 512],
                                         w3p[:, :], Z2[:, zs],
                                         start=True, stop=True)
                    for k in range(2):
                        nc.vector.bn_stats(out=acc3[:, 2 * t + k, :],
                                           in_=pz3[:, k * 512:(k + 1) * 512])

            # ---------------- stats3 -> a3, coa3, W4p
            mv3 = sm.tile([H1, 2], F32)
            nc.vector.bn_aggr(out=mv3, in_=acc3[:, :, :])
            sq3 = _small_f32(sm, H1)
            nc.vector.tensor_tensor(out=sq3, in0=mv3[:, 0:1], in1=mv3[:, 0:1],
                                    op=OP.mult)
            st3 = sm.tile([H1, 2], F32)
            nc.vector.tensor_copy(st3[:, 0:1], mv3[:, 0:1])
            nc.vector.tensor_tensor(out=st3[:, 1:2], in0=mv3[:, 1:2], in1=sq3,
                                    op=OP.add)
            hi3b = sm.tile([H2, 2], F32)
            nc.sync.dma_start(out=hi3b, in_=st3[H2:H1, :])
            cmA = sm.tile([H2, 2], F32)
            nc.vector.tensor_tensor(out=cmA, in0=st3[0:H2, :], in1=hi3b[:, :],
                                    op=OP.add)
            hi3 = sm.tile([H3, 2], F32)
            nc.sync.dma_start(out=hi3, in_=cmA[H3:H2, :])
            cm3 = sm.tile([H3, 2], F32)
            nc.vector.tensor_tensor(out=cm3, in0=cmA[0:H3, :], in1=hi3[:, :],
                                    op=OP.add)
            nc.vector.tensor_scalar(cm3[:, :], cm3[:, :], 0.25, None, OP.mult)
            msq3 = _small_f32(sm, H3)
            nc.vector.tensor_tensor(out=msq3, in0=cm3[:, 0:1], in1=cm3[:, 0:1],
                                    op=OP.mult)
            var3 = _small_f32(sm, H3)
            nc.vector.tensor_tensor(out=var3, in0=cm3[:, 1:2], in1=msq3,
                                    op=OP.subtract)
            sd3 = _small_f32(sm, H3)
            nc.scalar.activation(out=sd3, in_=var3[:, :], func=AF.Sqrt,
                                 bias=eps_t[0:H3, :], scale=1.0)
            inv3 = _small_f32(sm, H3)
            nc.vector.reciprocal(out=inv3, in_=sd3[:, :])
            a3v = _small_f32(sm, H3)
            nc.vector.tensor_tensor(out=a3v, in0=inv3[:, :], in1=g3_t,
                                    op=OP.mult)
            ra3 = _small_f32(sm, H3)
            nc.vector.reciprocal(out=ra3, in_=a3v[:, :])
            tb3 = _small_f32(sm, H3)
            nc.vector.tensor_tensor(out=tb3, in0=bt3_t, in1=ra3[:, :],
                                    op=OP.mult)
            coa3 = _small_f32(sm, H3)
            nc.vector.tensor_tensor(out=coa3, in0=tb3[:, :], in1=cm3[:, 0:1],
                                    op=OP.subtract)
            a3rep = _small_f32(sm, H1)
            nc.vector.tensor_copy(a3rep[0:H3, :], a3v[:, :])
            for g in range(1, 4):
                nc.sync.dma_start(out=a3rep[g * H3:(g + 1) * H3, :],
                                  in_=a3v[:, :])
            coa3rep = _small_f32(sm, H1)
            nc.vector.tensor_copy(coa3rep[0:H3, :], coa3[:, :])
            for g in range(1, 4):
                nc.sync.dma_start(out=coa3rep[g * H3:(g + 1) * H3, :],
                                  in_=coa3[:, :])
            w4p = sm.tile([H1, 4], BF16)
            nc.vector.tensor_scalar(w4p[:, :], w4_t, a3rep, None, OP.mult)

            # ---------------- pass 3: recompute z3, act3, z4, sigmoid, out
            with (
                tc.tile_pool(name="h3p", bufs=3) as h3p,
                tc.tile_pool(name="yp", bufs=2) as yp,
                tc.tile_pool(name="psz34", bufs=2, space="PSUM") as psz34,
                tc.tile_pool(name="psz4", bufs=2, space="PSUM") as psz4,
            ):
                for t4 in range(NSLICE // 4):
                    pz4 = psz4.tile([4, 1024], F32, tag="z4")
                    pz34 = psz34.tile([H1, 1024], F32, tag="z34")
                    for t in range(t4 * 2, t4 * 2 + 2):
                        co = 512 * (t % 2)
                        for sp in range(2):
                            s = 2 * t + sp
                            zs = slice(s * 512, (s + 1) * 512)
                            nc.tensor.matmul(
                                pz34[sp * H2:(sp + 1) * H2, co:co + 512],
                                w3p[:, :], Z2[:, zs],
                                start=True, stop=True)
                    h3_t = h3p.tile([H1, 1024], BF16, tag="h3")
                    nc.scalar.activation(out=h3_t, in_=pz34[:, :],
                                         func=AF.Relu, bias=coa3rep,
                                         scale=1.0)
                    for k in range(2):
                        nc.tensor.matmul(pz4[:, k * 512:(k + 1) * 512],
                                         w4p[:, :],
                                         h3_t[:, k * 512:(k + 1) * 512],
                                         start=True, stop=True)
                    y_t = yp.tile([4, 1024], F32, tag="y")
                    nc.scalar.activation(out=y_t, in_=pz4[:, :],
                                         func=AF.Sigmoid, bias=b4_t, scale=1.0)
                    nc.sync.dma_start(out=yout[:, t4 * 1024:(t4 + 1) * 1024],
                                      in_=y_t)
    return nc


def _get_program():
    if "F" not in _cache:
        _cache["F"] = build_fused()
    return _cache["F"]


def _run(prog, in_maps, cores, label):
    tr = bool(os.environ.get("BASS_KERNEL_TRACE"))
    r = run_bass_kernel_spmd(prog, in_maps, cores, trace=tr)
    if tr and r.exec_time_ns:
        _cache["hw_exec_ns"] = _cache.get("hw_exec_ns", 0) + r.exec_time_ns
        _cache[f"ns_{label}"] = r.exec_time_ns
        if r.instructions_and_trace:
            _cache[f"trace_{label}"] = r.instructions_and_trace[1]
    return r.results


def kernel(**inputs):
    inp = {k: np.asarray(v) for k, v in inputs.items()}
    cores = list(range(N_CORES))
    _cache.pop("hw_exec_ns", None)

    W1eff64 = _build_w1eff(inp["breed_emb"], inp["temp_emb"], inp["W1"])

    cats_all = [inp["pet1_breed"], inp["pet1_size"], inp["pet1_energy"],
                inp["pet1_temp"], inp["pet2_breed"], inp["pet2_size"],
                inp["pet2_energy"], inp["pet2_temp"]]
    nums_all = [inp["pet1_age"] / 15.0, inp["pet1_social"],
                inp["pet1_weight"] / 100.0, inp["pet2_age"] / 15.0,
                inp["pet2_social"], inp["pet2_weight"] / 100.0]
    nums_all = [np.asarray(x, np.float32) for x in nums_all]

    mu1, var1 = _host_stats1(cats_all, [x.astype(np.float64) for x in nums_all],
                             W1eff64)
    a1 = (np.asarray(inp["gamma1"], np.float64) / np.sqrt(var1 + EPS))
    c1 = (np.asarray(inp["beta1"], np.float64) - a1 * mu1)

    # recenter: z1 = b* + W1p^T u' with u' dropping each stream's last dim
    bstar = np.zeros(H1, np.float64)
    W1p = np.zeros((NU2, H1), np.float64)
    for i in range(8):
        o, k = CAT_OFFS[i], CAT_SIZES[i]
        o2, k2 = CAT2_OFFS[i], CAT2_SIZES[i]
        last = W1eff64[o + k - 1]
        bstar += last
        W1p[o2:o2 + k2] = W1eff64[o:o + k - 1] - last
    W1p[NCAT2:NU2] = W1eff64[NCAT:NU]
    c1p = c1 + a1 * bstar

    # packed selector [16, 116] and iota [122,1]: even chunk one-hot at
    # partitions 0:52 (+num 52:58), odd at 64:116 (+num 116:122)
    s_sel2 = np.zeros((16, 116), np.float32)
    iota2 = np.full((122, 1), 255.0, np.float32)
    for i in range(8):
        o2, k2 = CAT2_OFFS[i], CAT2_SIZES[i]
        s_sel2[i, o2:o2 + k2] = 1.0
        s_sel2[8 + i, ODD + o2:ODD + o2 + k2] = 1.0
        iota2[o2:o2 + k2, 0] = np.arange(k2, dtype=np.float32)
        iota2[ODD + o2:ODD + o2 + k2, 0] = np.arange(k2, dtype=np.float32)

    cbf = np.zeros((H1, CF_W), np.float32)
    cbf[0:122, CF_IOTA] = iota2[:, 0]
    cbf[0:H1, CF_A1] = a1.astype(np.float32)
    cbf[0:H1, CF_C1] = c1p.astype(np.float32)
    cbf[0:H2, CF_G2] = np.asarray(inp["gamma2"], np.float32)
    cbf[0:H2, CF_BT2] = np.asarray(inp["beta2"], np.float32)
    cbf[0:H3, CF_G3] = np.asarray(inp["gamma3"], np.float32)
    cbf[0:H3, CF_BT3] = np.asarray(inp["beta3"], np.float32)
    cbf[0:H2, CF_EPS] = EPS
    cbf[0:4, CF_B4] = float(np.asarray(inp["b4"]).reshape(-1)[0])

    W3blk = np.zeros((H1, H2), np.float32)
    W3 = np.asarray(inp["W3"], np.float32)
    W3blk[0:H2, 0:H3] = W3
    W3blk[H2:H1, H3:H2] = W3
    W4blk = np.zeros((H1, 4), np.float32)
    w4 = np.asarray(inp["W4"], np.float32)[:, 0]
    for g in range(4):
        W4blk[g * H3:(g + 1) * H3, g] = w4

    cbb = np.zeros((H1, CBW), np.float32)
    cbb[0:NU2, CB_W1:CB_W1 + H1] = W1p.astype(np.float32)
    cbb[ODD:ODD + NU2, CB_W1:CB_W1 + H1] = W1p.astype(np.float32)
    cbb[0:H1, CB_W2:CB_W2 + H2] = np.asarray(inp["W2"], np.float32)
    cbb[0:H1, CB_W3:CB_W3 + H2] = W3blk
    cbb[0:H1, CB_W4:CB_W4 + 4] = W4blk
    cbb[0:16, CB_SSEL:CB_SSEL + 116] = s_sel2
    cbb = cbb.astype(BF)

    prog = _get_program()

    cat_mat = np.stack(cats_all).astype(BF)       # [8, B]
    num_mat = np.stack(nums_all).astype(BF)       # [6, B]

    in_maps = []
    for c in cores:
        sl = slice(c * SHARD, (c + 1) * SHARD)
        # pack pairs: row s+8h = stream s of chunk-parity h, col p*FD+q
        cm = cat_mat[:, sl].reshape(8, NPAIR, 2, FD)
        cat2 = np.ascontiguousarray(
            cm.transpose(2, 0, 1, 3).reshape(16, SHARD // 2))
        nm = num_mat[:, sl].reshape(6, NPAIR, 2, FD)
        num2 = np.ascontiguousarray(
            nm.transpose(2, 0, 1, 3).reshape(12, SHARD // 2))
        in_maps.append({
            "catk": cat2, "numk": num2,
            "cbf": cbf, "cbb": cbb,
        })
    res = _run(prog, in_maps, cores, "F")

    perm = _cache.get("perm")
    if perm is None:
        perm = _out_perm()
        _cache["perm"] = perm
    out = np.empty(B, np.float32)
    for c in cores:
        out[c * SHARD:(c + 1) * SHARD] = res[c]["yout"].reshape(-1)[perm]
    return out
